# revision 52
# baseline (speedup 1.0000x reference)
"""TRN2 Bass kernel: DotProductAttentionTransformer (MD17-style GNN), 8-core SPMD.

Self-contained: host preprocessing (edge sorting/padding, selector matrices,
weight relayout) + Bass/Tile device program (edge-parallel attention with
S-matmul scatter/gather, bf16 GEMMs, fp32 softmax/LN).

v2: single merged k|v gather per edge chunk, spherical-harmonic logits folded
into the q side (Wqsh = Wq_h @ Wsh_h^T), fused multiply-reduce logits,
post-normalized aggregation, engine-balanced PSUM evacuations, and k/v-first
GEMM ordering so the AllGather overlaps q-side work.
"""
import math
import numpy as np
import ml_dtypes

import concourse.bass as bass
import concourse.mybir as mybir
import concourse.tile as tile_mod
from concourse.tile import TileContext
from concourse.masks import make_identity
from concourse.vector_clock import ScopedClock
from concourse.bass_utils import run_bass_kernel_spmd

bf16 = ml_dtypes.bfloat16

N, E, G, D, H, L = 10000, 160000, 64, 480, 4, 6
DH, NB, SH = 120, 128, 9
CUTOFF = 5.0
AVG_DEG = 15.57930850982666
AVG_NODES = 18.03065905448718
NC = 8
NPC = N // NC
NBLK = 10
DP = 512
FF = 1024
QSW = 64          # 4 heads x 16 (9 used) qsh columns
ONE_BF = np.float32(1.0).astype(bf16)

# ---------------------------------------------------------------------------
# harness patches: this walrus build allows only ONE sync-wait per
# instruction; split extras onto same-engine NoOps.
# ---------------------------------------------------------------------------

def _patched_drain_and_barrier(self, tick_clock, wait_clock):
    nc = self.nc
    drain_inst = nc.sync.drain()
    wait_clock.add_sem_waits(drain_inst.ins,
                             ScopedClock({None: tick_clock.global_clock}))
    si = drain_inst.ins.sync_info
    waits = list(si.on_wait or []) if si is not None else []
    if len(waits) > 1:
        id2sem = {h.num: h for h in self.sems.allocated().values()}
        si.on_wait = [waits[0]]
        for w in waits[1:]:
            nop = nc.sync.nop(nofuse=True)
            nop.wait_op(id2sem[w.id], w.wait_value, "sem-ge")
    nc.all_engine_barrier()
    popped = nc._tile_sem_poison_stack.pop()
    assert popped is self._sem_poison
    nc.clear_and_free_semaphores(list(self.sems.allocated().values()))
    nc.all_engine_barrier()


tile_mod.TileContext._drain_and_barrier = _patched_drain_and_barrier

_waitnop_counter = [0]


def split_multi_waits(nc):
    for f in nc.m.functions:
        for bb in f.blocks:
            insts = bb.instructions
            if not any(i.sync_info is not None and i.sync_info.on_wait
                       and len(i.sync_info.on_wait) > 1 for i in insts):
                continue
            new = []
            for inst in insts:
                si = inst.sync_info
                if si is not None and si.on_wait and len(si.on_wait) > 1:
                    waits = list(si.on_wait)
                    for w in waits[:-1]:
                        _waitnop_counter[0] += 1
                        nop = mybir.InstNoOp(
                            name=f"waitnop-{_waitnop_counter[0]}", ins=[], outs=[])
                        nop.engine = inst.engine
                        nop.sync_info = mybir.SyncInfo(on_wait=[w], on_update=[])
                        new.append(nop)
                    si.on_wait = [waits[-1]]
                new.append(inst)
            bb.instructions = new
    return nc


F32 = mybir.dt.float32
BF = mybir.dt.bfloat16
I32 = mybir.dt.int32
AX = mybir.AxisListType.X
OP = mybir.AluOpType
AF = mybir.ActivationFunctionType
INV = 1.0 / math.sqrt(DH)
CDEG = 1.0 / math.sqrt(AVG_DEG)
WIDTH = CUTOFF / NB

# packed layer-weight column offsets (bf16 [128, WCOLS])
OQ = 0
OK_ = OQ + 4 * DP
OV = OK_ + 4 * DP
OQS = OV + 4 * DP
OO = OQS + 4 * QSW
OF1 = OO + 4 * DP
OF2 = OF1 + 4 * FF
WCOLS = OF2 + 8 * DP


def head_pad_cols(W):
    """[in, 480] -> [in, 512]: head h cols 120h:120h+120 -> 128h:128h+120, pad zeros."""
    out = np.zeros((W.shape[0], DP), W.dtype)
    for h in range(H):
        out[:, 128 * h:128 * h + DH] = W[:, DH * h:DH * (h + 1)]
    return out


def plain_pad(W, rows, cols):
    out = np.zeros((rows, cols), W.dtype)
    out[:W.shape[0], :W.shape[1]] = W
    return out


def _amaj(W, a):
    """[a*128, m] -> [128, a*m] partition-major relayout for lhsT tiles."""
    return np.ascontiguousarray(
        W.reshape(a, 128, -1).transpose(1, 0, 2).reshape(128, -1))


def preprocess(inputs):
    """Returns (shared, per_core, CBLK) host arrays. Integer/relayout work only."""
    src = np.asarray(inputs["edge_src"]).astype(np.int64)
    dst = np.asarray(inputs["edge_dst"]).astype(np.int64)
    batch = np.asarray(inputs["batch"]).astype(np.int64)

    order = np.argsort(dst, kind="stable")
    dsts, srcs = dst[order], src[order]

    # per (core, block) edge lists
    per_block = [[[] for _ in range(NBLK)] for _ in range(NC)]
    core_of = dsts // NPC
    loc = dsts - core_of * NPC
    blk = loc // 128
    for i in range(E):
        per_block[core_of[i]][blk[i]].append(i)

    CBLK = 0
    for c in range(NC):
        for b in range(NBLK):
            CBLK = max(CBLK, (len(per_block[c][b]) + 127) // 128)
    C = NBLK * CBLK

    per_core = []
    for c in range(NC):
        src_idx = np.zeros((NBLK, CBLK, 128), np.int64)
        dst_glob = np.zeros((NBLK, CBLK, 128), np.int64)
        dst_local = np.full((NBLK, CBLK, 128), -1, np.int64)
        for b in range(NBLK):
            el = per_block[c][b]
            for j, i in enumerate(el):
                ch, p = j // 128, j % 128
                src_idx[b, ch, p] = srcs[i]
                dst_local[b, ch, p] = loc[i] - 128 * b
                dst_glob[b, ch, p] = dsts[i]
        # S [e, n] and S_T [n, e] per chunk, bf16 {0,1}
        iota = np.arange(128)
        S = (dst_local[..., None] == iota[None, None, None, :]).astype(bf16)
        ST = np.ascontiguousarray(np.swapaxes(S, 2, 3))
        # pad edges: point ST/dst at the block's max-in-degree node so the
        # expanded q values stay finite (S stays zero -> no contribution).
        for b in range(NBLK):
            deg_b = np.zeros(128, np.int64)
            for ch in range(CBLK):
                vals = dst_local[b, ch]
                np.add.at(deg_b, vals[vals >= 0], 1)
            assert deg_b.max() > 0, f"block {b} of core {c} has no edges"
            nmax = int(deg_b.argmax())
            for ch in range(CBLK):
                padmask = dst_local[b, ch] < 0
                ST[b, ch, nmax, padmask] = ONE_BF
                dst_glob[b, ch][padmask] = c * NPC + 128 * b + nmax
        # combined [ST | S] per chunk: [128, C*256]
        SST = np.empty((128, C * 256), bf16)
        for b in range(NBLK):
            for ch in range(CBLK):
                cc = b * CBLK + ch
                SST[:, cc * 256:cc * 256 + 128] = ST[b, ch]
                SST[:, cc * 256 + 128:(cc + 1) * 256] = S[b, ch]
        idxT = np.ascontiguousarray(
            src_idx.reshape(C, 128).T).astype(np.int32)
        # host-gathered per-edge endpoint positions [128, C*4] (data movement
        # only; subtraction happens on device)
        pos4 = plain_pad(np.asarray(inputs["pos"]).astype(np.float32), N, 4)
        pes = np.ascontiguousarray(
            pos4[src_idx.reshape(C, 128)].transpose(1, 0, 2).reshape(128, C * 4))
        ped = np.ascontiguousarray(
            pos4[dst_glob.reshape(C, 128)].transpose(1, 0, 2).reshape(128, C * 4))
        # graph one-hot for energy: [NBLK*128, G]
        Sg = np.zeros((NBLK * 128, G), np.float32)
        for nl in range(NPC):
            Sg[nl, batch[c * NPC + nl]] = 1.0
        per_core.append(dict(SST=SST, idxT=idxT, pes=pes, ped=ped, Sg=Sg))

    f32 = np.float32
    i = {k: np.asarray(v) for k, v in inputs.items()}

    # per-layer packed weights [L, 128, WCOLS]
    Wq = i["Wq"].astype(f32)
    Wk = i["Wk"].astype(f32)
    Wv = i["Wv"].astype(f32)
    Wsh = i["Wsh"].astype(f32)
    Wo = i["Wo"].astype(f32)
    Wf1 = i["Wf1"].astype(f32)
    Wf2 = i["Wf2"].astype(f32)
    Wall = np.zeros((L, 128, WCOLS), bf16)
    for l in range(L):
        Wqsh = np.zeros((DP, QSW), f32)
        for h in range(H):
            wq_h = Wq[l][:, DH * h:DH * (h + 1)]
            wsh_h = Wsh[l][:, DH * h:DH * (h + 1)]
            Wqsh[:D, 16 * h:16 * h + SH] = wq_h @ wsh_h.T
        parts = [
            _amaj(plain_pad(head_pad_cols(Wq[l]), DP, DP).astype(bf16), 4),
            _amaj(plain_pad(head_pad_cols(Wk[l]), DP, DP).astype(bf16), 4),
            _amaj(plain_pad(head_pad_cols(Wv[l]), DP, DP).astype(bf16), 4),
            _amaj(Wqsh.astype(bf16), 4),
            _amaj(plain_pad(_head_rows(Wo[l]), DP, DP).astype(bf16), 4),
            _amaj(plain_pad(Wf1[l], DP, FF).astype(bf16), 4),
            _amaj(plain_pad(Wf2[l], FF, DP).astype(bf16), 8),
        ]
        Wall[l] = np.concatenate(parts, axis=1)

    shared = dict(
        pos_pad=plain_pad(i["pos"].astype(f32), N, 64),
        atom_pad=plain_pad(i["atom_table"].astype(f32), 64, DP).astype(bf16),
        node_atom=i["node_atom"].astype(np.int32),
        wdeg16=plain_pad(i["Wdeg"].astype(f32), 16, DP).astype(bf16),
        Wd1=i["Wd1"].astype(bf16), Wd2=i["Wd2"].astype(bf16),
        Wd3=plain_pad(i["Wd3"].astype(f32), 64, 4).astype(bf16),
        W1=i["W1"].astype(bf16), W2=i["W2"].astype(bf16), W3=i["W3"].astype(bf16),
        Wall=Wall,
        Wh1=plain_pad(i["Wh1"].astype(f32), DP, DP).astype(bf16),
        Wh2=plain_pad(i["Wh2"].astype(f32), DP, 4).astype(bf16),
        centers=np.linspace(0, CUTOFF, NB).astype(f32),
    )
    return shared, per_core, CBLK


def _head_rows(W):
    """[480, m] -> [512, m]: head h rows 120h:120h+120 -> 128h:128h+120."""
    out = np.zeros((DP, W.shape[1]), W.dtype)
    for h in range(H):
        out[128 * h:128 * h + DH, :] = W[DH * h:DH * (h + 1), :]
    return out


def make_inmaps(inputs, shared=None, per_core=None, CBLK=None):
    if shared is None:
        shared, per_core, CBLK = preprocess(inputs)
    i32, f32 = np.int32, np.float32
    cenrep = np.broadcast_to(shared["centers"][None, :], (128, NB)).copy()
    na = shared["node_atom"]
    in_maps = []
    for c in range(NC):
        pc = per_core[c]
        naT = np.zeros((128, NBLK), i32)
        na_loc = np.zeros(NBLK * 128, i32)
        na_loc[:NPC] = na[c * NPC:(c + 1) * NPC]
        naT[:] = na_loc.reshape(NBLK, 128).T
        m = dict(
            atom_pad=shared["atom_pad"],
            idxT=pc["idxT"], naT=naT,
            pes=pc["pes"], ped=pc["ped"],
            SST=pc["SST"], Sg=pc["Sg"],
            cenrep=cenrep,
            wdeg16=shared["wdeg16"],
            Wd1=shared["Wd1"], Wd2=shared["Wd2"], Wd3=shared["Wd3"],
            W1=shared["W1"], W2=shared["W2"], W3=shared["W3"],
            Wall=shared["Wall"],
            Wh1=shared["Wh1"], Wh2=shared["Wh2"],
        )
        in_maps.append(m)
    return in_maps, CBLK


def build(CBLK, n_layers=L, n_blocks=NBLK, debug_dumps=()):
    C = n_blocks * CBLK
    nc = bass.Bass("TRN2")
    dt = {}

    def inp(name, shape, dtype):
        dt[name] = nc.dram_tensor(name, shape, dtype, kind="ExternalInput")
        return dt[name]

    inp("atom_pad", [64, DP], BF)
    inp("idxT", [128, C], I32)
    inp("pes", [128, C * 4], F32)
    inp("ped", [128, C * 4], F32)
    inp("naT", [128, NBLK], I32)
    inp("SST", [128, C * 256], BF)
    inp("Sg", [NBLK * 128, G], F32)
    inp("cenrep", [128, NB], F32)
    inp("wdeg16", [16, DP], BF)
    inp("Wd1", [NB, 64], BF)
    inp("Wd2", [64, 64], BF)
    inp("Wd3", [64, 4], BF)
    inp("W1", [L, NB, 64], BF)
    inp("W2", [L, 64, 64], BF)
    inp("W3", [L, 64, H], BF)
    inp("Wall", [L, 128, WCOLS], BF)
    inp("Wh1", [DP, DP], BF)
    inp("Wh2", [DP, 4], BF)

    energy_out = nc.dram_tensor("energy", [1, G], F32, kind="ExternalOutput")
    dumps = {}
    if "x" in debug_dumps:
        dumps["x"] = nc.dram_tensor("x_dump", [128, NBLK * DP], F32,
                                    kind="ExternalOutput")

    RG = [list(range(NC))]

    with TileContext(nc) as tc:
        with (
            tc.tile_pool(name="cst", bufs=1) as cst,
            tc.tile_pool(name="big", bufs=1) as big,
            tc.tile_pool(name="wp", bufs=1) as wp,
            tc.tile_pool(name="wpa", bufs=2) as wpa,
            tc.tile_pool(name="gp", bufs=1) as gp,
            tc.tile_pool(name="dram", bufs=1, space="DRAM") as dram,
        ):
            # ---------------- constants ----------------
            ident = cst.tile([128, 128], BF, tag="ident")
            make_identity(nc, ident[:])
            identf = cst.tile([128, 128], F32, tag="identf")
            make_identity(nc, identf[:])
            eps5 = cst.tile([128, 1], F32, tag="eps5")
            nc.vector.memset(eps5[:], 1e-5)
            cenrep = cst.tile([128, NB], F32, tag="cenrep")
            nc.sync.dma_start(out=cenrep[:], in_=dt["cenrep"][:])
            idxT_t = cst.tile([128, C], I32, tag="idxT")
            nc.sync.dma_start(out=idxT_t[:], in_=dt["idxT"][:])
            naT_t = cst.tile([128, NBLK], I32, tag="naT")
            nc.sync.dma_start(out=naT_t[:], in_=dt["naT"][:])
            wd1 = cst.tile([NB, 64], BF, tag="wd1")
            nc.sync.dma_start(out=wd1[:], in_=dt["Wd1"][:])
            wd2 = cst.tile([64, 64], BF, tag="wd2")
            nc.sync.dma_start(out=wd2[:], in_=dt["Wd2"][:])
            wd3 = cst.tile([64, 4], BF, tag="wd3")
            nc.sync.dma_start(out=wd3[:], in_=dt["Wd3"][:])
            w1g = cst.tile([NB, L * 64], BF, tag="w1g")
            nc.sync.dma_start(out=w1g[:].rearrange("k (l m) -> k l m", l=L),
                in_=dt["W1"].ap().rearrange("l k m -> k l m"))
            w2g = cst.tile([64, L * 64], BF, tag="w2g")
            nc.sync.dma_start(out=w2g[:].rearrange("k (l m) -> k l m", l=L),
                in_=dt["W2"].ap().rearrange("l k m -> k l m"))
            w3g = cst.tile([64, L * H], BF, tag="w3g")
            nc.sync.dma_start(out=w3g[:].rearrange("k (l m) -> k l m", l=L),
                in_=dt["W3"].ap().rearrange("l k m -> k l m"))
            wdeg16 = cst.tile([16, DP], BF, tag="wdeg16")
            nc.sync.dma_start(out=wdeg16[:], in_=dt["wdeg16"][:])
            # ---------------- persistent state ----------------
            x_t = big.tile([128, NBLK * DP], F32, tag="x")
            nc.vector.memset(x_t[:], 0.0)
            xT_t = big.tile([128, NBLK * DP], BF, tag="xT")
            q_t = big.tile([128, NBLK * DP], BF, tag="q")
            qsh_t = big.tile([128, NBLK * QSW], BF, tag="qsh")
            g0_t = big.tile([128, C], F32, tag="g0")
            rr_t = big.tile([128, C], F32, tag="rr")
            shpad = big.tile([128, C * 16], BF, tag="shpad")

            kvloc_d = dram.tile([NPC, 2 * DP], BF, tag="kvloc")
            rbf_d = dram.tile([128, C * 128], BF, tag="rbf_d")
            kvfull_d = nc.dram_tensor("kvfull_sh", [N, 2 * DP], BF,
                                      addr_space="Shared")
            eng_in_d = dram.tile([1, G], F32, tag="eng_in")
            eng_out_d = nc.dram_tensor("engout_sh", [1, G], F32,
                                       addr_space="Shared")

            # ============ PHASE 1: geometry ============
            with (
                tc.tile_pool(name="geo", bufs=1) as geo,
            ):
                shE = geo.tile([128, C * 12], F32, tag="shE")
                sh3 = shE[:].rearrange("p (c f) -> p c f", f=12)
                evi = geo.tile([128, C * 4], F32, tag="evi")
                ev3 = evi[:].rearrange("p (c f) -> p c f", f=4)
                tmp = geo.tile([128, C * 4], F32, tag="evtmp")
                tmp3 = tmp[:].rearrange("p (c f) -> p c f", f=4)
                uu = geo.tile([128, C * 3], F32, tag="uu")
                u3 = uu[:].rearrange("p (c f) -> p c f", f=3)
                rinv = geo.tile([128, C], F32, tag="rinv")

                pes_t = geo.tile([128, C * 4], F32, tag="pes")
                nc.sync.dma_start(out=pes_t[:], in_=dt["pes"][:])
                ped_t = geo.tile([128, C * 4], F32, tag="ped")
                nc.sync.dma_start(out=ped_t[:], in_=dt["ped"][:])
                nc.vector.tensor_tensor(out=evi[:], in0=pes_t[:], in1=ped_t[:],
                                        op=OP.subtract)
                nc.vector.tensor_tensor(out=tmp[:], in0=evi[:], in1=evi[:],
                                        op=OP.mult)
                nc.vector.tensor_reduce(out=ev3[:, :, 3:4], in_=tmp3[:, :, 0:3],
                                        op=OP.add, axis=AX)
                nc.scalar.activation(out=rr_t[:],
                                     in_=ev3[:, :, 3:4].rearrange("p c o -> p (c o)"),
                                     func=AF.Sqrt)
                radd = geo.tile([128, C], F32, tag="radd")
                nc.vector.tensor_scalar(out=radd[:], in0=rr_t[:], scalar1=1e-12,
                                        scalar2=None, op0=OP.add)
                nc.vector.reciprocal(out=rinv[:], in_=radd[:])
                nc.vector.tensor_tensor(
                    out=u3[:, :, 0:3], in0=ev3[:, :, 0:3],
                    in1=rinv[:].rearrange("p (c o) -> p c o", o=1).to_broadcast(
                        [128, C, 3]),
                    op=OP.mult)
                s3c, s5c, s15c = math.sqrt(3.0), math.sqrt(5.0), math.sqrt(15.0)
                nc.vector.memset(shE[:], 0.0)
                nc.vector.memset(sh3[:, :, 0:1].rearrange("p c o -> p (c o)"), 1.0)
                nc.vector.tensor_scalar(out=sh3[:, :, 1:4], in0=u3[:, :, 0:3],
                                        scalar1=s3c, scalar2=None, op0=OP.mult)
                nc.vector.scalar_tensor_tensor(out=sh3[:, :, 4:6], in0=u3[:, :, 0:2],
                                               scalar=s15c, in1=u3[:, :, 1:3],
                                               op0=OP.mult, op1=OP.mult)
                nc.vector.tensor_tensor(out=tmp3[:, :, 0:3], in0=u3[:, :, 0:3],
                                        in1=u3[:, :, 0:3], op=OP.mult)
                nc.vector.tensor_scalar(out=sh3[:, :, 6:7], in0=tmp3[:, :, 2:3],
                                        scalar1=1.5 * s5c, scalar2=-0.5 * s5c,
                                        op0=OP.mult, op1=OP.add)
                nc.vector.scalar_tensor_tensor(out=sh3[:, :, 7:8], in0=u3[:, :, 0:1],
                                               scalar=s15c, in1=u3[:, :, 2:3],
                                               op0=OP.mult, op1=OP.mult)
                nc.vector.tensor_tensor(out=sh3[:, :, 8:9], in0=tmp3[:, :, 0:1],
                                        in1=tmp3[:, :, 1:2], op=OP.subtract)
                nc.vector.tensor_scalar(
                    out=sh3[:, :, 8:9], in0=sh3[:, :, 8:9],
                    scalar1=0.5 * s15c, scalar2=None, op0=OP.mult)
                # shpad [128, C*16]: sh (9 comps) at cols cc*16+s, zero pad
                nc.vector.memset(shpad[:], 0.0)
                nc.scalar.copy(
                    out=shpad[:].rearrange("p (c w) -> p c w", w=16)[:, :, 0:SH],
                    in_=sh3[:, :, 0:SH])

            # ============ PHASE 2: rbf + gate MLPs ============
            with (
                tc.tile_pool(name="rw", bufs=4) as rw,
                tc.tile_pool(name="rw2", bufs=2) as rw2,
                tc.tile_pool(name="rps", bufs=2, space="PSUM") as rps,
                tc.tile_pool(name="rps2", bufs=2, space="PSUM") as rps2,
            ):
                for c0 in range(0, C, 4):
                    nb4 = min(4, C - c0)
                    rbfT = rw.tile([128, 4 * 128], BF, tag="rbfT")
                    for j in range(nb4):
                        cc = c0 + j
                        z = rw.tile([128, NB], F32, tag="z")
                        nc.vector.tensor_scalar(out=z[:], in0=cenrep[:],
                                                scalar1=rr_t[:, cc:cc + 1],
                                                scalar2=1.0 / WIDTH,
                                                op0=OP.subtract, op1=OP.mult)
                        z2 = rw.tile([128, NB], F32, tag="z2")
                        nc.vector.tensor_tensor(out=z2[:], in0=z[:], in1=z[:],
                                                op=OP.mult)
                        rbfe = rw.tile([128, NB], BF, tag="rbfe")
                        nc.scalar.activation(out=rbfe[:], in_=z2[:], func=AF.Exp,
                                             scale=-1.0)
                        rps_t = rps.tile([128, 128], BF, tag="rbf_ps")
                        nc.tensor.transpose(out=rps_t[:], in_=rbfe[:],
                                            identity=ident[:])
                        nc.vector.tensor_copy(out=rbfT[:, j * 128:(j + 1) * 128],
                                              in_=rps_t[:])
                    nc.sync.dma_start(out=rbf_d[:, c0 * 128:(c0 + nb4) * 128],
                                      in_=rbfT[:, 0:nb4 * 128])
                    h1ps = rps.tile([64, 4 * 128], F32, tag="h1ps")
                    nc.tensor.matmul(h1ps[:, 0:nb4 * 128], lhsT=wd1[:],
                                     rhs=rbfT[:, 0:nb4 * 128], start=True, stop=True)
                    h1sb = rw.tile([64, 4 * 128], BF, tag="h1sb")
                    nc.scalar.activation(out=h1sb[:, 0:nb4 * 128],
                                         in_=h1ps[:, 0:nb4 * 128], func=AF.Silu)
                    h2ps = rps.tile([64, 4 * 128], F32, tag="h2ps")
                    nc.tensor.matmul(h2ps[:, 0:nb4 * 128], lhsT=wd2[:],
                                     rhs=h1sb[:, 0:nb4 * 128], start=True, stop=True)
                    h2sb = rw2.tile([64, 4 * 128], BF, tag="h2sb")
                    nc.scalar.activation(out=h2sb[:, 0:nb4 * 128],
                                         in_=h2ps[:, 0:nb4 * 128], func=AF.Silu)
                    gps_o = rps2.tile([128, 16], F32, tag="gate_ps")
                    for j in range(nb4):
                        nc.tensor.matmul(
                            gps_o[:, j * 4:j * 4 + 4],
                            lhsT=h2sb[:, j * 128:(j + 1) * 128],
                            rhs=wd3[:], start=True, stop=True)
                    for j in range(nb4):
                        cc = c0 + j
                        nc.scalar.copy(out=g0_t[:, cc:cc + 1],
                                       in_=gps_o[:, j * 4:j * 4 + 1])

            # ============ PHASE 3: x0 + deg embedding ============
            with (
                tc.tile_pool(name="dw", bufs=3) as dw,
                tc.tile_pool(name="dw2", bufs=2) as dw2,
                tc.tile_pool(name="dps", bufs=2, space="PSUM") as dps,
                tc.tile_pool(name="dpsD", bufs=1, space="PSUM") as dpsD,
            ):
                for b in range(n_blocks):
                    sst = dw2.tile([128, CBLK * 256], BF, tag="sst")
                    nc.sync.dma_start(
                        out=sst[:],
                        in_=dt["SST"][:, b * CBLK * 256:(b + 1) * CBLK * 256])
                    x0g = dw.tile([128, DP], BF, tag="x0g")
                    nc.gpsimd.indirect_dma_start(
                        out=x0g[:], out_offset=None, in_=dt["atom_pad"][:],
                        in_offset=bass.IndirectOffsetOnAxis(ap=naT_t[:, b:b + 1],
                                                            axis=0))
                    shg0 = dw.tile([128, CBLK * 16], BF, tag="shg0")
                    shagg = dpsD.tile([128, 16], F32, tag="shagg")
                    for ch in range(CBLK):
                        cc = b * CBLK + ch
                        nc.vector.tensor_scalar(
                            out=shg0[:, ch * 16:(ch + 1) * 16],
                            in0=shpad[:, cc * 16:(cc + 1) * 16],
                            scalar1=g0_t[:, cc:cc + 1], scalar2=None, op0=OP.mult)
                        nc.tensor.matmul(
                            shagg[:], lhsT=sst[:, ch * 256 + 128:(ch + 1) * 256],
                            rhs=shg0[:, ch * 16:(ch + 1) * 16],
                            start=(ch == 0), stop=(ch == CBLK - 1))
                    shaggb = dw.tile([128, 16], BF, tag="shaggb")
                    nc.scalar.copy(out=shaggb[:], in_=shagg[:])
                    shaggT = dps.tile([128, 128], BF, tag="shaggT")
                    nc.tensor.transpose(out=shaggT[0:16, :], in_=shaggb[:],
                                        identity=ident[:])
                    shaggTb = dw.tile([16, 128], BF, tag="shaggTb")
                    nc.scalar.copy(out=shaggTb[:], in_=shaggT[0:16, :])
                    degps = dps.tile([128, DP], F32, tag="degps")
                    nc.tensor.matmul(degps[:], lhsT=shaggTb[:], rhs=wdeg16[:],
                                     start=True, stop=True)
                    nc.vector.scalar_tensor_tensor(
                        out=x_t[:, b * DP:(b + 1) * DP], in0=degps[:], scalar=CDEG,
                        in1=x0g[:], op0=OP.mult, op1=OP.add)

            if "x" in dumps and n_layers == 0:
                nc.sync.dma_start(out=dumps["x"][:], in_=x_t[:])

            # ============ PHASE 4: layers ============
            # wallA layout: q 0, k 2048, v 4096, qsh 6144 (cols); wallB: wo 0,
            # f1 2048, f2 6144.
            AW = OO          # wallA width (q|k|v|qsh)
            BW = WCOLS - OO  # wallB width (wo|f1|f2)

            def emit_kv_block(b, wallA_t, sb_pool, ps_pool):
                """xT transpose + k/v GEMMs + kvloc store for block b."""
                rows = min(128, NPC - 128 * b)
                xtp = ps_pool.tile([128, DP], F32, tag="ops")
                for f in range(4):
                    nc.tensor.transpose(
                        out=xtp[:, f * 128:(f + 1) * 128],
                        in_=x_t[:, b * DP + f * 128:b * DP + (f + 1) * 128],
                        identity=identf[:])
                nc.scalar.copy(out=xT_t[:, b * DP:(b + 1) * DP], in_=xtp[:])
                kvb = sb_pool.tile([128, 2 * DP], BF, tag="kvb")
                for nm, off, dst_sl in (("k", 2048, kvb[:, 0:DP]),
                                        ("v", 4096, kvb[:, DP:2 * DP])):
                    ps = ps_pool.tile([128, DP], F32, tag="ops")
                    for f in range(4):
                        nc.tensor.matmul(
                            ps[:],
                            lhsT=xT_t[:, b * DP + f * 128:b * DP + (f + 1) * 128],
                            rhs=wallA_t[:, off + f * DP:off + (f + 1) * DP],
                            start=(f == 0), stop=(f == 3))
                    if nm == "k":
                        nc.scalar.copy(out=dst_sl, in_=ps[:])
                    else:
                        nc.vector.tensor_copy(out=dst_sl, in_=ps[:])
                nc.sync.dma_start(out=kvloc_d[128 * b:128 * b + rows, :],
                                  in_=kvb[0:rows, :])

            wallA_cur = wpa.tile([128, AW], BF, tag="wallA")
            nc.sync.dma_start(out=wallA_cur[:], in_=dt["Wall"][0][:, 0:AW])
            with (
                tc.tile_pool(name="pw", bufs=3) as pw,
                tc.tile_pool(name="pps", bufs=2, space="PSUM") as pps,
            ):
                for b in range(n_blocks):
                    emit_kv_block(b, wallA_cur, pw, pps)
            if n_layers > 0:
                nc.gpsimd.collective_compute(
                    "AllGather", OP.bypass, ins=[kvloc_d[:].opt()],
                    outs=[kvfull_d[:].opt()], replica_groups=RG)

            for l in range(n_layers):
                wallB = wp.tile([128, BW], BF, tag="wallB")
                nc.sync.dma_start(out=wallB[:], in_=dt["Wall"][l][:, OO:WCOLS])
                if l + 1 < n_layers:
                    wallA_next = wpa.tile([128, AW], BF, tag="wallA")
                    nc.sync.dma_start(out=wallA_next[:],
                                      in_=dt["Wall"][l + 1][:, 0:AW])
                # q/qsh GEMMs (overlap the AllGather)
                with tc.tile_pool(name="qps", bufs=2, space="PSUM") as qps:
                    for b in range(n_blocks):
                        ps = qps.tile([128, DP], F32, tag="qp")
                        pss = qps.tile([128, QSW], F32, tag="qsp")
                        for f in range(4):
                            nc.tensor.matmul(
                                ps[:],
                                lhsT=xT_t[:, b * DP + f * 128:b * DP + (f + 1) * 128],
                                rhs=wallA_cur[:, f * DP:(f + 1) * DP],
                                start=(f == 0), stop=(f == 3))
                            nc.tensor.matmul(
                                pss[:],
                                lhsT=xT_t[:, b * DP + f * 128:b * DP + (f + 1) * 128],
                                rhs=wallA_cur[:, 6144 + f * QSW:6144 + (f + 1) * QSW],
                                start=(f == 0), stop=(f == 3))
                        nc.scalar.copy(out=q_t[:, b * DP:(b + 1) * DP], in_=ps[:])
                        nc.vector.tensor_copy(out=qsh_t[:, b * QSW:(b + 1) * QSW],
                                              in_=pss[:])
                # gate MLP for this layer (overlaps the AllGather)
                gate_l = gp.tile([128, C * 4], BF, tag="gate")
                with (
                    tc.tile_pool(name="glw", bufs=3) as glw,
                    tc.tile_pool(name="glps", bufs=2, space="PSUM") as glps,
                ):
                    for c0 in range(0, C, 4):
                        nb4 = min(4, C - c0)
                        rbfT = glw.tile([128, 4 * 128], BF, tag="rbfTl")
                        nc.sync.dma_start(out=rbfT[:, 0:nb4 * 128],
                                          in_=rbf_d[:, c0 * 128:(c0 + nb4) * 128])
                        h1ps = glps.tile([64, 4 * 128], F32, tag="h1ps")
                        nc.tensor.matmul(h1ps[:, 0:nb4 * 128],
                                         lhsT=w1g[:, l * 64:(l + 1) * 64],
                                         rhs=rbfT[:, 0:nb4 * 128],
                                         start=True, stop=True)
                        h1sb = glw.tile([64, 4 * 128], BF, tag="h1sb")
                        nc.scalar.activation(out=h1sb[:, 0:nb4 * 128],
                                             in_=h1ps[:, 0:nb4 * 128], func=AF.Silu)
                        h2ps = glps.tile([64, 4 * 128], F32, tag="h2ps")
                        nc.tensor.matmul(h2ps[:, 0:nb4 * 128],
                                         lhsT=w2g[:, l * 64:(l + 1) * 64],
                                         rhs=h1sb[:, 0:nb4 * 128],
                                         start=True, stop=True)
                        h2sb = glw.tile([64, 4 * 128], BF, tag="h2sb")
                        nc.scalar.activation(out=h2sb[:, 0:nb4 * 128],
                                             in_=h2ps[:, 0:nb4 * 128], func=AF.Silu)
                        gpo = glps.tile([128, 16], F32, tag="gpo")
                        for j in range(nb4):
                            nc.tensor.matmul(gpo[:, j * 4:(j + 1) * 4],
                                             lhsT=h2sb[:, j * 128:(j + 1) * 128],
                                             rhs=w3g[:, l * 4:(l + 1) * 4],
                                             start=True, stop=True)
                        nc.vector.tensor_scalar(
                            out=gate_l[:, c0 * 4:(c0 + nb4) * 4],
                            in0=gpo[:, 0:nb4 * 4], scalar1=INV, scalar2=None,
                            op0=OP.mult)

                # ---- edge phase ----
                with (
                    tc.tile_pool(name="ew", bufs=2) as ew,
                    tc.tile_pool(name="ew3", bufs=4) as ew3,
                    tc.tile_pool(name="ew2", bufs=2) as ew2,
                    tc.tile_pool(name="ekv", bufs=3) as ekv,
                    tc.tile_pool(name="eps_q", bufs=2, space="PSUM") as eps_q,
                    tc.tile_pool(name="eps_s", bufs=1, space="PSUM") as eps_s,
                    tc.tile_pool(name="eps_o", bufs=2, space="PSUM") as eps_o,
                    tc.tile_pool(name="epsD", bufs=1, space="PSUM") as epsD,
                    tc.tile_pool(name="epsT", bufs=1, space="PSUM") as epsT,
                ):
                    for b in range(n_blocks):
                        sst = ew2.tile([128, CBLK * 256], BF, tag="sst")
                        nc.sync.dma_start(
                            out=sst[:],
                            in_=dt["SST"][:, b * CBLK * 256:(b + 1) * CBLK * 256])
                        CH1 = (CBLK + 1) // 2
                        kvga = ekv.tile([128, CH1 * 1024], BF, tag="kvg")
                        kvgb = ekv.tile([128, CH1 * 1024], BF, tag="kvg")

                        def kv_sl(ch, w=1024):
                            t = kvga if ch < CH1 else kvgb
                            o = (ch if ch < CH1 else ch - CH1) * 1024
                            return t[:, o:o + w]

                        for ch in range(CBLK):
                            cc = b * CBLK + ch
                            nc.gpsimd.indirect_dma_start(
                                out=kv_sl(ch), out_offset=None, in_=kvfull_d[:],
                                in_offset=bass.IndirectOffsetOnAxis(
                                    ap=idxT_t[:, cc:cc + 1], axis=0))
                        lgall = ew.tile([128, CBLK * 4], F32, tag="lgall")
                        lgsha = ew.tile([128, CBLK * 4], F32, tag="lgsha")
                        qshb = ew.tile([128, CBLK * QSW], BF, tag="qshb")
                        astore = ew.tile([128, CBLK * 4], BF, tag="astore")
                        denps = epsD.tile([128, 4], F32, tag="denps")
                        aggps = epsD.tile([128, DP], F32, tag="aggps")
                        # pass 1: logits
                        for ch in range(CBLK):
                            cc = b * CBLK + ch
                            st_ap = sst[:, ch * 256:ch * 256 + 128]
                            qexp = eps_q.tile([128, DP], F32, tag="qexp")
                            nc.tensor.matmul(qexp[:], lhsT=st_ap,
                                             rhs=q_t[:, b * DP:(b + 1) * DP],
                                             start=True, stop=True)
                            qshe = eps_s.tile([128, QSW], F32, tag="qshe")
                            nc.tensor.matmul(qshe[:], lhsT=st_ap,
                                             rhs=qsh_t[:, b * QSW:(b + 1) * QSW],
                                             start=True, stop=True)
                            qexpb = ew3.tile([128, DP], BF, tag="qexpb")
                            nc.scalar.copy(out=qexpb[:], in_=qexp[:])
                            nc.scalar.copy(out=qshb[:, ch * QSW:(ch + 1) * QSW],
                                           in_=qshe[:])
                            mtj = ew3.tile([128, DP], BF, tag="mtj")
                            for h in range(4):
                                nc.vector.scalar_tensor_tensor(
                                    out=mtj[:, h * 128:(h + 1) * 128],
                                    in0=kv_sl(ch, DP)[:, h * 128:(h + 1) * 128],
                                    scalar=1.0,
                                    in1=qexpb[:, h * 128:(h + 1) * 128],
                                    op0=OP.mult, op1=OP.mult,
                                    accum_out=lgall[:, ch * 4 + h:ch * 4 + h + 1])
                        # batched sh-logit term for all chunks of this block
                        nc.vector.tensor_tensor(
                            out=qshb[:].rearrange("p (c h w) -> p c h w",
                                                  h=H, w=16),
                            in0=qshb[:].rearrange("p (c h w) -> p c h w",
                                                  h=H, w=16),
                            in1=shpad[:, b * CBLK * 16:(b + 1) * CBLK * 16]
                                .rearrange("p (c o w) -> p c o w", o=1, w=16)
                                .to_broadcast([128, CBLK, H, 16]),
                            op=OP.mult)
                        nc.vector.tensor_reduce(
                            out=lgsha[:].rearrange("p (a o) -> p a o", o=1),
                            in_=qshb[:].rearrange("p (a w) -> p a w", w=16),
                            op=OP.add, axis=AX)
                        # gate + exp + den
                        asb = ew.tile([128, CBLK * 4], F32, tag="asb")
                        nc.vector.tensor_tensor(out=asb[:], in0=lgall[:],
                                                in1=lgsha[:], op=OP.add)
                        nc.vector.tensor_tensor(
                            out=asb[:], in0=asb[:],
                            in1=gate_l[:, b * CBLK * 4:(b + 1) * CBLK * 4],
                            op=OP.mult)
                        astf = ew.tile([128, CBLK * 4], F32, tag="astf")
                        nc.scalar.activation(out=astf[:], in_=asb[:], func=AF.Exp)
                        nc.vector.tensor_copy(out=astore[:], in_=astf[:])
                        for ch in range(CBLK):
                            nc.tensor.matmul(
                                denps[:], lhsT=sst[:, ch * 256 + 128:(ch + 1) * 256],
                                rhs=astore[:, ch * 4:(ch + 1) * 4],
                                start=(ch == 0), stop=(ch == CBLK - 1))
                        dene = ew.tile([128, 4], F32, tag="dene")
                        nc.vector.tensor_scalar(out=dene[:], in0=denps[:],
                                                scalar1=1e-30, scalar2=None,
                                                op0=OP.add)
                        recf = ew.tile([128, 4], F32, tag="recf")
                        nc.vector.reciprocal(out=recf[:], in_=dene[:])
                        # pass 2: unnormalized messages + scatter
                        for ch in range(CBLK):
                            msgb = ew.tile([128, DP], BF, tag="msgb")
                            vsl = kv_sl(ch)
                            for h in range(4):
                                src_sl = vsl[:, 512 + h * 128:512 + (h + 1) * 128]
                                dst_sl = msgb[:, h * 128:(h + 1) * 128]
                                a_col = astf[:, ch * 4 + h:ch * 4 + h + 1]
                                if h == 0:
                                    nc.scalar.mul(out=dst_sl, in_=src_sl, mul=a_col)
                                else:
                                    nc.vector.tensor_scalar(
                                        out=dst_sl, in0=src_sl, scalar1=a_col,
                                        scalar2=None, op0=OP.mult)
                            nc.tensor.matmul(
                                aggps[:], lhsT=sst[:, ch * 256 + 128:(ch + 1) * 256],
                                rhs=msgb[:], start=(ch == 0),
                                stop=(ch == CBLK - 1))
                        # normalize during PSUM evacuation
                        aggb = ew.tile([128, DP], BF, tag="aggb")
                        for h in range(4):
                            if h < 3:
                                nc.scalar.mul(out=aggb[:, h * 128:(h + 1) * 128],
                                              in_=aggps[:, h * 128:(h + 1) * 128],
                                              mul=recf[:, h:h + 1])
                            else:
                                nc.vector.tensor_scalar(
                                    out=aggb[:, h * 128:(h + 1) * 128],
                                    in0=aggps[:, h * 128:(h + 1) * 128],
                                    scalar1=recf[:, h:h + 1], scalar2=None,
                                    op0=OP.mult)
                        aggtp = epsT.tile([128, DP], BF, tag="aggtp")
                        for f in range(4):
                            nc.tensor.transpose(
                                out=aggtp[:, f * 128:(f + 1) * 128],
                                in_=aggb[:, f * 128:(f + 1) * 128],
                                identity=ident[:])
                        aggtb = ew.tile([128, DP], BF, tag="aggtb")
                        nc.vector.tensor_copy(out=aggtb[:], in_=aggtp[:])
                        ops_ = eps_o.tile([128, DP], F32, tag="ops")
                        for f in range(4):
                            nc.tensor.matmul(ops_[:],
                                             lhsT=aggtb[:, f * 128:(f + 1) * 128],
                                             rhs=wallB[:, f * DP:(f + 1) * DP],
                                             start=(f == 0), stop=(f == 3))
                        resid = ew.tile([128, DP], F32, tag="resid")
                        nc.vector.scalar_tensor_tensor(
                            out=resid[:], in0=ops_[:], scalar=CDEG,
                            in1=x_t[:, b * DP:(b + 1) * DP], op0=OP.mult, op1=OP.add)
                        _ln_bn(nc, ew, resid, x_t, b, eps5)
                        # FF block
                        xtp2 = eps_o.tile([128, DP], F32, tag="ops")
                        for f in range(4):
                            nc.tensor.transpose(
                                out=xtp2[:, f * 128:(f + 1) * 128],
                                in_=x_t[:, b * DP + f * 128:b * DP + (f + 1) * 128],
                                identity=identf[:])
                        xtb2 = ew.tile([128, DP], BF, tag="xtb2")
                        nc.scalar.copy(out=xtb2[:], in_=xtp2[:])
                        htb = ew.tile([128, FF], BF, tag="htb")
                        for g2 in range(2):
                            f1a = eps_o.tile([128, DP], F32, tag="ops")
                            for f in range(4):
                                nc.tensor.matmul(
                                    f1a[:],
                                    lhsT=xtb2[:, f * 128:(f + 1) * 128],
                                    rhs=wallB[:, 2048 + f * FF + g2 * DP:
                                              2048 + f * FF + (g2 + 1) * DP],
                                    start=(f == 0), stop=(f == 3))
                            hb = ew.tile([128, DP], BF, tag="hb")
                            nc.scalar.activation(out=hb[:], in_=f1a[:], func=AF.Silu)
                            htp = epsT.tile([128, DP], BF, tag="aggtp")
                            for f in range(4):
                                nc.tensor.transpose(
                                    out=htp[:, f * 128:(f + 1) * 128],
                                    in_=hb[:, f * 128:(f + 1) * 128],
                                    identity=ident[:])
                            nc.vector.tensor_copy(out=htb[:, g2 * DP:(g2 + 1) * DP],
                                                  in_=htp[:])
                        f2p = eps_o.tile([128, DP], F32, tag="ops")
                        for f in range(8):
                            nc.tensor.matmul(f2p[:],
                                             lhsT=htb[:, f * 128:(f + 1) * 128],
                                             rhs=wallB[:, 6144 + f * DP:6144 + (f + 1) * DP],
                                             start=(f == 0), stop=(f == 7))
                        resid2 = ew.tile([128, DP], F32, tag="resid")
                        nc.vector.tensor_tensor(out=resid2[:], in0=f2p[:],
                                                in1=x_t[:, b * DP:(b + 1) * DP],
                                                op=OP.add)
                        _ln_bn(nc, ew, resid2, x_t, b, eps5)
                        if l + 1 < n_layers:
                            emit_kv_block(b, wallA_next, ew, eps_o)
                if l + 1 < n_layers:
                    nc.gpsimd.collective_compute(
                        "AllGather", OP.bypass, ins=[kvloc_d[:].opt()],
                        outs=[kvfull_d[:].opt()], replica_groups=RG)
                    wallA_cur = wallA_next
                if "x" in dumps and l == n_layers - 1:
                    nc.sync.dma_start(out=dumps["x"][:], in_=x_t[:])

            # ============ PHASE 5: readout ============
            with (
                tc.tile_pool(name="fw", bufs=3) as fw,
                tc.tile_pool(name="fps", bufs=1, space="PSUM") as fps,
                tc.tile_pool(name="fpsD", bufs=1, space="PSUM") as fpsD,
            ):
                Sg_t = fw.tile([128, NBLK * G], F32, tag="Sg")
                nc.sync.dma_start(
                    out=Sg_t[:].rearrange("p (b g)   -> p b g", g=G),
                    in_=dt["Sg"].ap().rearrange("(b p) g -> p b g", p=128))
                wh1 = fw.tile([128, 4 * DP], BF, tag="wh1")
                nc.sync.dma_start(
                    out=wh1[:].rearrange("p (a m) -> p a m", a=4),
                    in_=dt["Wh1"].ap().rearrange("(a p) m -> p a m", p=128))
                wh2 = fw.tile([128, 4 * 4], BF, tag="wh2")
                nc.sync.dma_start(
                    out=wh2[:].rearrange("p (a m) -> p a m", a=4),
                    in_=dt["Wh2"].ap().rearrange("(a p) m -> p a m", p=128))
                engps = fpsD.tile([64, 4], F32, tag="engps")
                for b in range(n_blocks):
                    xtp = fps.tile([128, DP], F32, tag="xtp")
                    for f in range(4):
                        nc.tensor.transpose(
                            out=xtp[:, f * 128:(f + 1) * 128],
                            in_=x_t[:, b * DP + f * 128:b * DP + (f + 1) * 128],
                            identity=identf[:])
                    xtb = fw.tile([128, DP], BF, tag="xtb")
                    nc.scalar.copy(out=xtb[:], in_=xtp[:])
                    h1p = fps.tile([128, DP], F32, tag="h1p")
                    for f in range(4):
                        nc.tensor.matmul(h1p[:], lhsT=xtb[:, f * 128:(f + 1) * 128],
                                         rhs=wh1[:, f * DP:(f + 1) * DP],
                                         start=(f == 0), stop=(f == 3))
                    h1b = fw.tile([128, DP], BF, tag="h1b")
                    nc.scalar.activation(out=h1b[:], in_=h1p[:], func=AF.Silu)
                    h1tp = fps.tile([128, DP], BF, tag="h1tp")
                    for f in range(4):
                        nc.tensor.transpose(out=h1tp[:, f * 128:(f + 1) * 128],
                                            in_=h1b[:, f * 128:(f + 1) * 128],
                                            identity=ident[:])
                    h1tb = fw.tile([128, DP], BF, tag="h1tb")
                    nc.scalar.copy(out=h1tb[:], in_=h1tp[:])
                    nep = fps.tile([128, 4], F32, tag="nep")
                    for f in range(4):
                        nc.tensor.matmul(nep[:], lhsT=h1tb[:, f * 128:(f + 1) * 128],
                                         rhs=wh2[:, f * 4:(f + 1) * 4],
                                         start=(f == 0), stop=(f == 3))
                    nef = fw.tile([128, 4], F32, tag="nef")
                    nc.scalar.copy(out=nef[:], in_=nep[:])
                    nc.tensor.matmul(engps[:], lhsT=Sg_t[:, b * G:(b + 1) * G],
                                     rhs=nef[:], start=(b == 0),
                                     stop=(b == n_blocks - 1))
                engsb = fw.tile([64, 1], F32, tag="engsb")
                nc.scalar.mul(out=engsb[:], in_=engps[:, 0:1], mul=1.0 / AVG_NODES)
                engt = fps.tile([64, 64], F32, tag="engt")
                nc.tensor.transpose(out=engt[0:1, 0:64], in_=engsb[:],
                                    identity=identf[0:64, 0:64])
                engrow = fw.tile([1, 64], F32, tag="engrow")
                nc.scalar.copy(out=engrow[:], in_=engt[0:1, 0:64])
                nc.sync.dma_start(out=eng_in_d[:], in_=engrow[:])
                nc.gpsimd.collective_compute(
                    "AllReduce", OP.add, ins=[eng_in_d[:].opt()],
                    outs=[eng_out_d[:].opt()], replica_groups=RG)
                nc.sync.dma_start(out=energy_out[:], in_=eng_out_d[:])

    return nc


def _ln_bn(nc, pool, resid, x_t, b, eps_t):
    """LayerNorm over resid[:, :D] -> x_t[:, b*DP : b*DP+D] via bn_stats."""
    st6 = pool.tile([128, 6], F32, tag="st6")
    nc.vector.bn_stats(out=st6[:], in_=resid[:, 0:D])
    mv = pool.tile([128, 2], F32, tag="mv")
    nc.vector.bn_aggr(out=mv[:], in_=st6[:])
    stdv = pool.tile([128, 1], F32, tag="stdv")
    nc.scalar.activation(out=stdv[:], in_=mv[:, 1:2], func=AF.Sqrt,
                         bias=eps_t[:])
    rstd = pool.tile([128, 1], F32, tag="rstd")
    nc.vector.reciprocal(out=rstd[:], in_=stdv[:])
    nc.vector.tensor_scalar(out=x_t[:, b * DP:b * DP + D], in0=resid[:, 0:D],
                            scalar1=mv[:, 0:1], scalar2=rstd[:],
                            op0=OP.subtract, op1=OP.mult)


# ---------------------------------------------------------------------------
# entry point
# ---------------------------------------------------------------------------

def kernel(**inputs):
    shared, per_core, CBLK = preprocess(inputs)
    in_maps, _ = make_inmaps(inputs, shared, per_core, CBLK)
    nc = build(CBLK)
    split_multi_waits(nc)
    res = run_bass_kernel_spmd(nc, in_maps, core_ids=list(range(NC)))
    return np.asarray(res.results[0]["energy"][0], np.float32).reshape(G)


# revision 53
# speedup vs baseline: 1.0241x; 1.0241x over previous
"""TRN2 Bass kernel: DotProductAttentionTransformer (MD17-style GNN), 8-core SPMD.

Self-contained: host preprocessing (edge sorting/padding, selector matrices,
weight relayout) + Bass/Tile device program (edge-parallel attention with
S-matmul scatter/gather, bf16 GEMMs, fp32 softmax/LN).

v2: single merged k|v gather per edge chunk, spherical-harmonic logits folded
into the q side (Wqsh = Wq_h @ Wsh_h^T), fused multiply-reduce logits,
post-normalized aggregation, engine-balanced PSUM evacuations, and k/v-first
GEMM ordering so the AllGather overlaps q-side work.
"""
import math
import numpy as np
import ml_dtypes

import concourse.bass as bass
import concourse.mybir as mybir
import concourse.tile as tile_mod
from concourse.tile import TileContext
from concourse.masks import make_identity
from concourse.vector_clock import ScopedClock
from concourse.bass_utils import run_bass_kernel_spmd

bf16 = ml_dtypes.bfloat16

N, E, G, D, H, L = 10000, 160000, 64, 480, 4, 6
DH, NB, SH = 120, 128, 9
CUTOFF = 5.0
AVG_DEG = 15.57930850982666
AVG_NODES = 18.03065905448718
NC = 8
NPC = N // NC
NBLK = 10
DP = 512
FF = 1024
QSW = 64          # 4 heads x 16 (9 used) qsh columns
ONE_BF = np.float32(1.0).astype(bf16)

# ---------------------------------------------------------------------------
# harness patches: this walrus build allows only ONE sync-wait per
# instruction; split extras onto same-engine NoOps.
# ---------------------------------------------------------------------------

def _patched_drain_and_barrier(self, tick_clock, wait_clock):
    nc = self.nc
    drain_inst = nc.sync.drain()
    wait_clock.add_sem_waits(drain_inst.ins,
                             ScopedClock({None: tick_clock.global_clock}))
    si = drain_inst.ins.sync_info
    waits = list(si.on_wait or []) if si is not None else []
    if len(waits) > 1:
        id2sem = {h.num: h for h in self.sems.allocated().values()}
        si.on_wait = [waits[0]]
        for w in waits[1:]:
            nop = nc.sync.nop(nofuse=True)
            nop.wait_op(id2sem[w.id], w.wait_value, "sem-ge")
    nc.all_engine_barrier()
    popped = nc._tile_sem_poison_stack.pop()
    assert popped is self._sem_poison
    nc.clear_and_free_semaphores(list(self.sems.allocated().values()))
    nc.all_engine_barrier()


tile_mod.TileContext._drain_and_barrier = _patched_drain_and_barrier

_waitnop_counter = [0]


def split_multi_waits(nc):
    for f in nc.m.functions:
        for bb in f.blocks:
            insts = bb.instructions
            if not any(i.sync_info is not None and i.sync_info.on_wait
                       and len(i.sync_info.on_wait) > 1 for i in insts):
                continue
            new = []
            for inst in insts:
                si = inst.sync_info
                if si is not None and si.on_wait and len(si.on_wait) > 1:
                    waits = list(si.on_wait)
                    for w in waits[:-1]:
                        _waitnop_counter[0] += 1
                        nop = mybir.InstNoOp(
                            name=f"waitnop-{_waitnop_counter[0]}", ins=[], outs=[])
                        nop.engine = inst.engine
                        nop.sync_info = mybir.SyncInfo(on_wait=[w], on_update=[])
                        new.append(nop)
                    si.on_wait = [waits[-1]]
                new.append(inst)
            bb.instructions = new
    return nc


F32 = mybir.dt.float32
BF = mybir.dt.bfloat16
I32 = mybir.dt.int32
AX = mybir.AxisListType.X
OP = mybir.AluOpType
AF = mybir.ActivationFunctionType
INV = 1.0 / math.sqrt(DH)
CDEG = 1.0 / math.sqrt(AVG_DEG)
WIDTH = CUTOFF / NB

# packed layer-weight column offsets (bf16 [128, WCOLS])
OQ = 0
OK_ = OQ + 4 * DP
OV = OK_ + 4 * DP
OQS = OV + 4 * DP
OO = OQS + 4 * QSW
OF1 = OO + 4 * DP
OF2 = OF1 + 4 * FF
WCOLS = OF2 + 8 * DP


def head_pad_cols(W):
    """[in, 480] -> [in, 512]: head h cols 120h:120h+120 -> 128h:128h+120, pad zeros."""
    out = np.zeros((W.shape[0], DP), W.dtype)
    for h in range(H):
        out[:, 128 * h:128 * h + DH] = W[:, DH * h:DH * (h + 1)]
    return out


def plain_pad(W, rows, cols):
    out = np.zeros((rows, cols), W.dtype)
    out[:W.shape[0], :W.shape[1]] = W
    return out


def _amaj(W, a):
    """[a*128, m] -> [128, a*m] partition-major relayout for lhsT tiles."""
    return np.ascontiguousarray(
        W.reshape(a, 128, -1).transpose(1, 0, 2).reshape(128, -1))


def _balance_perm(dst):
    """Within-core node permutation equalizing per-block in-degree sums.

    Returns perm with perm[old_global_id] = new_global_id. The last block of
    each core has only 98 slots, so it is pre-seeded with a top-k/bottom-(98-k)
    degree mix that lands near the per-block average; the 128-slot blocks are
    then filled greedily (min load, then min count).
    """
    perm = np.empty(N, np.int64)
    caps = [128] * 9 + [NPC - 9 * 128]
    for c in range(NC):
        loc = dst[(dst >= c * NPC) & (dst < (c + 1) * NPC)] - c * NPC
        deg = np.bincount(loc, minlength=NPC).astype(np.int64)
        order = np.argsort(-deg, kind="stable")
        sdeg = deg[order]
        target = deg.sum() / NBLK
        pre_top = np.concatenate([[0], np.cumsum(sdeg)])
        pre_bot = np.concatenate([[0], np.cumsum(sdeg[::-1])])
        c9 = caps[9]
        bestk, bestsum = 0, -1
        for k in range(0, c9 + 1):
            s = pre_top[k] + pre_bot[c9 - k]
            if s <= target + 12 and s > bestsum:
                bestsum, bestk = s, k
        members = [[] for _ in range(NBLK)]
        assigned = np.zeros(NPC, bool)
        for n in np.concatenate([order[:bestk],
                                 order[NPC - (c9 - bestk):] if c9 > bestk
                                 else order[:0]]):
            members[9].append(n)
            assigned[n] = True
        load = [0.0] * NBLK
        load[9] = float(deg[np.array(members[9], np.int64)].sum()) \
            if members[9] else 0.0
        cnt = [len(m) for m in members]
        for n in order:
            if assigned[n]:
                continue
            best = min((b for b in range(9) if cnt[b] < 128),
                       key=lambda b: (load[b], cnt[b]))
            members[best].append(n)
            load[best] += float(deg[n])
            cnt[best] += 1
        for b in range(NBLK):
            base = c * NPC + b * 128
            for i, n in enumerate(members[b]):
                perm[c * NPC + n] = base + i
    return perm


def preprocess(inputs):
    """Returns (shared, per_core, CBLK) host arrays. Integer/relayout work only."""
    src = np.asarray(inputs["edge_src"]).astype(np.int64)
    dst = np.asarray(inputs["edge_dst"]).astype(np.int64)
    batch = np.asarray(inputs["batch"]).astype(np.int64)

    # rebalance node->block assignment to minimize the padded chunk count
    perm = _balance_perm(dst)
    inv = np.empty(N, np.int64)
    inv[perm] = np.arange(N)
    src = perm[src]
    dst = perm[dst]
    batch = batch[inv]
    pos_bal = np.asarray(inputs["pos"])[inv]
    natom_bal = np.asarray(inputs["node_atom"])[inv]

    order = np.argsort(dst, kind="stable")
    dsts, srcs = dst[order], src[order]

    # per (core, block) edge lists
    per_block = [[[] for _ in range(NBLK)] for _ in range(NC)]
    core_of = dsts // NPC
    loc = dsts - core_of * NPC
    blk = loc // 128
    for i in range(E):
        per_block[core_of[i]][blk[i]].append(i)

    CBLK = 0
    for c in range(NC):
        for b in range(NBLK):
            CBLK = max(CBLK, (len(per_block[c][b]) + 127) // 128)
    C = NBLK * CBLK

    per_core = []
    for c in range(NC):
        src_idx = np.zeros((NBLK, CBLK, 128), np.int64)
        dst_glob = np.zeros((NBLK, CBLK, 128), np.int64)
        dst_local = np.full((NBLK, CBLK, 128), -1, np.int64)
        for b in range(NBLK):
            el = per_block[c][b]
            for j, i in enumerate(el):
                ch, p = j // 128, j % 128
                src_idx[b, ch, p] = srcs[i]
                dst_local[b, ch, p] = loc[i] - 128 * b
                dst_glob[b, ch, p] = dsts[i]
        # S [e, n] and S_T [n, e] per chunk, bf16 {0,1}
        iota = np.arange(128)
        S = (dst_local[..., None] == iota[None, None, None, :]).astype(bf16)
        ST = np.ascontiguousarray(np.swapaxes(S, 2, 3))
        # pad edges: point ST/dst at the block's max-in-degree node so the
        # expanded q values stay finite (S stays zero -> no contribution).
        for b in range(NBLK):
            deg_b = np.zeros(128, np.int64)
            for ch in range(CBLK):
                vals = dst_local[b, ch]
                np.add.at(deg_b, vals[vals >= 0], 1)
            assert deg_b.max() > 0, f"block {b} of core {c} has no edges"
            nmax = int(deg_b.argmax())
            for ch in range(CBLK):
                padmask = dst_local[b, ch] < 0
                ST[b, ch, nmax, padmask] = ONE_BF
                dst_glob[b, ch][padmask] = c * NPC + 128 * b + nmax
        # combined [ST | S] per chunk: [128, C*256]
        SST = np.empty((128, C * 256), bf16)
        for b in range(NBLK):
            for ch in range(CBLK):
                cc = b * CBLK + ch
                SST[:, cc * 256:cc * 256 + 128] = ST[b, ch]
                SST[:, cc * 256 + 128:(cc + 1) * 256] = S[b, ch]
        idxT = np.ascontiguousarray(
            src_idx.reshape(C, 128).T).astype(np.int32)
        # host-gathered per-edge endpoint positions [128, C*4] (data movement
        # only; subtraction happens on device)
        pos4 = plain_pad(pos_bal.astype(np.float32), N, 4)
        pes = np.ascontiguousarray(
            pos4[src_idx.reshape(C, 128)].transpose(1, 0, 2).reshape(128, C * 4))
        ped = np.ascontiguousarray(
            pos4[dst_glob.reshape(C, 128)].transpose(1, 0, 2).reshape(128, C * 4))
        # graph one-hot for energy: [NBLK*128, G]
        Sg = np.zeros((NBLK * 128, G), np.float32)
        for nl in range(NPC):
            Sg[nl, batch[c * NPC + nl]] = 1.0
        per_core.append(dict(SST=SST, idxT=idxT, pes=pes, ped=ped, Sg=Sg))

    f32 = np.float32
    i = {k: np.asarray(v) for k, v in inputs.items()}

    # per-layer packed weights [L, 128, WCOLS]
    Wq = i["Wq"].astype(f32)
    Wk = i["Wk"].astype(f32)
    Wv = i["Wv"].astype(f32)
    Wsh = i["Wsh"].astype(f32)
    Wo = i["Wo"].astype(f32)
    Wf1 = i["Wf1"].astype(f32)
    Wf2 = i["Wf2"].astype(f32)
    Wall = np.zeros((L, 128, WCOLS), bf16)
    for l in range(L):
        Wqsh = np.zeros((DP, QSW), f32)
        for h in range(H):
            wq_h = Wq[l][:, DH * h:DH * (h + 1)]
            wsh_h = Wsh[l][:, DH * h:DH * (h + 1)]
            Wqsh[:D, 16 * h:16 * h + SH] = wq_h @ wsh_h.T
        parts = [
            _amaj(plain_pad(head_pad_cols(Wq[l]), DP, DP).astype(bf16), 4),
            _amaj(plain_pad(head_pad_cols(Wk[l]), DP, DP).astype(bf16), 4),
            _amaj(plain_pad(head_pad_cols(Wv[l]), DP, DP).astype(bf16), 4),
            _amaj(Wqsh.astype(bf16), 4),
            _amaj(plain_pad(_head_rows(Wo[l]), DP, DP).astype(bf16), 4),
            _amaj(plain_pad(Wf1[l], DP, FF).astype(bf16), 4),
            _amaj(plain_pad(Wf2[l], FF, DP).astype(bf16), 8),
        ]
        Wall[l] = np.concatenate(parts, axis=1)

    shared = dict(
        pos_pad=plain_pad(i["pos"].astype(f32), N, 64),
        atom_pad=plain_pad(i["atom_table"].astype(f32), 64, DP).astype(bf16),
        node_atom=natom_bal.astype(np.int32),
        wdeg16=plain_pad(i["Wdeg"].astype(f32), 16, DP).astype(bf16),
        Wd1=i["Wd1"].astype(bf16), Wd2=i["Wd2"].astype(bf16),
        Wd3=plain_pad(i["Wd3"].astype(f32), 64, 4).astype(bf16),
        W1=i["W1"].astype(bf16), W2=i["W2"].astype(bf16), W3=i["W3"].astype(bf16),
        Wall=Wall,
        Wh1=plain_pad(i["Wh1"].astype(f32), DP, DP).astype(bf16),
        Wh2=plain_pad(i["Wh2"].astype(f32), DP, 4).astype(bf16),
        centers=np.linspace(0, CUTOFF, NB).astype(f32),
    )
    return shared, per_core, CBLK


def _head_rows(W):
    """[480, m] -> [512, m]: head h rows 120h:120h+120 -> 128h:128h+120."""
    out = np.zeros((DP, W.shape[1]), W.dtype)
    for h in range(H):
        out[128 * h:128 * h + DH, :] = W[DH * h:DH * (h + 1), :]
    return out


def make_inmaps(inputs, shared=None, per_core=None, CBLK=None):
    if shared is None:
        shared, per_core, CBLK = preprocess(inputs)
    i32, f32 = np.int32, np.float32
    cenrep = np.broadcast_to(shared["centers"][None, :], (128, NB)).copy()
    na = shared["node_atom"]
    in_maps = []
    for c in range(NC):
        pc = per_core[c]
        naT = np.zeros((128, NBLK), i32)
        na_loc = np.zeros(NBLK * 128, i32)
        na_loc[:NPC] = na[c * NPC:(c + 1) * NPC]
        naT[:] = na_loc.reshape(NBLK, 128).T
        m = dict(
            atom_pad=shared["atom_pad"],
            idxT=pc["idxT"], naT=naT,
            pes=pc["pes"], ped=pc["ped"],
            SST=pc["SST"], Sg=pc["Sg"],
            cenrep=cenrep,
            wdeg16=shared["wdeg16"],
            Wd1=shared["Wd1"], Wd2=shared["Wd2"], Wd3=shared["Wd3"],
            W1=shared["W1"], W2=shared["W2"], W3=shared["W3"],
            Wall=shared["Wall"],
            Wh1=shared["Wh1"], Wh2=shared["Wh2"],
        )
        in_maps.append(m)
    return in_maps, CBLK


def build(CBLK, n_layers=L, n_blocks=NBLK, debug_dumps=()):
    C = n_blocks * CBLK
    nc = bass.Bass("TRN2")
    dt = {}

    def inp(name, shape, dtype):
        dt[name] = nc.dram_tensor(name, shape, dtype, kind="ExternalInput")
        return dt[name]

    inp("atom_pad", [64, DP], BF)
    inp("idxT", [128, C], I32)
    inp("pes", [128, C * 4], F32)
    inp("ped", [128, C * 4], F32)
    inp("naT", [128, NBLK], I32)
    inp("SST", [128, C * 256], BF)
    inp("Sg", [NBLK * 128, G], F32)
    inp("cenrep", [128, NB], F32)
    inp("wdeg16", [16, DP], BF)
    inp("Wd1", [NB, 64], BF)
    inp("Wd2", [64, 64], BF)
    inp("Wd3", [64, 4], BF)
    inp("W1", [L, NB, 64], BF)
    inp("W2", [L, 64, 64], BF)
    inp("W3", [L, 64, H], BF)
    inp("Wall", [L, 128, WCOLS], BF)
    inp("Wh1", [DP, DP], BF)
    inp("Wh2", [DP, 4], BF)

    energy_out = nc.dram_tensor("energy", [1, G], F32, kind="ExternalOutput")
    dumps = {}
    if "x" in debug_dumps:
        dumps["x"] = nc.dram_tensor("x_dump", [128, NBLK * DP], F32,
                                    kind="ExternalOutput")

    RG = [list(range(NC))]

    with TileContext(nc) as tc:
        with (
            tc.tile_pool(name="cst", bufs=1) as cst,
            tc.tile_pool(name="big", bufs=1) as big,
            tc.tile_pool(name="wp", bufs=1) as wp,
            tc.tile_pool(name="wpa", bufs=2) as wpa,
            tc.tile_pool(name="gp", bufs=1) as gp,
            tc.tile_pool(name="dram", bufs=1, space="DRAM") as dram,
        ):
            # ---------------- constants ----------------
            ident = cst.tile([128, 128], BF, tag="ident")
            make_identity(nc, ident[:])
            identf = cst.tile([128, 128], F32, tag="identf")
            make_identity(nc, identf[:])
            eps5 = cst.tile([128, 1], F32, tag="eps5")
            nc.vector.memset(eps5[:], 1e-5)
            cenrep = cst.tile([128, NB], F32, tag="cenrep")
            nc.sync.dma_start(out=cenrep[:], in_=dt["cenrep"][:])
            idxT_t = cst.tile([128, C], I32, tag="idxT")
            nc.sync.dma_start(out=idxT_t[:], in_=dt["idxT"][:])
            naT_t = cst.tile([128, NBLK], I32, tag="naT")
            nc.sync.dma_start(out=naT_t[:], in_=dt["naT"][:])
            wd1 = cst.tile([NB, 64], BF, tag="wd1")
            nc.sync.dma_start(out=wd1[:], in_=dt["Wd1"][:])
            wd2 = cst.tile([64, 64], BF, tag="wd2")
            nc.sync.dma_start(out=wd2[:], in_=dt["Wd2"][:])
            wd3 = cst.tile([64, 4], BF, tag="wd3")
            nc.sync.dma_start(out=wd3[:], in_=dt["Wd3"][:])
            w1g = cst.tile([NB, L * 64], BF, tag="w1g")
            nc.sync.dma_start(out=w1g[:].rearrange("k (l m) -> k l m", l=L),
                in_=dt["W1"].ap().rearrange("l k m -> k l m"))
            w2g = cst.tile([64, L * 64], BF, tag="w2g")
            nc.sync.dma_start(out=w2g[:].rearrange("k (l m) -> k l m", l=L),
                in_=dt["W2"].ap().rearrange("l k m -> k l m"))
            w3g = cst.tile([64, L * H], BF, tag="w3g")
            nc.sync.dma_start(out=w3g[:].rearrange("k (l m) -> k l m", l=L),
                in_=dt["W3"].ap().rearrange("l k m -> k l m"))
            wdeg16 = cst.tile([16, DP], BF, tag="wdeg16")
            nc.sync.dma_start(out=wdeg16[:], in_=dt["wdeg16"][:])
            # ---------------- persistent state ----------------
            x_t = big.tile([128, NBLK * DP], F32, tag="x")
            nc.vector.memset(x_t[:], 0.0)
            xT_t = big.tile([128, NBLK * DP], BF, tag="xT")
            q_t = big.tile([128, NBLK * DP], BF, tag="q")
            qsh_t = big.tile([128, NBLK * QSW], BF, tag="qsh")
            g0_t = big.tile([128, C], F32, tag="g0")
            rr_t = big.tile([128, C], F32, tag="rr")
            shpad = big.tile([128, C * 16], BF, tag="shpad")

            kvloc_d = dram.tile([NPC, 2 * DP], BF, tag="kvloc")
            rbf_d = dram.tile([128, C * 128], BF, tag="rbf_d")
            kvfull_d = nc.dram_tensor("kvfull_sh", [N, 2 * DP], BF,
                                      addr_space="Shared")
            eng_in_d = dram.tile([1, G], F32, tag="eng_in")
            eng_out_d = nc.dram_tensor("engout_sh", [1, G], F32,
                                       addr_space="Shared")

            # ============ PHASE 1: geometry ============
            with (
                tc.tile_pool(name="geo", bufs=1) as geo,
            ):
                shE = geo.tile([128, C * 12], F32, tag="shE")
                sh3 = shE[:].rearrange("p (c f) -> p c f", f=12)
                evi = geo.tile([128, C * 4], F32, tag="evi")
                ev3 = evi[:].rearrange("p (c f) -> p c f", f=4)
                tmp = geo.tile([128, C * 4], F32, tag="evtmp")
                tmp3 = tmp[:].rearrange("p (c f) -> p c f", f=4)
                uu = geo.tile([128, C * 3], F32, tag="uu")
                u3 = uu[:].rearrange("p (c f) -> p c f", f=3)
                rinv = geo.tile([128, C], F32, tag="rinv")

                pes_t = geo.tile([128, C * 4], F32, tag="pes")
                nc.sync.dma_start(out=pes_t[:], in_=dt["pes"][:])
                ped_t = geo.tile([128, C * 4], F32, tag="ped")
                nc.sync.dma_start(out=ped_t[:], in_=dt["ped"][:])
                nc.vector.tensor_tensor(out=evi[:], in0=pes_t[:], in1=ped_t[:],
                                        op=OP.subtract)
                nc.vector.tensor_tensor(out=tmp[:], in0=evi[:], in1=evi[:],
                                        op=OP.mult)
                nc.vector.tensor_reduce(out=ev3[:, :, 3:4], in_=tmp3[:, :, 0:3],
                                        op=OP.add, axis=AX)
                nc.scalar.activation(out=rr_t[:],
                                     in_=ev3[:, :, 3:4].rearrange("p c o -> p (c o)"),
                                     func=AF.Sqrt)
                radd = geo.tile([128, C], F32, tag="radd")
                nc.vector.tensor_scalar(out=radd[:], in0=rr_t[:], scalar1=1e-12,
                                        scalar2=None, op0=OP.add)
                nc.vector.reciprocal(out=rinv[:], in_=radd[:])
                nc.vector.tensor_tensor(
                    out=u3[:, :, 0:3], in0=ev3[:, :, 0:3],
                    in1=rinv[:].rearrange("p (c o) -> p c o", o=1).to_broadcast(
                        [128, C, 3]),
                    op=OP.mult)
                s3c, s5c, s15c = math.sqrt(3.0), math.sqrt(5.0), math.sqrt(15.0)
                nc.vector.memset(shE[:], 0.0)
                nc.vector.memset(sh3[:, :, 0:1].rearrange("p c o -> p (c o)"), 1.0)
                nc.vector.tensor_scalar(out=sh3[:, :, 1:4], in0=u3[:, :, 0:3],
                                        scalar1=s3c, scalar2=None, op0=OP.mult)
                nc.vector.scalar_tensor_tensor(out=sh3[:, :, 4:6], in0=u3[:, :, 0:2],
                                               scalar=s15c, in1=u3[:, :, 1:3],
                                               op0=OP.mult, op1=OP.mult)
                nc.vector.tensor_tensor(out=tmp3[:, :, 0:3], in0=u3[:, :, 0:3],
                                        in1=u3[:, :, 0:3], op=OP.mult)
                nc.vector.tensor_scalar(out=sh3[:, :, 6:7], in0=tmp3[:, :, 2:3],
                                        scalar1=1.5 * s5c, scalar2=-0.5 * s5c,
                                        op0=OP.mult, op1=OP.add)
                nc.vector.scalar_tensor_tensor(out=sh3[:, :, 7:8], in0=u3[:, :, 0:1],
                                               scalar=s15c, in1=u3[:, :, 2:3],
                                               op0=OP.mult, op1=OP.mult)
                nc.vector.tensor_tensor(out=sh3[:, :, 8:9], in0=tmp3[:, :, 0:1],
                                        in1=tmp3[:, :, 1:2], op=OP.subtract)
                nc.vector.tensor_scalar(
                    out=sh3[:, :, 8:9], in0=sh3[:, :, 8:9],
                    scalar1=0.5 * s15c, scalar2=None, op0=OP.mult)
                # shpad [128, C*16]: sh (9 comps) at cols cc*16+s, zero pad
                nc.vector.memset(shpad[:], 0.0)
                nc.scalar.copy(
                    out=shpad[:].rearrange("p (c w) -> p c w", w=16)[:, :, 0:SH],
                    in_=sh3[:, :, 0:SH])

            # ============ PHASE 2: rbf + gate MLPs ============
            with (
                tc.tile_pool(name="rw", bufs=4) as rw,
                tc.tile_pool(name="rw2", bufs=2) as rw2,
                tc.tile_pool(name="rps", bufs=2, space="PSUM") as rps,
                tc.tile_pool(name="rps2", bufs=2, space="PSUM") as rps2,
            ):
                for c0 in range(0, C, 4):
                    nb4 = min(4, C - c0)
                    rbfT = rw.tile([128, 4 * 128], BF, tag="rbfT")
                    for j in range(nb4):
                        cc = c0 + j
                        z = rw.tile([128, NB], F32, tag="z")
                        nc.vector.tensor_scalar(out=z[:], in0=cenrep[:],
                                                scalar1=rr_t[:, cc:cc + 1],
                                                scalar2=1.0 / WIDTH,
                                                op0=OP.subtract, op1=OP.mult)
                        z2 = rw.tile([128, NB], F32, tag="z2")
                        nc.vector.tensor_tensor(out=z2[:], in0=z[:], in1=z[:],
                                                op=OP.mult)
                        rbfe = rw.tile([128, NB], BF, tag="rbfe")
                        nc.scalar.activation(out=rbfe[:], in_=z2[:], func=AF.Exp,
                                             scale=-1.0)
                        rps_t = rps.tile([128, 128], BF, tag="rbf_ps")
                        nc.tensor.transpose(out=rps_t[:], in_=rbfe[:],
                                            identity=ident[:])
                        nc.vector.tensor_copy(out=rbfT[:, j * 128:(j + 1) * 128],
                                              in_=rps_t[:])
                    nc.sync.dma_start(out=rbf_d[:, c0 * 128:(c0 + nb4) * 128],
                                      in_=rbfT[:, 0:nb4 * 128])
                    h1ps = rps.tile([64, 4 * 128], F32, tag="h1ps")
                    nc.tensor.matmul(h1ps[:, 0:nb4 * 128], lhsT=wd1[:],
                                     rhs=rbfT[:, 0:nb4 * 128], start=True, stop=True)
                    h1sb = rw.tile([64, 4 * 128], BF, tag="h1sb")
                    nc.scalar.activation(out=h1sb[:, 0:nb4 * 128],
                                         in_=h1ps[:, 0:nb4 * 128], func=AF.Silu)
                    h2ps = rps.tile([64, 4 * 128], F32, tag="h2ps")
                    nc.tensor.matmul(h2ps[:, 0:nb4 * 128], lhsT=wd2[:],
                                     rhs=h1sb[:, 0:nb4 * 128], start=True, stop=True)
                    h2sb = rw2.tile([64, 4 * 128], BF, tag="h2sb")
                    nc.scalar.activation(out=h2sb[:, 0:nb4 * 128],
                                         in_=h2ps[:, 0:nb4 * 128], func=AF.Silu)
                    gps_o = rps2.tile([128, 16], F32, tag="gate_ps")
                    for j in range(nb4):
                        nc.tensor.matmul(
                            gps_o[:, j * 4:j * 4 + 4],
                            lhsT=h2sb[:, j * 128:(j + 1) * 128],
                            rhs=wd3[:], start=True, stop=True)
                    for j in range(nb4):
                        cc = c0 + j
                        nc.scalar.copy(out=g0_t[:, cc:cc + 1],
                                       in_=gps_o[:, j * 4:j * 4 + 1])

            # ============ PHASE 3: x0 + deg embedding ============
            with (
                tc.tile_pool(name="dw", bufs=3) as dw,
                tc.tile_pool(name="dw2", bufs=2) as dw2,
                tc.tile_pool(name="dps", bufs=2, space="PSUM") as dps,
                tc.tile_pool(name="dpsD", bufs=1, space="PSUM") as dpsD,
            ):
                for b in range(n_blocks):
                    sst = dw2.tile([128, CBLK * 256], BF, tag="sst")
                    nc.sync.dma_start(
                        out=sst[:],
                        in_=dt["SST"][:, b * CBLK * 256:(b + 1) * CBLK * 256])
                    x0g = dw.tile([128, DP], BF, tag="x0g")
                    nc.gpsimd.indirect_dma_start(
                        out=x0g[:], out_offset=None, in_=dt["atom_pad"][:],
                        in_offset=bass.IndirectOffsetOnAxis(ap=naT_t[:, b:b + 1],
                                                            axis=0))
                    shg0 = dw.tile([128, CBLK * 16], BF, tag="shg0")
                    shagg = dpsD.tile([128, 16], F32, tag="shagg")
                    for ch in range(CBLK):
                        cc = b * CBLK + ch
                        nc.vector.tensor_scalar(
                            out=shg0[:, ch * 16:(ch + 1) * 16],
                            in0=shpad[:, cc * 16:(cc + 1) * 16],
                            scalar1=g0_t[:, cc:cc + 1], scalar2=None, op0=OP.mult)
                        nc.tensor.matmul(
                            shagg[:], lhsT=sst[:, ch * 256 + 128:(ch + 1) * 256],
                            rhs=shg0[:, ch * 16:(ch + 1) * 16],
                            start=(ch == 0), stop=(ch == CBLK - 1))
                    shaggb = dw.tile([128, 16], BF, tag="shaggb")
                    nc.scalar.copy(out=shaggb[:], in_=shagg[:])
                    shaggT = dps.tile([128, 128], BF, tag="shaggT")
                    nc.tensor.transpose(out=shaggT[0:16, :], in_=shaggb[:],
                                        identity=ident[:])
                    shaggTb = dw.tile([16, 128], BF, tag="shaggTb")
                    nc.scalar.copy(out=shaggTb[:], in_=shaggT[0:16, :])
                    degps = dps.tile([128, DP], F32, tag="degps")
                    nc.tensor.matmul(degps[:], lhsT=shaggTb[:], rhs=wdeg16[:],
                                     start=True, stop=True)
                    nc.vector.scalar_tensor_tensor(
                        out=x_t[:, b * DP:(b + 1) * DP], in0=degps[:], scalar=CDEG,
                        in1=x0g[:], op0=OP.mult, op1=OP.add)

            if "x" in dumps and n_layers == 0:
                nc.sync.dma_start(out=dumps["x"][:], in_=x_t[:])

            # ============ PHASE 4: layers ============
            # wallA layout: q 0, k 2048, v 4096, qsh 6144 (cols); wallB: wo 0,
            # f1 2048, f2 6144.
            AW = OO          # wallA width (q|k|v|qsh)
            BW = WCOLS - OO  # wallB width (wo|f1|f2)

            def emit_kv_block(b, wallA_t, sb_pool, ps_pool):
                """xT transpose + k/v GEMMs + kvloc store for block b."""
                rows = min(128, NPC - 128 * b)
                xtp = ps_pool.tile([128, DP], F32, tag="ops")
                for f in range(4):
                    nc.tensor.transpose(
                        out=xtp[:, f * 128:(f + 1) * 128],
                        in_=x_t[:, b * DP + f * 128:b * DP + (f + 1) * 128],
                        identity=identf[:])
                nc.scalar.copy(out=xT_t[:, b * DP:(b + 1) * DP], in_=xtp[:])
                kvb = sb_pool.tile([128, 2 * DP], BF, tag="kvb")
                for nm, off, dst_sl in (("k", 2048, kvb[:, 0:DP]),
                                        ("v", 4096, kvb[:, DP:2 * DP])):
                    ps = ps_pool.tile([128, DP], F32, tag="ops")
                    for f in range(4):
                        nc.tensor.matmul(
                            ps[:],
                            lhsT=xT_t[:, b * DP + f * 128:b * DP + (f + 1) * 128],
                            rhs=wallA_t[:, off + f * DP:off + (f + 1) * DP],
                            start=(f == 0), stop=(f == 3))
                    if nm == "k":
                        nc.scalar.copy(out=dst_sl, in_=ps[:])
                    else:
                        nc.vector.tensor_copy(out=dst_sl, in_=ps[:])
                nc.sync.dma_start(out=kvloc_d[128 * b:128 * b + rows, :],
                                  in_=kvb[0:rows, :])

            wallA_cur = wpa.tile([128, AW], BF, tag="wallA")
            nc.sync.dma_start(out=wallA_cur[:], in_=dt["Wall"][0][:, 0:AW])
            with (
                tc.tile_pool(name="pw", bufs=3) as pw,
                tc.tile_pool(name="pps", bufs=2, space="PSUM") as pps,
            ):
                for b in range(n_blocks):
                    emit_kv_block(b, wallA_cur, pw, pps)
            if n_layers > 0:
                nc.gpsimd.collective_compute(
                    "AllGather", OP.bypass, ins=[kvloc_d[:].opt()],
                    outs=[kvfull_d[:].opt()], replica_groups=RG)

            for l in range(n_layers):
                wallB = wp.tile([128, BW], BF, tag="wallB")
                nc.sync.dma_start(out=wallB[:], in_=dt["Wall"][l][:, OO:WCOLS])
                if l + 1 < n_layers:
                    wallA_next = wpa.tile([128, AW], BF, tag="wallA")
                    nc.sync.dma_start(out=wallA_next[:],
                                      in_=dt["Wall"][l + 1][:, 0:AW])
                # q/qsh GEMMs (overlap the AllGather)
                with tc.tile_pool(name="qps", bufs=2, space="PSUM") as qps:
                    for b in range(n_blocks):
                        ps = qps.tile([128, DP], F32, tag="qp")
                        pss = qps.tile([128, QSW], F32, tag="qsp")
                        for f in range(4):
                            nc.tensor.matmul(
                                ps[:],
                                lhsT=xT_t[:, b * DP + f * 128:b * DP + (f + 1) * 128],
                                rhs=wallA_cur[:, f * DP:(f + 1) * DP],
                                start=(f == 0), stop=(f == 3))
                            nc.tensor.matmul(
                                pss[:],
                                lhsT=xT_t[:, b * DP + f * 128:b * DP + (f + 1) * 128],
                                rhs=wallA_cur[:, 6144 + f * QSW:6144 + (f + 1) * QSW],
                                start=(f == 0), stop=(f == 3))
                        nc.scalar.copy(out=q_t[:, b * DP:(b + 1) * DP], in_=ps[:])
                        nc.vector.tensor_copy(out=qsh_t[:, b * QSW:(b + 1) * QSW],
                                              in_=pss[:])
                # gate MLP for this layer (overlaps the AllGather)
                gate_l = gp.tile([128, C * 4], BF, tag="gate")
                with (
                    tc.tile_pool(name="glw", bufs=3) as glw,
                    tc.tile_pool(name="glps", bufs=2, space="PSUM") as glps,
                ):
                    for c0 in range(0, C, 4):
                        nb4 = min(4, C - c0)
                        rbfT = glw.tile([128, 4 * 128], BF, tag="rbfTl")
                        nc.sync.dma_start(out=rbfT[:, 0:nb4 * 128],
                                          in_=rbf_d[:, c0 * 128:(c0 + nb4) * 128])
                        h1ps = glps.tile([64, 4 * 128], F32, tag="h1ps")
                        nc.tensor.matmul(h1ps[:, 0:nb4 * 128],
                                         lhsT=w1g[:, l * 64:(l + 1) * 64],
                                         rhs=rbfT[:, 0:nb4 * 128],
                                         start=True, stop=True)
                        h1sb = glw.tile([64, 4 * 128], BF, tag="h1sb")
                        nc.scalar.activation(out=h1sb[:, 0:nb4 * 128],
                                             in_=h1ps[:, 0:nb4 * 128], func=AF.Silu)
                        h2ps = glps.tile([64, 4 * 128], F32, tag="h2ps")
                        nc.tensor.matmul(h2ps[:, 0:nb4 * 128],
                                         lhsT=w2g[:, l * 64:(l + 1) * 64],
                                         rhs=h1sb[:, 0:nb4 * 128],
                                         start=True, stop=True)
                        h2sb = glw.tile([64, 4 * 128], BF, tag="h2sb")
                        nc.scalar.activation(out=h2sb[:, 0:nb4 * 128],
                                             in_=h2ps[:, 0:nb4 * 128], func=AF.Silu)
                        gpo = glps.tile([128, 16], F32, tag="gpo")
                        for j in range(nb4):
                            nc.tensor.matmul(gpo[:, j * 4:(j + 1) * 4],
                                             lhsT=h2sb[:, j * 128:(j + 1) * 128],
                                             rhs=w3g[:, l * 4:(l + 1) * 4],
                                             start=True, stop=True)
                        nc.vector.tensor_scalar(
                            out=gate_l[:, c0 * 4:(c0 + nb4) * 4],
                            in0=gpo[:, 0:nb4 * 4], scalar1=INV, scalar2=None,
                            op0=OP.mult)

                # ---- edge phase ----
                with (
                    tc.tile_pool(name="ew", bufs=2) as ew,
                    tc.tile_pool(name="ew3", bufs=4) as ew3,
                    tc.tile_pool(name="ew2", bufs=2) as ew2,
                    tc.tile_pool(name="ekv", bufs=3) as ekv,
                    tc.tile_pool(name="eps_q", bufs=2, space="PSUM") as eps_q,
                    tc.tile_pool(name="eps_s", bufs=1, space="PSUM") as eps_s,
                    tc.tile_pool(name="eps_o", bufs=2, space="PSUM") as eps_o,
                    tc.tile_pool(name="epsD", bufs=1, space="PSUM") as epsD,
                    tc.tile_pool(name="epsT", bufs=1, space="PSUM") as epsT,
                ):
                    for b in range(n_blocks):
                        sst = ew2.tile([128, CBLK * 256], BF, tag="sst")
                        nc.sync.dma_start(
                            out=sst[:],
                            in_=dt["SST"][:, b * CBLK * 256:(b + 1) * CBLK * 256])
                        CH1 = (CBLK + 1) // 2
                        kvga = ekv.tile([128, CH1 * 1024], BF, tag="kvg")
                        kvgb = ekv.tile([128, CH1 * 1024], BF, tag="kvg")

                        def kv_sl(ch, w=1024):
                            t = kvga if ch < CH1 else kvgb
                            o = (ch if ch < CH1 else ch - CH1) * 1024
                            return t[:, o:o + w]

                        for ch in range(CBLK):
                            cc = b * CBLK + ch
                            nc.gpsimd.indirect_dma_start(
                                out=kv_sl(ch), out_offset=None, in_=kvfull_d[:],
                                in_offset=bass.IndirectOffsetOnAxis(
                                    ap=idxT_t[:, cc:cc + 1], axis=0))
                        lgall = ew.tile([128, CBLK * 4], F32, tag="lgall")
                        lgsha = ew.tile([128, CBLK * 4], F32, tag="lgsha")
                        qshb = ew.tile([128, CBLK * QSW], BF, tag="qshb")
                        astore = ew.tile([128, CBLK * 4], BF, tag="astore")
                        denps = epsD.tile([128, 4], F32, tag="denps")
                        aggps = epsD.tile([128, DP], F32, tag="aggps")
                        # pass 1: logits
                        for ch in range(CBLK):
                            cc = b * CBLK + ch
                            st_ap = sst[:, ch * 256:ch * 256 + 128]
                            qexp = eps_q.tile([128, DP], F32, tag="qexp")
                            nc.tensor.matmul(qexp[:], lhsT=st_ap,
                                             rhs=q_t[:, b * DP:(b + 1) * DP],
                                             start=True, stop=True)
                            qshe = eps_s.tile([128, QSW], F32, tag="qshe")
                            nc.tensor.matmul(qshe[:], lhsT=st_ap,
                                             rhs=qsh_t[:, b * QSW:(b + 1) * QSW],
                                             start=True, stop=True)
                            qexpb = ew3.tile([128, DP], BF, tag="qexpb")
                            nc.scalar.copy(out=qexpb[:], in_=qexp[:])
                            nc.scalar.copy(out=qshb[:, ch * QSW:(ch + 1) * QSW],
                                           in_=qshe[:])
                            mtj = ew3.tile([128, DP], BF, tag="mtj")
                            for h in range(4):
                                nc.vector.scalar_tensor_tensor(
                                    out=mtj[:, h * 128:(h + 1) * 128],
                                    in0=kv_sl(ch, DP)[:, h * 128:(h + 1) * 128],
                                    scalar=1.0,
                                    in1=qexpb[:, h * 128:(h + 1) * 128],
                                    op0=OP.mult, op1=OP.mult,
                                    accum_out=lgall[:, ch * 4 + h:ch * 4 + h + 1])
                        # batched sh-logit term for all chunks of this block
                        nc.vector.tensor_tensor(
                            out=qshb[:].rearrange("p (c h w) -> p c h w",
                                                  h=H, w=16),
                            in0=qshb[:].rearrange("p (c h w) -> p c h w",
                                                  h=H, w=16),
                            in1=shpad[:, b * CBLK * 16:(b + 1) * CBLK * 16]
                                .rearrange("p (c o w) -> p c o w", o=1, w=16)
                                .to_broadcast([128, CBLK, H, 16]),
                            op=OP.mult)
                        nc.vector.tensor_reduce(
                            out=lgsha[:].rearrange("p (a o) -> p a o", o=1),
                            in_=qshb[:].rearrange("p (a w) -> p a w", w=16),
                            op=OP.add, axis=AX)
                        # gate + exp + den
                        asb = ew.tile([128, CBLK * 4], F32, tag="asb")
                        nc.vector.tensor_tensor(out=asb[:], in0=lgall[:],
                                                in1=lgsha[:], op=OP.add)
                        nc.vector.tensor_tensor(
                            out=asb[:], in0=asb[:],
                            in1=gate_l[:, b * CBLK * 4:(b + 1) * CBLK * 4],
                            op=OP.mult)
                        astf = ew.tile([128, CBLK * 4], F32, tag="astf")
                        nc.scalar.activation(out=astf[:], in_=asb[:], func=AF.Exp)
                        nc.vector.tensor_copy(out=astore[:], in_=astf[:])
                        for ch in range(CBLK):
                            nc.tensor.matmul(
                                denps[:], lhsT=sst[:, ch * 256 + 128:(ch + 1) * 256],
                                rhs=astore[:, ch * 4:(ch + 1) * 4],
                                start=(ch == 0), stop=(ch == CBLK - 1))
                        dene = ew.tile([128, 4], F32, tag="dene")
                        nc.vector.tensor_scalar(out=dene[:], in0=denps[:],
                                                scalar1=1e-30, scalar2=None,
                                                op0=OP.add)
                        recf = ew.tile([128, 4], F32, tag="recf")
                        nc.vector.reciprocal(out=recf[:], in_=dene[:])
                        # pass 2: unnormalized messages + scatter
                        for ch in range(CBLK):
                            msgb = ew.tile([128, DP], BF, tag="msgb")
                            vsl = kv_sl(ch)
                            for h in range(4):
                                src_sl = vsl[:, 512 + h * 128:512 + (h + 1) * 128]
                                dst_sl = msgb[:, h * 128:(h + 1) * 128]
                                a_col = astf[:, ch * 4 + h:ch * 4 + h + 1]
                                if h == 0:
                                    nc.scalar.mul(out=dst_sl, in_=src_sl, mul=a_col)
                                else:
                                    nc.vector.tensor_scalar(
                                        out=dst_sl, in0=src_sl, scalar1=a_col,
                                        scalar2=None, op0=OP.mult)
                            nc.tensor.matmul(
                                aggps[:], lhsT=sst[:, ch * 256 + 128:(ch + 1) * 256],
                                rhs=msgb[:], start=(ch == 0),
                                stop=(ch == CBLK - 1))
                        # normalize during PSUM evacuation
                        aggb = ew.tile([128, DP], BF, tag="aggb")
                        for h in range(4):
                            if h < 3:
                                nc.scalar.mul(out=aggb[:, h * 128:(h + 1) * 128],
                                              in_=aggps[:, h * 128:(h + 1) * 128],
                                              mul=recf[:, h:h + 1])
                            else:
                                nc.vector.tensor_scalar(
                                    out=aggb[:, h * 128:(h + 1) * 128],
                                    in0=aggps[:, h * 128:(h + 1) * 128],
                                    scalar1=recf[:, h:h + 1], scalar2=None,
                                    op0=OP.mult)
                        aggtp = epsT.tile([128, DP], BF, tag="aggtp")
                        for f in range(4):
                            nc.tensor.transpose(
                                out=aggtp[:, f * 128:(f + 1) * 128],
                                in_=aggb[:, f * 128:(f + 1) * 128],
                                identity=ident[:])
                        aggtb = ew.tile([128, DP], BF, tag="aggtb")
                        nc.vector.tensor_copy(out=aggtb[:], in_=aggtp[:])
                        ops_ = eps_o.tile([128, DP], F32, tag="ops")
                        for f in range(4):
                            nc.tensor.matmul(ops_[:],
                                             lhsT=aggtb[:, f * 128:(f + 1) * 128],
                                             rhs=wallB[:, f * DP:(f + 1) * DP],
                                             start=(f == 0), stop=(f == 3))
                        resid = ew.tile([128, DP], F32, tag="resid")
                        nc.vector.scalar_tensor_tensor(
                            out=resid[:], in0=ops_[:], scalar=CDEG,
                            in1=x_t[:, b * DP:(b + 1) * DP], op0=OP.mult, op1=OP.add)
                        _ln_bn(nc, ew, resid, x_t, b, eps5)
                        # FF block
                        xtp2 = eps_o.tile([128, DP], F32, tag="ops")
                        for f in range(4):
                            nc.tensor.transpose(
                                out=xtp2[:, f * 128:(f + 1) * 128],
                                in_=x_t[:, b * DP + f * 128:b * DP + (f + 1) * 128],
                                identity=identf[:])
                        xtb2 = ew.tile([128, DP], BF, tag="xtb2")
                        nc.scalar.copy(out=xtb2[:], in_=xtp2[:])
                        htb = ew.tile([128, FF], BF, tag="htb")
                        for g2 in range(2):
                            f1a = eps_o.tile([128, DP], F32, tag="ops")
                            for f in range(4):
                                nc.tensor.matmul(
                                    f1a[:],
                                    lhsT=xtb2[:, f * 128:(f + 1) * 128],
                                    rhs=wallB[:, 2048 + f * FF + g2 * DP:
                                              2048 + f * FF + (g2 + 1) * DP],
                                    start=(f == 0), stop=(f == 3))
                            hb = ew.tile([128, DP], BF, tag="hb")
                            nc.scalar.activation(out=hb[:], in_=f1a[:], func=AF.Silu)
                            htp = epsT.tile([128, DP], BF, tag="aggtp")
                            for f in range(4):
                                nc.tensor.transpose(
                                    out=htp[:, f * 128:(f + 1) * 128],
                                    in_=hb[:, f * 128:(f + 1) * 128],
                                    identity=ident[:])
                            nc.vector.tensor_copy(out=htb[:, g2 * DP:(g2 + 1) * DP],
                                                  in_=htp[:])
                        f2p = eps_o.tile([128, DP], F32, tag="ops")
                        for f in range(8):
                            nc.tensor.matmul(f2p[:],
                                             lhsT=htb[:, f * 128:(f + 1) * 128],
                                             rhs=wallB[:, 6144 + f * DP:6144 + (f + 1) * DP],
                                             start=(f == 0), stop=(f == 7))
                        resid2 = ew.tile([128, DP], F32, tag="resid")
                        nc.vector.tensor_tensor(out=resid2[:], in0=f2p[:],
                                                in1=x_t[:, b * DP:(b + 1) * DP],
                                                op=OP.add)
                        _ln_bn(nc, ew, resid2, x_t, b, eps5)
                        if l + 1 < n_layers:
                            emit_kv_block(b, wallA_next, ew, eps_o)
                if l + 1 < n_layers:
                    nc.gpsimd.collective_compute(
                        "AllGather", OP.bypass, ins=[kvloc_d[:].opt()],
                        outs=[kvfull_d[:].opt()], replica_groups=RG)
                    wallA_cur = wallA_next
                if "x" in dumps and l == n_layers - 1:
                    nc.sync.dma_start(out=dumps["x"][:], in_=x_t[:])

            # ============ PHASE 5: readout ============
            with (
                tc.tile_pool(name="fw", bufs=3) as fw,
                tc.tile_pool(name="fps", bufs=1, space="PSUM") as fps,
                tc.tile_pool(name="fpsD", bufs=1, space="PSUM") as fpsD,
            ):
                Sg_t = fw.tile([128, NBLK * G], F32, tag="Sg")
                nc.sync.dma_start(
                    out=Sg_t[:].rearrange("p (b g)   -> p b g", g=G),
                    in_=dt["Sg"].ap().rearrange("(b p) g -> p b g", p=128))
                wh1 = fw.tile([128, 4 * DP], BF, tag="wh1")
                nc.sync.dma_start(
                    out=wh1[:].rearrange("p (a m) -> p a m", a=4),
                    in_=dt["Wh1"].ap().rearrange("(a p) m -> p a m", p=128))
                wh2 = fw.tile([128, 4 * 4], BF, tag="wh2")
                nc.sync.dma_start(
                    out=wh2[:].rearrange("p (a m) -> p a m", a=4),
                    in_=dt["Wh2"].ap().rearrange("(a p) m -> p a m", p=128))
                engps = fpsD.tile([64, 4], F32, tag="engps")
                for b in range(n_blocks):
                    xtp = fps.tile([128, DP], F32, tag="xtp")
                    for f in range(4):
                        nc.tensor.transpose(
                            out=xtp[:, f * 128:(f + 1) * 128],
                            in_=x_t[:, b * DP + f * 128:b * DP + (f + 1) * 128],
                            identity=identf[:])
                    xtb = fw.tile([128, DP], BF, tag="xtb")
                    nc.scalar.copy(out=xtb[:], in_=xtp[:])
                    h1p = fps.tile([128, DP], F32, tag="h1p")
                    for f in range(4):
                        nc.tensor.matmul(h1p[:], lhsT=xtb[:, f * 128:(f + 1) * 128],
                                         rhs=wh1[:, f * DP:(f + 1) * DP],
                                         start=(f == 0), stop=(f == 3))
                    h1b = fw.tile([128, DP], BF, tag="h1b")
                    nc.scalar.activation(out=h1b[:], in_=h1p[:], func=AF.Silu)
                    h1tp = fps.tile([128, DP], BF, tag="h1tp")
                    for f in range(4):
                        nc.tensor.transpose(out=h1tp[:, f * 128:(f + 1) * 128],
                                            in_=h1b[:, f * 128:(f + 1) * 128],
                                            identity=ident[:])
                    h1tb = fw.tile([128, DP], BF, tag="h1tb")
                    nc.scalar.copy(out=h1tb[:], in_=h1tp[:])
                    nep = fps.tile([128, 4], F32, tag="nep")
                    for f in range(4):
                        nc.tensor.matmul(nep[:], lhsT=h1tb[:, f * 128:(f + 1) * 128],
                                         rhs=wh2[:, f * 4:(f + 1) * 4],
                                         start=(f == 0), stop=(f == 3))
                    nef = fw.tile([128, 4], F32, tag="nef")
                    nc.scalar.copy(out=nef[:], in_=nep[:])
                    nc.tensor.matmul(engps[:], lhsT=Sg_t[:, b * G:(b + 1) * G],
                                     rhs=nef[:], start=(b == 0),
                                     stop=(b == n_blocks - 1))
                engsb = fw.tile([64, 1], F32, tag="engsb")
                nc.scalar.mul(out=engsb[:], in_=engps[:, 0:1], mul=1.0 / AVG_NODES)
                engt = fps.tile([64, 64], F32, tag="engt")
                nc.tensor.transpose(out=engt[0:1, 0:64], in_=engsb[:],
                                    identity=identf[0:64, 0:64])
                engrow = fw.tile([1, 64], F32, tag="engrow")
                nc.scalar.copy(out=engrow[:], in_=engt[0:1, 0:64])
                nc.sync.dma_start(out=eng_in_d[:], in_=engrow[:])
                nc.gpsimd.collective_compute(
                    "AllReduce", OP.add, ins=[eng_in_d[:].opt()],
                    outs=[eng_out_d[:].opt()], replica_groups=RG)
                nc.sync.dma_start(out=energy_out[:], in_=eng_out_d[:])

    return nc


def _ln_bn(nc, pool, resid, x_t, b, eps_t):
    """LayerNorm over resid[:, :D] -> x_t[:, b*DP : b*DP+D] via bn_stats."""
    st6 = pool.tile([128, 6], F32, tag="st6")
    nc.vector.bn_stats(out=st6[:], in_=resid[:, 0:D])
    mv = pool.tile([128, 2], F32, tag="mv")
    nc.vector.bn_aggr(out=mv[:], in_=st6[:])
    stdv = pool.tile([128, 1], F32, tag="stdv")
    nc.scalar.activation(out=stdv[:], in_=mv[:, 1:2], func=AF.Sqrt,
                         bias=eps_t[:])
    rstd = pool.tile([128, 1], F32, tag="rstd")
    nc.vector.reciprocal(out=rstd[:], in_=stdv[:])
    nc.vector.tensor_scalar(out=x_t[:, b * DP:b * DP + D], in0=resid[:, 0:D],
                            scalar1=mv[:, 0:1], scalar2=rstd[:],
                            op0=OP.subtract, op1=OP.mult)


# ---------------------------------------------------------------------------
# entry point
# ---------------------------------------------------------------------------

def kernel(**inputs):
    shared, per_core, CBLK = preprocess(inputs)
    in_maps, _ = make_inmaps(inputs, shared, per_core, CBLK)
    nc = build(CBLK)
    split_multi_waits(nc)
    res = run_bass_kernel_spmd(nc, in_maps, core_ids=list(range(NC)))
    return np.asarray(res.results[0]["energy"][0], np.float32).reshape(G)


# revision 64
# speedup vs baseline: 1.0267x; 1.0025x over previous
"""TRN2 Bass kernel: DotProductAttentionTransformer (MD17-style GNN), 8-core SPMD.

Self-contained: host preprocessing (edge sorting/padding, selector matrices,
weight relayout) + Bass/Tile device program (edge-parallel attention with
S-matmul scatter/gather, bf16 GEMMs, fp32 softmax/LN).

v2: single merged k|v gather per edge chunk, spherical-harmonic logits folded
into the q side (Wqsh = Wq_h @ Wsh_h^T), fused multiply-reduce logits,
post-normalized aggregation, engine-balanced PSUM evacuations, and k/v-first
GEMM ordering so the AllGather overlaps q-side work.
"""
import math
import numpy as np
import ml_dtypes

import concourse.bass as bass
import concourse.mybir as mybir
import concourse.tile as tile_mod
from concourse.tile import TileContext
from concourse.masks import make_identity
from concourse.vector_clock import ScopedClock
from concourse.bass_utils import run_bass_kernel_spmd

bf16 = ml_dtypes.bfloat16

N, E, G, D, H, L = 10000, 160000, 64, 480, 4, 6
DH, NB, SH = 120, 128, 9
CUTOFF = 5.0
AVG_DEG = 15.57930850982666
AVG_NODES = 18.03065905448718
NC = 8
NPC = N // NC
NBLK = 10
DP = 512
FF = 1024
QSW = 64          # 4 heads x 16 (9 used) qsh columns
ONE_BF = np.float32(1.0).astype(bf16)

# ---------------------------------------------------------------------------
# harness patches: this walrus build allows only ONE sync-wait per
# instruction; split extras onto same-engine NoOps.
# ---------------------------------------------------------------------------

def _patched_drain_and_barrier(self, tick_clock, wait_clock):
    nc = self.nc
    drain_inst = nc.sync.drain()
    wait_clock.add_sem_waits(drain_inst.ins,
                             ScopedClock({None: tick_clock.global_clock}))
    si = drain_inst.ins.sync_info
    waits = list(si.on_wait or []) if si is not None else []
    if len(waits) > 1:
        id2sem = {h.num: h for h in self.sems.allocated().values()}
        si.on_wait = [waits[0]]
        for w in waits[1:]:
            nop = nc.sync.nop(nofuse=True)
            nop.wait_op(id2sem[w.id], w.wait_value, "sem-ge")
    nc.all_engine_barrier()
    popped = nc._tile_sem_poison_stack.pop()
    assert popped is self._sem_poison
    nc.clear_and_free_semaphores(list(self.sems.allocated().values()))
    nc.all_engine_barrier()


tile_mod.TileContext._drain_and_barrier = _patched_drain_and_barrier

_waitnop_counter = [0]


def split_multi_waits(nc):
    for f in nc.m.functions:
        for bb in f.blocks:
            insts = bb.instructions
            if not any(i.sync_info is not None and i.sync_info.on_wait
                       and len(i.sync_info.on_wait) > 1 for i in insts):
                continue
            new = []
            for inst in insts:
                si = inst.sync_info
                if si is not None and si.on_wait and len(si.on_wait) > 1:
                    waits = list(si.on_wait)
                    for w in waits[:-1]:
                        _waitnop_counter[0] += 1
                        nop = mybir.InstNoOp(
                            name=f"waitnop-{_waitnop_counter[0]}", ins=[], outs=[])
                        nop.engine = inst.engine
                        nop.sync_info = mybir.SyncInfo(on_wait=[w], on_update=[])
                        new.append(nop)
                    si.on_wait = [waits[-1]]
                new.append(inst)
            bb.instructions = new
    return nc


F32 = mybir.dt.float32
BF = mybir.dt.bfloat16
I32 = mybir.dt.int32
AX = mybir.AxisListType.X
OP = mybir.AluOpType
AF = mybir.ActivationFunctionType
INV = 1.0 / math.sqrt(DH)
CDEG = 1.0 / math.sqrt(AVG_DEG)
WIDTH = CUTOFF / NB

# packed layer-weight column offsets (bf16 [128, WCOLS])
OQ = 0
OK_ = OQ + 4 * DP
OV = OK_ + 4 * DP
OQS = OV + 4 * DP
OO = OQS + 4 * QSW
OF1 = OO + 4 * DP
OF2 = OF1 + 4 * FF
WCOLS = OF2 + 8 * DP


def head_pad_cols(W):
    """[in, 480] -> [in, 512]: head h cols 120h:120h+120 -> 128h:128h+120, pad zeros."""
    out = np.zeros((W.shape[0], DP), W.dtype)
    for h in range(H):
        out[:, 128 * h:128 * h + DH] = W[:, DH * h:DH * (h + 1)]
    return out


def plain_pad(W, rows, cols):
    out = np.zeros((rows, cols), W.dtype)
    out[:W.shape[0], :W.shape[1]] = W
    return out


def _amaj(W, a):
    """[a*128, m] -> [128, a*m] partition-major relayout for lhsT tiles."""
    return np.ascontiguousarray(
        W.reshape(a, 128, -1).transpose(1, 0, 2).reshape(128, -1))


def _balance_perm(dst):
    """Within-core node permutation equalizing per-block in-degree sums.

    Returns perm with perm[old_global_id] = new_global_id. The last block of
    each core has only 98 slots, so it is pre-seeded with a top-k/bottom-(98-k)
    degree mix that lands near the per-block average; the 128-slot blocks are
    then filled greedily (min load, then min count).
    """
    perm = np.empty(N, np.int64)
    caps = [128] * 9 + [NPC - 9 * 128]
    for c in range(NC):
        loc = dst[(dst >= c * NPC) & (dst < (c + 1) * NPC)] - c * NPC
        deg = np.bincount(loc, minlength=NPC).astype(np.int64)
        order = np.argsort(-deg, kind="stable")
        sdeg = deg[order]
        target = deg.sum() / NBLK
        pre_top = np.concatenate([[0], np.cumsum(sdeg)])
        pre_bot = np.concatenate([[0], np.cumsum(sdeg[::-1])])
        c9 = caps[9]
        bestk, bestsum = 0, -1
        for k in range(0, c9 + 1):
            s = pre_top[k] + pre_bot[c9 - k]
            if s <= target + 12 and s > bestsum:
                bestsum, bestk = s, k
        members = [[] for _ in range(NBLK)]
        assigned = np.zeros(NPC, bool)
        for n in np.concatenate([order[:bestk],
                                 order[NPC - (c9 - bestk):] if c9 > bestk
                                 else order[:0]]):
            members[9].append(n)
            assigned[n] = True
        load = [0.0] * NBLK
        load[9] = float(deg[np.array(members[9], np.int64)].sum()) \
            if members[9] else 0.0
        cnt = [len(m) for m in members]
        for n in order:
            if assigned[n]:
                continue
            best = min((b for b in range(9) if cnt[b] < 128),
                       key=lambda b: (load[b], cnt[b]))
            members[best].append(n)
            load[best] += float(deg[n])
            cnt[best] += 1
        for b in range(NBLK):
            base = c * NPC + b * 128
            for i, n in enumerate(members[b]):
                perm[c * NPC + n] = base + i
    return perm


def preprocess(inputs):
    """Returns (shared, per_core, CBLK) host arrays. Integer/relayout work only."""
    src = np.asarray(inputs["edge_src"]).astype(np.int64)
    dst = np.asarray(inputs["edge_dst"]).astype(np.int64)
    batch = np.asarray(inputs["batch"]).astype(np.int64)

    # rebalance node->block assignment to minimize the padded chunk count
    perm = _balance_perm(dst)
    inv = np.empty(N, np.int64)
    inv[perm] = np.arange(N)
    src = perm[src]
    dst = perm[dst]
    batch = batch[inv]
    pos_bal = np.asarray(inputs["pos"])[inv]
    natom_bal = np.asarray(inputs["node_atom"])[inv]

    order = np.argsort(dst, kind="stable")
    dsts, srcs = dst[order], src[order]

    # per (core, block) edge lists
    per_block = [[[] for _ in range(NBLK)] for _ in range(NC)]
    core_of = dsts // NPC
    loc = dsts - core_of * NPC
    blk = loc // 128
    for i in range(E):
        per_block[core_of[i]][blk[i]].append(i)

    CBLK = 0
    for c in range(NC):
        for b in range(NBLK):
            CBLK = max(CBLK, (len(per_block[c][b]) + 127) // 128)
    C = NBLK * CBLK

    per_core = []
    for c in range(NC):
        src_idx = np.zeros((NBLK, CBLK, 128), np.int64)
        dst_glob = np.zeros((NBLK, CBLK, 128), np.int64)
        dst_local = np.full((NBLK, CBLK, 128), -1, np.int64)
        for b in range(NBLK):
            el = per_block[c][b]
            for j, i in enumerate(el):
                ch, p = j // 128, j % 128
                src_idx[b, ch, p] = srcs[i]
                dst_local[b, ch, p] = loc[i] - 128 * b
                dst_glob[b, ch, p] = dsts[i]
        # S [e, n] and S_T [n, e] per chunk, bf16 {0,1}
        iota = np.arange(128)
        S = (dst_local[..., None] == iota[None, None, None, :]).astype(bf16)
        ST = np.ascontiguousarray(np.swapaxes(S, 2, 3))
        # pad edges: point ST/dst at the block's max-in-degree node so the
        # expanded q values stay finite (S stays zero -> no contribution).
        for b in range(NBLK):
            deg_b = np.zeros(128, np.int64)
            for ch in range(CBLK):
                vals = dst_local[b, ch]
                np.add.at(deg_b, vals[vals >= 0], 1)
            assert deg_b.max() > 0, f"block {b} of core {c} has no edges"
            nmax = int(deg_b.argmax())
            for ch in range(CBLK):
                padmask = dst_local[b, ch] < 0
                ST[b, ch, nmax, padmask] = ONE_BF
                dst_glob[b, ch][padmask] = c * NPC + 128 * b + nmax
        # combined [ST | S] per chunk: [128, C*256]
        SST = np.empty((128, C * 256), bf16)
        for b in range(NBLK):
            for ch in range(CBLK):
                cc = b * CBLK + ch
                SST[:, cc * 256:cc * 256 + 128] = ST[b, ch]
                SST[:, cc * 256 + 128:(cc + 1) * 256] = S[b, ch]
        idxT = np.ascontiguousarray(
            src_idx.reshape(C, 128).T).astype(np.int32)
        # host-gathered per-edge endpoint positions [128, C*4] (data movement
        # only; subtraction happens on device)
        pos4 = plain_pad(pos_bal.astype(np.float32), N, 4)
        pes = np.ascontiguousarray(
            pos4[src_idx.reshape(C, 128)].transpose(1, 0, 2).reshape(128, C * 4))
        ped = np.ascontiguousarray(
            pos4[dst_glob.reshape(C, 128)].transpose(1, 0, 2).reshape(128, C * 4))
        # graph one-hot for energy: [NBLK*128, G]
        Sg = np.zeros((NBLK * 128, G), np.float32)
        for nl in range(NPC):
            Sg[nl, batch[c * NPC + nl]] = 1.0
        per_core.append(dict(SST=SST, idxT=idxT, pes=pes, ped=ped, Sg=Sg))

    f32 = np.float32
    i = {k: np.asarray(v) for k, v in inputs.items()}

    # per-layer packed weights [L, 128, WCOLS]
    Wq = i["Wq"].astype(f32)
    Wk = i["Wk"].astype(f32)
    Wv = i["Wv"].astype(f32)
    Wsh = i["Wsh"].astype(f32)
    Wo = i["Wo"].astype(f32)
    Wf1 = i["Wf1"].astype(f32)
    Wf2 = i["Wf2"].astype(f32)
    Wall = np.zeros((L, 128, WCOLS), bf16)
    for l in range(L):
        Wqsh = np.zeros((DP, QSW), f32)
        for h in range(H):
            wq_h = Wq[l][:, DH * h:DH * (h + 1)]
            wsh_h = Wsh[l][:, DH * h:DH * (h + 1)]
            Wqsh[:D, 16 * h:16 * h + SH] = wq_h @ wsh_h.T
        parts = [
            _amaj(plain_pad(head_pad_cols(Wq[l]), DP, DP).astype(bf16), 4),
            _amaj(plain_pad(head_pad_cols(Wk[l]), DP, DP).astype(bf16), 4),
            _amaj(plain_pad(head_pad_cols(Wv[l]), DP, DP).astype(bf16), 4),
            _amaj(Wqsh.astype(bf16), 4),
            _amaj(plain_pad(_head_rows(Wo[l]), DP, DP).astype(bf16), 4),
            _amaj(plain_pad(Wf1[l], DP, FF).astype(bf16), 4),
            _amaj(plain_pad(Wf2[l], FF, DP).astype(bf16), 8),
        ]
        Wall[l] = np.concatenate(parts, axis=1)

    shared = dict(
        pos_pad=plain_pad(i["pos"].astype(f32), N, 64),
        atom_pad=plain_pad(i["atom_table"].astype(f32), 64, DP).astype(bf16),
        node_atom=natom_bal.astype(np.int32),
        wdeg16=plain_pad(i["Wdeg"].astype(f32), 16, DP).astype(bf16),
        Wd1=i["Wd1"].astype(bf16), Wd2=i["Wd2"].astype(bf16),
        Wd3=plain_pad(i["Wd3"].astype(f32), 64, 4).astype(bf16),
        W1=i["W1"].astype(bf16), W2=i["W2"].astype(bf16), W3=i["W3"].astype(bf16),
        Wall=Wall,
        Wh1=plain_pad(i["Wh1"].astype(f32), DP, DP).astype(bf16),
        Wh2=plain_pad(i["Wh2"].astype(f32), DP, 4).astype(bf16),
        centers=np.linspace(0, CUTOFF, NB).astype(f32),
    )
    return shared, per_core, CBLK


def _head_rows(W):
    """[480, m] -> [512, m]: head h rows 120h:120h+120 -> 128h:128h+120."""
    out = np.zeros((DP, W.shape[1]), W.dtype)
    for h in range(H):
        out[128 * h:128 * h + DH, :] = W[DH * h:DH * (h + 1), :]
    return out


def make_inmaps(inputs, shared=None, per_core=None, CBLK=None):
    if shared is None:
        shared, per_core, CBLK = preprocess(inputs)
    i32, f32 = np.int32, np.float32
    cenrep = np.broadcast_to(shared["centers"][None, :], (128, NB)).copy()
    na = shared["node_atom"]
    in_maps = []
    for c in range(NC):
        pc = per_core[c]
        naT = np.zeros((128, NBLK), i32)
        na_loc = np.zeros(NBLK * 128, i32)
        na_loc[:NPC] = na[c * NPC:(c + 1) * NPC]
        naT[:] = na_loc.reshape(NBLK, 128).T
        m = dict(
            atom_pad=shared["atom_pad"],
            idxT=pc["idxT"], naT=naT,
            pes=pc["pes"], ped=pc["ped"],
            SST=pc["SST"], Sg=pc["Sg"],
            cenrep=cenrep,
            wdeg16=shared["wdeg16"],
            Wd1=shared["Wd1"], Wd2=shared["Wd2"], Wd3=shared["Wd3"],
            W1=shared["W1"], W2=shared["W2"], W3=shared["W3"],
            Wall=shared["Wall"],
            Wh1=shared["Wh1"], Wh2=shared["Wh2"],
        )
        in_maps.append(m)
    return in_maps, CBLK


def build(CBLK, n_layers=L, n_blocks=NBLK, debug_dumps=()):
    C = n_blocks * CBLK
    nc = bass.Bass("TRN2")
    dt = {}

    def inp(name, shape, dtype):
        dt[name] = nc.dram_tensor(name, shape, dtype, kind="ExternalInput")
        return dt[name]

    inp("atom_pad", [64, DP], BF)
    inp("idxT", [128, C], I32)
    inp("pes", [128, C * 4], F32)
    inp("ped", [128, C * 4], F32)
    inp("naT", [128, NBLK], I32)
    inp("SST", [128, C * 256], BF)
    inp("Sg", [NBLK * 128, G], F32)
    inp("cenrep", [128, NB], F32)
    inp("wdeg16", [16, DP], BF)
    inp("Wd1", [NB, 64], BF)
    inp("Wd2", [64, 64], BF)
    inp("Wd3", [64, 4], BF)
    inp("W1", [L, NB, 64], BF)
    inp("W2", [L, 64, 64], BF)
    inp("W3", [L, 64, H], BF)
    inp("Wall", [L, 128, WCOLS], BF)
    inp("Wh1", [DP, DP], BF)
    inp("Wh2", [DP, 4], BF)

    energy_out = nc.dram_tensor("energy", [1, G], F32, kind="ExternalOutput")
    dumps = {}
    if "x" in debug_dumps:
        dumps["x"] = nc.dram_tensor("x_dump", [128, NBLK * DP], F32,
                                    kind="ExternalOutput")

    RG = [list(range(NC))]

    with TileContext(nc) as tc:
        with (
            tc.tile_pool(name="cst", bufs=1) as cst,
            tc.tile_pool(name="big", bufs=1) as big,
            tc.tile_pool(name="wp", bufs=1) as wp,
            tc.tile_pool(name="wpa", bufs=2) as wpa,
            tc.tile_pool(name="gp", bufs=1) as gp,
            tc.tile_pool(name="dram", bufs=1, space="DRAM") as dram,
        ):
            # ---------------- constants ----------------
            ident = cst.tile([128, 128], BF, tag="ident")
            make_identity(nc, ident[:])
            identf = cst.tile([128, 128], F32, tag="identf")
            make_identity(nc, identf[:])
            eps5 = cst.tile([128, 1], F32, tag="eps5")
            nc.vector.memset(eps5[:], 1e-5)
            cenrep = cst.tile([128, NB], F32, tag="cenrep")
            nc.sync.dma_start(out=cenrep[:], in_=dt["cenrep"][:])
            idxT_t = cst.tile([128, C], I32, tag="idxT")
            nc.sync.dma_start(out=idxT_t[:], in_=dt["idxT"][:])
            naT_t = cst.tile([128, NBLK], I32, tag="naT")
            nc.sync.dma_start(out=naT_t[:], in_=dt["naT"][:])
            wd1 = cst.tile([NB, 64], BF, tag="wd1")
            nc.sync.dma_start(out=wd1[:], in_=dt["Wd1"][:])
            wd2 = cst.tile([64, 64], BF, tag="wd2")
            nc.sync.dma_start(out=wd2[:], in_=dt["Wd2"][:])
            wd3 = cst.tile([64, 4], BF, tag="wd3")
            nc.sync.dma_start(out=wd3[:], in_=dt["Wd3"][:])
            w1g = cst.tile([NB, L * 64], BF, tag="w1g")
            nc.sync.dma_start(out=w1g[:].rearrange("k (l m) -> k l m", l=L),
                in_=dt["W1"].ap().rearrange("l k m -> k l m"))
            w2g = cst.tile([64, L * 64], BF, tag="w2g")
            nc.sync.dma_start(out=w2g[:].rearrange("k (l m) -> k l m", l=L),
                in_=dt["W2"].ap().rearrange("l k m -> k l m"))
            w3g = cst.tile([64, L * H], BF, tag="w3g")
            nc.sync.dma_start(out=w3g[:].rearrange("k (l m) -> k l m", l=L),
                in_=dt["W3"].ap().rearrange("l k m -> k l m"))
            wdeg16 = cst.tile([16, DP], BF, tag="wdeg16")
            nc.sync.dma_start(out=wdeg16[:], in_=dt["wdeg16"][:])
            # ---------------- persistent state ----------------
            x_t = big.tile([128, NBLK * DP], F32, tag="x")
            nc.vector.memset(x_t[:], 0.0)
            xT_t = big.tile([128, NBLK * DP], BF, tag="xT")
            q_t = big.tile([128, NBLK * DP], BF, tag="q")
            qsh_t = big.tile([128, NBLK * QSW], BF, tag="qsh")
            g0_t = big.tile([128, C], F32, tag="g0")
            rr_t = big.tile([128, C], F32, tag="rr")
            shpad = big.tile([128, C * 16], BF, tag="shpad")

            kvloc_d = dram.tile([NPC, 2 * DP], BF, tag="kvloc")
            rbf_d = dram.tile([128, C * 128], BF, tag="rbf_d")
            kvfull_d = nc.dram_tensor("kvfull_sh", [N, 2 * DP], BF,
                                      addr_space="Shared")
            eng_in_d = dram.tile([1, G], F32, tag="eng_in")
            eng_out_d = nc.dram_tensor("engout_sh", [1, G], F32,
                                       addr_space="Shared")

            # ============ PHASE 1: geometry ============
            with (
                tc.tile_pool(name="geo", bufs=1) as geo,
            ):
                shE = geo.tile([128, C * 12], F32, tag="shE")
                sh3 = shE[:].rearrange("p (c f) -> p c f", f=12)
                evi = geo.tile([128, C * 4], F32, tag="evi")
                ev3 = evi[:].rearrange("p (c f) -> p c f", f=4)
                tmp = geo.tile([128, C * 4], F32, tag="evtmp")
                tmp3 = tmp[:].rearrange("p (c f) -> p c f", f=4)
                uu = geo.tile([128, C * 3], F32, tag="uu")
                u3 = uu[:].rearrange("p (c f) -> p c f", f=3)
                rinv = geo.tile([128, C], F32, tag="rinv")

                pes_t = geo.tile([128, C * 4], F32, tag="pes")
                nc.sync.dma_start(out=pes_t[:], in_=dt["pes"][:])
                ped_t = geo.tile([128, C * 4], F32, tag="ped")
                nc.sync.dma_start(out=ped_t[:], in_=dt["ped"][:])
                nc.vector.tensor_tensor(out=evi[:], in0=pes_t[:], in1=ped_t[:],
                                        op=OP.subtract)
                nc.vector.tensor_tensor(out=tmp[:], in0=evi[:], in1=evi[:],
                                        op=OP.mult)
                nc.vector.tensor_reduce(out=ev3[:, :, 3:4], in_=tmp3[:, :, 0:3],
                                        op=OP.add, axis=AX)
                nc.scalar.activation(out=rr_t[:],
                                     in_=ev3[:, :, 3:4].rearrange("p c o -> p (c o)"),
                                     func=AF.Sqrt)
                radd = geo.tile([128, C], F32, tag="radd")
                nc.vector.tensor_scalar(out=radd[:], in0=rr_t[:], scalar1=1e-12,
                                        scalar2=None, op0=OP.add)
                nc.vector.reciprocal(out=rinv[:], in_=radd[:])
                nc.vector.tensor_tensor(
                    out=u3[:, :, 0:3], in0=ev3[:, :, 0:3],
                    in1=rinv[:].rearrange("p (c o) -> p c o", o=1).to_broadcast(
                        [128, C, 3]),
                    op=OP.mult)
                s3c, s5c, s15c = math.sqrt(3.0), math.sqrt(5.0), math.sqrt(15.0)
                nc.vector.memset(shE[:], 0.0)
                nc.vector.memset(sh3[:, :, 0:1].rearrange("p c o -> p (c o)"), 1.0)
                nc.vector.tensor_scalar(out=sh3[:, :, 1:4], in0=u3[:, :, 0:3],
                                        scalar1=s3c, scalar2=None, op0=OP.mult)
                nc.vector.scalar_tensor_tensor(out=sh3[:, :, 4:6], in0=u3[:, :, 0:2],
                                               scalar=s15c, in1=u3[:, :, 1:3],
                                               op0=OP.mult, op1=OP.mult)
                nc.vector.tensor_tensor(out=tmp3[:, :, 0:3], in0=u3[:, :, 0:3],
                                        in1=u3[:, :, 0:3], op=OP.mult)
                nc.vector.tensor_scalar(out=sh3[:, :, 6:7], in0=tmp3[:, :, 2:3],
                                        scalar1=1.5 * s5c, scalar2=-0.5 * s5c,
                                        op0=OP.mult, op1=OP.add)
                nc.vector.scalar_tensor_tensor(out=sh3[:, :, 7:8], in0=u3[:, :, 0:1],
                                               scalar=s15c, in1=u3[:, :, 2:3],
                                               op0=OP.mult, op1=OP.mult)
                nc.vector.tensor_tensor(out=sh3[:, :, 8:9], in0=tmp3[:, :, 0:1],
                                        in1=tmp3[:, :, 1:2], op=OP.subtract)
                nc.vector.tensor_scalar(
                    out=sh3[:, :, 8:9], in0=sh3[:, :, 8:9],
                    scalar1=0.5 * s15c, scalar2=None, op0=OP.mult)
                # shpad [128, C*16]: sh (9 comps) at cols cc*16+s, zero pad
                nc.vector.memset(shpad[:], 0.0)
                nc.scalar.copy(
                    out=shpad[:].rearrange("p (c w) -> p c w", w=16)[:, :, 0:SH],
                    in_=sh3[:, :, 0:SH])

            # ============ PHASE 2: rbf + gate MLPs ============
            with (
                tc.tile_pool(name="rw", bufs=4) as rw,
                tc.tile_pool(name="rw2", bufs=2) as rw2,
                tc.tile_pool(name="rps", bufs=2, space="PSUM") as rps,
                tc.tile_pool(name="rps2", bufs=2, space="PSUM") as rps2,
            ):
                for c0 in range(0, C, 4):
                    nb4 = min(4, C - c0)
                    rbfT = rw.tile([128, 4 * 128], BF, tag="rbfT")
                    for j in range(nb4):
                        cc = c0 + j
                        z = rw.tile([128, NB], F32, tag="z")
                        nc.vector.tensor_scalar(out=z[:], in0=cenrep[:],
                                                scalar1=rr_t[:, cc:cc + 1],
                                                scalar2=1.0 / WIDTH,
                                                op0=OP.subtract, op1=OP.mult)
                        z2 = rw.tile([128, NB], F32, tag="z2")
                        nc.vector.tensor_tensor(out=z2[:], in0=z[:], in1=z[:],
                                                op=OP.mult)
                        rbfe = rw.tile([128, NB], BF, tag="rbfe")
                        nc.scalar.activation(out=rbfe[:], in_=z2[:], func=AF.Exp,
                                             scale=-1.0)
                        rps_t = rps.tile([128, 128], BF, tag="rbf_ps")
                        nc.tensor.transpose(out=rps_t[:], in_=rbfe[:],
                                            identity=ident[:])
                        nc.vector.tensor_copy(out=rbfT[:, j * 128:(j + 1) * 128],
                                              in_=rps_t[:])
                    nc.sync.dma_start(out=rbf_d[:, c0 * 128:(c0 + nb4) * 128],
                                      in_=rbfT[:, 0:nb4 * 128])
                    h1ps = rps.tile([64, 4 * 128], F32, tag="h1ps")
                    nc.tensor.matmul(h1ps[:, 0:nb4 * 128], lhsT=wd1[:],
                                     rhs=rbfT[:, 0:nb4 * 128], start=True, stop=True)
                    h1sb = rw.tile([64, 4 * 128], BF, tag="h1sb")
                    nc.scalar.activation(out=h1sb[:, 0:nb4 * 128],
                                         in_=h1ps[:, 0:nb4 * 128], func=AF.Silu)
                    h2ps = rps.tile([64, 4 * 128], F32, tag="h2ps")
                    nc.tensor.matmul(h2ps[:, 0:nb4 * 128], lhsT=wd2[:],
                                     rhs=h1sb[:, 0:nb4 * 128], start=True, stop=True)
                    h2sb = rw2.tile([64, 4 * 128], BF, tag="h2sb")
                    nc.scalar.activation(out=h2sb[:, 0:nb4 * 128],
                                         in_=h2ps[:, 0:nb4 * 128], func=AF.Silu)
                    gps_o = rps2.tile([128, 16], F32, tag="gate_ps")
                    for j in range(nb4):
                        nc.tensor.matmul(
                            gps_o[:, j * 4:j * 4 + 4],
                            lhsT=h2sb[:, j * 128:(j + 1) * 128],
                            rhs=wd3[:], start=True, stop=True)
                    for j in range(nb4):
                        cc = c0 + j
                        nc.scalar.copy(out=g0_t[:, cc:cc + 1],
                                       in_=gps_o[:, j * 4:j * 4 + 1])

            # ============ PHASE 3: x0 + deg embedding ============
            with (
                tc.tile_pool(name="dw", bufs=3) as dw,
                tc.tile_pool(name="dw2", bufs=2) as dw2,
                tc.tile_pool(name="dps", bufs=2, space="PSUM") as dps,
                tc.tile_pool(name="dpsD", bufs=1, space="PSUM") as dpsD,
            ):
                for b in range(n_blocks):
                    sst = dw2.tile([128, CBLK * 256], BF, tag="sst")
                    nc.sync.dma_start(
                        out=sst[:],
                        in_=dt["SST"][:, b * CBLK * 256:(b + 1) * CBLK * 256])
                    x0g = dw.tile([128, DP], BF, tag="x0g")
                    nc.gpsimd.indirect_dma_start(
                        out=x0g[:], out_offset=None, in_=dt["atom_pad"][:],
                        in_offset=bass.IndirectOffsetOnAxis(ap=naT_t[:, b:b + 1],
                                                            axis=0))
                    shg0 = dw.tile([128, CBLK * 16], BF, tag="shg0")
                    shagg = dpsD.tile([128, 16], F32, tag="shagg")
                    for ch in range(CBLK):
                        cc = b * CBLK + ch
                        nc.vector.tensor_scalar(
                            out=shg0[:, ch * 16:(ch + 1) * 16],
                            in0=shpad[:, cc * 16:(cc + 1) * 16],
                            scalar1=g0_t[:, cc:cc + 1], scalar2=None, op0=OP.mult)
                        nc.tensor.matmul(
                            shagg[:], lhsT=sst[:, ch * 256 + 128:(ch + 1) * 256],
                            rhs=shg0[:, ch * 16:(ch + 1) * 16],
                            start=(ch == 0), stop=(ch == CBLK - 1))
                    shaggb = dw.tile([128, 16], BF, tag="shaggb")
                    nc.scalar.copy(out=shaggb[:], in_=shagg[:])
                    shaggT = dps.tile([128, 128], BF, tag="shaggT")
                    nc.tensor.transpose(out=shaggT[0:16, :], in_=shaggb[:],
                                        identity=ident[:])
                    shaggTb = dw.tile([16, 128], BF, tag="shaggTb")
                    nc.scalar.copy(out=shaggTb[:], in_=shaggT[0:16, :])
                    degps = dps.tile([128, DP], F32, tag="degps")
                    nc.tensor.matmul(degps[:], lhsT=shaggTb[:], rhs=wdeg16[:],
                                     start=True, stop=True)
                    nc.vector.scalar_tensor_tensor(
                        out=x_t[:, b * DP:(b + 1) * DP], in0=degps[:], scalar=CDEG,
                        in1=x0g[:], op0=OP.mult, op1=OP.add)

            if "x" in dumps and n_layers == 0:
                nc.sync.dma_start(out=dumps["x"][:], in_=x_t[:])

            # ============ PHASE 4: layers ============
            # wallA layout: q 0, k 2048, v 4096, qsh 6144 (cols); wallB: wo 0,
            # f1 2048, f2 6144.
            AW = OO          # wallA width (q|k|v|qsh)
            BW = WCOLS - OO  # wallB width (wo|f1|f2)

            def emit_kv_block(b, wallA_t, sb_pool, ps_pool):
                """xT transpose + k/v GEMMs + kvloc store for block b."""
                rows = min(128, NPC - 128 * b)
                xtp = ps_pool.tile([128, DP], F32, tag="ops")
                for f in range(4):
                    nc.tensor.transpose(
                        out=xtp[:, f * 128:(f + 1) * 128],
                        in_=x_t[:, b * DP + f * 128:b * DP + (f + 1) * 128],
                        identity=identf[:])
                nc.scalar.copy(out=xT_t[:, b * DP:(b + 1) * DP], in_=xtp[:])
                kvb = sb_pool.tile([128, 2 * DP], BF, tag="kvb")
                for nm, off, dst_sl in (("k", 2048, kvb[:, 0:DP]),
                                        ("v", 4096, kvb[:, DP:2 * DP])):
                    ps = ps_pool.tile([128, DP], F32, tag="ops")
                    for f in range(4):
                        nc.tensor.matmul(
                            ps[:],
                            lhsT=xT_t[:, b * DP + f * 128:b * DP + (f + 1) * 128],
                            rhs=wallA_t[:, off + f * DP:off + (f + 1) * DP],
                            start=(f == 0), stop=(f == 3))
                    if nm == "k":
                        nc.scalar.copy(out=dst_sl, in_=ps[:])
                    else:
                        nc.vector.tensor_copy(out=dst_sl, in_=ps[:])
                nc.sync.dma_start(out=kvloc_d[128 * b:128 * b + rows, :],
                                  in_=kvb[0:rows, :])

            wallA_cur = wpa.tile([128, AW], BF, tag="wallA")
            nc.sync.dma_start(out=wallA_cur[:], in_=dt["Wall"][0][:, 0:AW])
            with (
                tc.tile_pool(name="pw", bufs=3) as pw,
                tc.tile_pool(name="pps", bufs=2, space="PSUM") as pps,
            ):
                for b in range(n_blocks):
                    emit_kv_block(b, wallA_cur, pw, pps)
            if n_layers > 0:
                nc.gpsimd.collective_compute(
                    "AllGather", OP.bypass, ins=[kvloc_d[:].opt()],
                    outs=[kvfull_d[:].opt()], replica_groups=RG)

            for l in range(n_layers):
                wallB = wp.tile([128, BW], BF, tag="wallB")
                nc.sync.dma_start(out=wallB[:], in_=dt["Wall"][l][:, OO:WCOLS])
                if l + 1 < n_layers:
                    wallA_next = wpa.tile([128, AW], BF, tag="wallA")
                    nc.sync.dma_start(out=wallA_next[:],
                                      in_=dt["Wall"][l + 1][:, 0:AW])
                # q/qsh GEMMs (overlap the AllGather)
                with tc.tile_pool(name="qps", bufs=2, space="PSUM") as qps:
                    for b in range(n_blocks):
                        ps = qps.tile([128, DP], F32, tag="qp")
                        pss = qps.tile([128, QSW], F32, tag="qsp")
                        for f in range(4):
                            nc.tensor.matmul(
                                ps[:],
                                lhsT=xT_t[:, b * DP + f * 128:b * DP + (f + 1) * 128],
                                rhs=wallA_cur[:, f * DP:(f + 1) * DP],
                                start=(f == 0), stop=(f == 3))
                            nc.tensor.matmul(
                                pss[:],
                                lhsT=xT_t[:, b * DP + f * 128:b * DP + (f + 1) * 128],
                                rhs=wallA_cur[:, 6144 + f * QSW:6144 + (f + 1) * QSW],
                                start=(f == 0), stop=(f == 3))
                        nc.scalar.copy(out=q_t[:, b * DP:(b + 1) * DP], in_=ps[:])
                        nc.vector.tensor_copy(out=qsh_t[:, b * QSW:(b + 1) * QSW],
                                              in_=pss[:])
                # gate MLP for this layer (overlaps the AllGather)
                gate_l = gp.tile([128, C * 4], BF, tag="gate")
                with (
                    tc.tile_pool(name="glw", bufs=3) as glw,
                    tc.tile_pool(name="glps", bufs=2, space="PSUM") as glps,
                ):
                    for c0 in range(0, C, 4):
                        nb4 = min(4, C - c0)
                        rbfT = glw.tile([128, 4 * 128], BF, tag="rbfTl")
                        nc.sync.dma_start(out=rbfT[:, 0:nb4 * 128],
                                          in_=rbf_d[:, c0 * 128:(c0 + nb4) * 128])
                        h1ps = glps.tile([64, 4 * 128], F32, tag="h1ps")
                        nc.tensor.matmul(h1ps[:, 0:nb4 * 128],
                                         lhsT=w1g[:, l * 64:(l + 1) * 64],
                                         rhs=rbfT[:, 0:nb4 * 128],
                                         start=True, stop=True)
                        h1sb = glw.tile([64, 4 * 128], BF, tag="h1sb")
                        nc.scalar.activation(out=h1sb[:, 0:nb4 * 128],
                                             in_=h1ps[:, 0:nb4 * 128], func=AF.Silu)
                        h2ps = glps.tile([64, 4 * 128], F32, tag="h2ps")
                        nc.tensor.matmul(h2ps[:, 0:nb4 * 128],
                                         lhsT=w2g[:, l * 64:(l + 1) * 64],
                                         rhs=h1sb[:, 0:nb4 * 128],
                                         start=True, stop=True)
                        h2sb = glw.tile([64, 4 * 128], BF, tag="h2sb")
                        nc.scalar.activation(out=h2sb[:, 0:nb4 * 128],
                                             in_=h2ps[:, 0:nb4 * 128], func=AF.Silu)
                        gpo = glps.tile([128, 16], F32, tag="gpo")
                        for j in range(nb4):
                            nc.tensor.matmul(gpo[:, j * 4:(j + 1) * 4],
                                             lhsT=h2sb[:, j * 128:(j + 1) * 128],
                                             rhs=w3g[:, l * 4:(l + 1) * 4],
                                             start=True, stop=True)
                        nc.vector.tensor_scalar(
                            out=gate_l[:, c0 * 4:(c0 + nb4) * 4],
                            in0=gpo[:, 0:nb4 * 4], scalar1=INV, scalar2=None,
                            op0=OP.mult)

                # ---- edge phase ----
                with (
                    tc.tile_pool(name="ew", bufs=2) as ew,
                    tc.tile_pool(name="ew3", bufs=4) as ew3,
                    tc.tile_pool(name="ew2", bufs=2) as ew2,
                    tc.tile_pool(name="ekv", bufs=3) as ekv,
                    tc.tile_pool(name="eps_q", bufs=2, space="PSUM") as eps_q,
                    tc.tile_pool(name="eps_s", bufs=1, space="PSUM") as eps_s,
                    tc.tile_pool(name="eps_o", bufs=2, space="PSUM") as eps_o,
                    tc.tile_pool(name="epsD", bufs=1, space="PSUM") as epsD,
                    tc.tile_pool(name="epsT", bufs=1, space="PSUM") as epsT,
                ):
                    for b in range(n_blocks):
                        sst = ew2.tile([128, CBLK * 256], BF, tag="sst")
                        nc.sync.dma_start(
                            out=sst[:],
                            in_=dt["SST"][:, b * CBLK * 256:(b + 1) * CBLK * 256])
                        CH1 = (CBLK + 1) // 2
                        kvga = ekv.tile([128, CH1 * 1024], BF, tag="kvg")
                        kvgb = ekv.tile([128, CH1 * 1024], BF, tag="kvg")

                        def kv_sl(ch, w=1024):
                            t = kvga if ch < CH1 else kvgb
                            o = (ch if ch < CH1 else ch - CH1) * 1024
                            return t[:, o:o + w]

                        for ch in range(CBLK):
                            cc = b * CBLK + ch
                            nc.gpsimd.indirect_dma_start(
                                out=kv_sl(ch), out_offset=None, in_=kvfull_d[:],
                                in_offset=bass.IndirectOffsetOnAxis(
                                    ap=idxT_t[:, cc:cc + 1], axis=0))
                        lgall = ew.tile([128, CBLK * 4], F32, tag="lgall")
                        lgsha = ew.tile([128, CBLK * 4], F32, tag="lgsha")
                        qshb = ew.tile([128, CBLK * QSW], BF, tag="qshb")
                        astore = ew.tile([128, CBLK * 4], BF, tag="astore")
                        denps = epsD.tile([128, 4], F32, tag="denps")
                        aggps = epsD.tile([128, DP], F32, tag="aggps")
                        # pass 1: logits
                        for ch in range(CBLK):
                            cc = b * CBLK + ch
                            st_ap = sst[:, ch * 256:ch * 256 + 128]
                            qexp = eps_q.tile([128, DP], F32, tag="qexp")
                            nc.tensor.matmul(qexp[:], lhsT=st_ap,
                                             rhs=q_t[:, b * DP:(b + 1) * DP],
                                             start=True, stop=True)
                            qshe = eps_s.tile([128, QSW], F32, tag="qshe")
                            nc.tensor.matmul(qshe[:], lhsT=st_ap,
                                             rhs=qsh_t[:, b * QSW:(b + 1) * QSW],
                                             start=True, stop=True)
                            qexpb = ew3.tile([128, DP], BF, tag="qexpb")
                            nc.scalar.copy(out=qexpb[:], in_=qexp[:])
                            nc.scalar.copy(out=qshb[:, ch * QSW:(ch + 1) * QSW],
                                           in_=qshe[:])
                            mtj = ew3.tile([128, DP], BF, tag="mtj")
                            for h in range(4):
                                nc.vector.scalar_tensor_tensor(
                                    out=mtj[:, h * 128:(h + 1) * 128],
                                    in0=kv_sl(ch, DP)[:, h * 128:(h + 1) * 128],
                                    scalar=1.0,
                                    in1=qexpb[:, h * 128:(h + 1) * 128],
                                    op0=OP.mult, op1=OP.mult,
                                    accum_out=lgall[:, ch * 4 + h:ch * 4 + h + 1])
                        # batched sh-logit term for all chunks of this block
                        nc.vector.tensor_tensor(
                            out=qshb[:].rearrange("p (c h w) -> p c h w",
                                                  h=H, w=16),
                            in0=qshb[:].rearrange("p (c h w) -> p c h w",
                                                  h=H, w=16),
                            in1=shpad[:, b * CBLK * 16:(b + 1) * CBLK * 16]
                                .rearrange("p (c o w) -> p c o w", o=1, w=16)
                                .to_broadcast([128, CBLK, H, 16]),
                            op=OP.mult)
                        nc.vector.tensor_reduce(
                            out=lgsha[:].rearrange("p (a o) -> p a o", o=1),
                            in_=qshb[:].rearrange("p (a w) -> p a w", w=16),
                            op=OP.add, axis=AX)
                        # gate + exp + den
                        asb = ew.tile([128, CBLK * 4], F32, tag="asb")
                        nc.vector.tensor_tensor(out=asb[:], in0=lgall[:],
                                                in1=lgsha[:], op=OP.add)
                        nc.vector.tensor_tensor(
                            out=asb[:], in0=asb[:],
                            in1=gate_l[:, b * CBLK * 4:(b + 1) * CBLK * 4],
                            op=OP.mult)
                        astf = ew.tile([128, CBLK * 4], F32, tag="astf")
                        nc.scalar.activation(out=astf[:], in_=asb[:], func=AF.Exp)
                        nc.vector.tensor_copy(out=astore[:], in_=astf[:])
                        for ch in range(CBLK):
                            nc.tensor.matmul(
                                denps[:], lhsT=sst[:, ch * 256 + 128:(ch + 1) * 256],
                                rhs=astore[:, ch * 4:(ch + 1) * 4],
                                start=(ch == 0), stop=(ch == CBLK - 1))
                        dene = ew.tile([128, 4], F32, tag="dene")
                        nc.vector.tensor_scalar(out=dene[:], in0=denps[:],
                                                scalar1=1e-30, scalar2=None,
                                                op0=OP.add)
                        recf = ew.tile([128, 4], F32, tag="recf")
                        nc.vector.reciprocal(out=recf[:], in_=dene[:])
                        # pass 2: unnormalized messages + scatter
                        for ch in range(CBLK):
                            msgb = ew3.tile([128, DP], BF, tag="msgb")
                            vsl = kv_sl(ch)
                            for h in range(4):
                                src_sl = vsl[:, 512 + h * 128:512 + (h + 1) * 128]
                                dst_sl = msgb[:, h * 128:(h + 1) * 128]
                                a_col = astf[:, ch * 4 + h:ch * 4 + h + 1]
                                if h == 0:
                                    nc.scalar.mul(out=dst_sl, in_=src_sl, mul=a_col)
                                else:
                                    nc.vector.tensor_scalar(
                                        out=dst_sl, in0=src_sl, scalar1=a_col,
                                        scalar2=None, op0=OP.mult)
                            nc.tensor.matmul(
                                aggps[:], lhsT=sst[:, ch * 256 + 128:(ch + 1) * 256],
                                rhs=msgb[:], start=(ch == 0),
                                stop=(ch == CBLK - 1))
                        # normalize during PSUM evacuation
                        aggb = ew.tile([128, DP], BF, tag="aggb")
                        for h in range(4):
                            if h < 3:
                                nc.scalar.mul(out=aggb[:, h * 128:(h + 1) * 128],
                                              in_=aggps[:, h * 128:(h + 1) * 128],
                                              mul=recf[:, h:h + 1])
                            else:
                                nc.vector.tensor_scalar(
                                    out=aggb[:, h * 128:(h + 1) * 128],
                                    in0=aggps[:, h * 128:(h + 1) * 128],
                                    scalar1=recf[:, h:h + 1], scalar2=None,
                                    op0=OP.mult)
                        aggtp = epsT.tile([128, DP], BF, tag="aggtp")
                        for f in range(4):
                            nc.tensor.transpose(
                                out=aggtp[:, f * 128:(f + 1) * 128],
                                in_=aggb[:, f * 128:(f + 1) * 128],
                                identity=ident[:])
                        aggtb = ew.tile([128, DP], BF, tag="aggtb")
                        nc.vector.tensor_copy(out=aggtb[:], in_=aggtp[:])
                        ops_ = eps_o.tile([128, DP], F32, tag="ops")
                        for f in range(4):
                            nc.tensor.matmul(ops_[:],
                                             lhsT=aggtb[:, f * 128:(f + 1) * 128],
                                             rhs=wallB[:, f * DP:(f + 1) * DP],
                                             start=(f == 0), stop=(f == 3))
                        resid = ew.tile([128, DP], F32, tag="resid")
                        nc.vector.scalar_tensor_tensor(
                            out=resid[:], in0=ops_[:], scalar=CDEG,
                            in1=x_t[:, b * DP:(b + 1) * DP], op0=OP.mult, op1=OP.add)
                        _ln_bn(nc, ew, resid, x_t, b, eps5)
                        # FF block
                        xtp2 = eps_o.tile([128, DP], F32, tag="ops")
                        for f in range(4):
                            nc.tensor.transpose(
                                out=xtp2[:, f * 128:(f + 1) * 128],
                                in_=x_t[:, b * DP + f * 128:b * DP + (f + 1) * 128],
                                identity=identf[:])
                        xtb2 = ew.tile([128, DP], BF, tag="xtb2")
                        nc.scalar.copy(out=xtb2[:], in_=xtp2[:])
                        htb = ew.tile([128, FF], BF, tag="htb")
                        for g2 in range(2):
                            f1a = eps_o.tile([128, DP], F32, tag="ops")
                            for f in range(4):
                                nc.tensor.matmul(
                                    f1a[:],
                                    lhsT=xtb2[:, f * 128:(f + 1) * 128],
                                    rhs=wallB[:, 2048 + f * FF + g2 * DP:
                                              2048 + f * FF + (g2 + 1) * DP],
                                    start=(f == 0), stop=(f == 3))
                            hb = ew.tile([128, DP], BF, tag="hb")
                            nc.scalar.activation(out=hb[:], in_=f1a[:], func=AF.Silu)
                            htp = epsT.tile([128, DP], BF, tag="aggtp")
                            for f in range(4):
                                nc.tensor.transpose(
                                    out=htp[:, f * 128:(f + 1) * 128],
                                    in_=hb[:, f * 128:(f + 1) * 128],
                                    identity=ident[:])
                            nc.vector.tensor_copy(out=htb[:, g2 * DP:(g2 + 1) * DP],
                                                  in_=htp[:])
                        f2p = eps_o.tile([128, DP], F32, tag="ops")
                        for f in range(8):
                            nc.tensor.matmul(f2p[:],
                                             lhsT=htb[:, f * 128:(f + 1) * 128],
                                             rhs=wallB[:, 6144 + f * DP:6144 + (f + 1) * DP],
                                             start=(f == 0), stop=(f == 7))
                        resid2 = ew.tile([128, DP], F32, tag="resid")
                        nc.vector.tensor_tensor(out=resid2[:], in0=f2p[:],
                                                in1=x_t[:, b * DP:(b + 1) * DP],
                                                op=OP.add)
                        _ln_bn(nc, ew, resid2, x_t, b, eps5)
                        if l + 1 < n_layers:
                            emit_kv_block(b, wallA_next, ew, eps_o)
                if l + 1 < n_layers:
                    nc.gpsimd.collective_compute(
                        "AllGather", OP.bypass, ins=[kvloc_d[:].opt()],
                        outs=[kvfull_d[:].opt()], replica_groups=RG)
                    wallA_cur = wallA_next
                if "x" in dumps and l == n_layers - 1:
                    nc.sync.dma_start(out=dumps["x"][:], in_=x_t[:])

            # ============ PHASE 5: readout ============
            with (
                tc.tile_pool(name="fw", bufs=3) as fw,
                tc.tile_pool(name="fps", bufs=1, space="PSUM") as fps,
                tc.tile_pool(name="fpsD", bufs=1, space="PSUM") as fpsD,
            ):
                Sg_t = fw.tile([128, NBLK * G], F32, tag="Sg")
                nc.sync.dma_start(
                    out=Sg_t[:].rearrange("p (b g)   -> p b g", g=G),
                    in_=dt["Sg"].ap().rearrange("(b p) g -> p b g", p=128))
                wh1 = fw.tile([128, 4 * DP], BF, tag="wh1")
                nc.sync.dma_start(
                    out=wh1[:].rearrange("p (a m) -> p a m", a=4),
                    in_=dt["Wh1"].ap().rearrange("(a p) m -> p a m", p=128))
                wh2 = fw.tile([128, 4 * 4], BF, tag="wh2")
                nc.sync.dma_start(
                    out=wh2[:].rearrange("p (a m) -> p a m", a=4),
                    in_=dt["Wh2"].ap().rearrange("(a p) m -> p a m", p=128))
                engps = fpsD.tile([64, 4], F32, tag="engps")
                for b in range(n_blocks):
                    xtp = fps.tile([128, DP], F32, tag="xtp")
                    for f in range(4):
                        nc.tensor.transpose(
                            out=xtp[:, f * 128:(f + 1) * 128],
                            in_=x_t[:, b * DP + f * 128:b * DP + (f + 1) * 128],
                            identity=identf[:])
                    xtb = fw.tile([128, DP], BF, tag="xtb")
                    nc.scalar.copy(out=xtb[:], in_=xtp[:])
                    h1p = fps.tile([128, DP], F32, tag="h1p")
                    for f in range(4):
                        nc.tensor.matmul(h1p[:], lhsT=xtb[:, f * 128:(f + 1) * 128],
                                         rhs=wh1[:, f * DP:(f + 1) * DP],
                                         start=(f == 0), stop=(f == 3))
                    h1b = fw.tile([128, DP], BF, tag="h1b")
                    nc.scalar.activation(out=h1b[:], in_=h1p[:], func=AF.Silu)
                    h1tp = fps.tile([128, DP], BF, tag="h1tp")
                    for f in range(4):
                        nc.tensor.transpose(out=h1tp[:, f * 128:(f + 1) * 128],
                                            in_=h1b[:, f * 128:(f + 1) * 128],
                                            identity=ident[:])
                    h1tb = fw.tile([128, DP], BF, tag="h1tb")
                    nc.scalar.copy(out=h1tb[:], in_=h1tp[:])
                    nep = fps.tile([128, 4], F32, tag="nep")
                    for f in range(4):
                        nc.tensor.matmul(nep[:], lhsT=h1tb[:, f * 128:(f + 1) * 128],
                                         rhs=wh2[:, f * 4:(f + 1) * 4],
                                         start=(f == 0), stop=(f == 3))
                    nef = fw.tile([128, 4], F32, tag="nef")
                    nc.scalar.copy(out=nef[:], in_=nep[:])
                    nc.tensor.matmul(engps[:], lhsT=Sg_t[:, b * G:(b + 1) * G],
                                     rhs=nef[:], start=(b == 0),
                                     stop=(b == n_blocks - 1))
                engsb = fw.tile([64, 1], F32, tag="engsb")
                nc.scalar.mul(out=engsb[:], in_=engps[:, 0:1], mul=1.0 / AVG_NODES)
                engt = fps.tile([64, 64], F32, tag="engt")
                nc.tensor.transpose(out=engt[0:1, 0:64], in_=engsb[:],
                                    identity=identf[0:64, 0:64])
                engrow = fw.tile([1, 64], F32, tag="engrow")
                nc.scalar.copy(out=engrow[:], in_=engt[0:1, 0:64])
                nc.sync.dma_start(out=eng_in_d[:], in_=engrow[:])
                nc.gpsimd.collective_compute(
                    "AllReduce", OP.add, ins=[eng_in_d[:].opt()],
                    outs=[eng_out_d[:].opt()], replica_groups=RG)
                nc.sync.dma_start(out=energy_out[:], in_=eng_out_d[:])

    return nc


def _ln_bn(nc, pool, resid, x_t, b, eps_t):
    """LayerNorm over resid[:, :D] -> x_t[:, b*DP : b*DP+D] via bn_stats."""
    st6 = pool.tile([128, 6], F32, tag="st6")
    nc.vector.bn_stats(out=st6[:], in_=resid[:, 0:D])
    mv = pool.tile([128, 2], F32, tag="mv")
    nc.vector.bn_aggr(out=mv[:], in_=st6[:])
    stdv = pool.tile([128, 1], F32, tag="stdv")
    nc.scalar.activation(out=stdv[:], in_=mv[:, 1:2], func=AF.Sqrt,
                         bias=eps_t[:])
    rstd = pool.tile([128, 1], F32, tag="rstd")
    nc.vector.reciprocal(out=rstd[:], in_=stdv[:])
    nc.vector.tensor_scalar(out=x_t[:, b * DP:b * DP + D], in0=resid[:, 0:D],
                            scalar1=mv[:, 0:1], scalar2=rstd[:],
                            op0=OP.subtract, op1=OP.mult)


# ---------------------------------------------------------------------------
# entry point
# ---------------------------------------------------------------------------

def kernel(**inputs):
    shared, per_core, CBLK = preprocess(inputs)
    in_maps, _ = make_inmaps(inputs, shared, per_core, CBLK)
    nc = build(CBLK)
    split_multi_waits(nc)
    res = run_bass_kernel_spmd(nc, in_maps, core_ids=list(range(NC)))
    return np.asarray(res.results[0]["energy"][0], np.float32).reshape(G)


# revision 65
# speedup vs baseline: 1.0336x; 1.0067x over previous
"""TRN2 Bass kernel: DotProductAttentionTransformer (MD17-style GNN), 8-core SPMD.

Self-contained: host preprocessing (edge sorting/padding, selector matrices,
weight relayout) + Bass/Tile device program (edge-parallel attention with
S-matmul scatter/gather, bf16 GEMMs, fp32 softmax/LN).

v2: single merged k|v gather per edge chunk, spherical-harmonic logits folded
into the q side (Wqsh = Wq_h @ Wsh_h^T), fused multiply-reduce logits,
post-normalized aggregation, engine-balanced PSUM evacuations, and k/v-first
GEMM ordering so the AllGather overlaps q-side work.
"""
import math
import numpy as np
import ml_dtypes

import concourse.bass as bass
import concourse.mybir as mybir
import concourse.tile as tile_mod
from concourse.tile import TileContext
from concourse.masks import make_identity
from concourse.vector_clock import ScopedClock
from concourse.bass_utils import run_bass_kernel_spmd

bf16 = ml_dtypes.bfloat16

N, E, G, D, H, L = 10000, 160000, 64, 480, 4, 6
DH, NB, SH = 120, 128, 9
CUTOFF = 5.0
AVG_DEG = 15.57930850982666
AVG_NODES = 18.03065905448718
NC = 8
NPC = N // NC
NBLK = 10
DP = 512
FF = 1024
QSW = 64          # 4 heads x 16 (9 used) qsh columns
ONE_BF = np.float32(1.0).astype(bf16)

# ---------------------------------------------------------------------------
# harness patches: this walrus build allows only ONE sync-wait per
# instruction; split extras onto same-engine NoOps.
# ---------------------------------------------------------------------------

def _patched_drain_and_barrier(self, tick_clock, wait_clock):
    nc = self.nc
    drain_inst = nc.sync.drain()
    wait_clock.add_sem_waits(drain_inst.ins,
                             ScopedClock({None: tick_clock.global_clock}))
    si = drain_inst.ins.sync_info
    waits = list(si.on_wait or []) if si is not None else []
    if len(waits) > 1:
        id2sem = {h.num: h for h in self.sems.allocated().values()}
        si.on_wait = [waits[0]]
        for w in waits[1:]:
            nop = nc.sync.nop(nofuse=True)
            nop.wait_op(id2sem[w.id], w.wait_value, "sem-ge")
    nc.all_engine_barrier()
    popped = nc._tile_sem_poison_stack.pop()
    assert popped is self._sem_poison
    nc.clear_and_free_semaphores(list(self.sems.allocated().values()))
    nc.all_engine_barrier()


tile_mod.TileContext._drain_and_barrier = _patched_drain_and_barrier

_waitnop_counter = [0]


def split_multi_waits(nc):
    for f in nc.m.functions:
        for bb in f.blocks:
            insts = bb.instructions
            if not any(i.sync_info is not None and i.sync_info.on_wait
                       and len(i.sync_info.on_wait) > 1 for i in insts):
                continue
            new = []
            for inst in insts:
                si = inst.sync_info
                if si is not None and si.on_wait and len(si.on_wait) > 1:
                    waits = list(si.on_wait)
                    for w in waits[:-1]:
                        _waitnop_counter[0] += 1
                        nop = mybir.InstNoOp(
                            name=f"waitnop-{_waitnop_counter[0]}", ins=[], outs=[])
                        nop.engine = inst.engine
                        nop.sync_info = mybir.SyncInfo(on_wait=[w], on_update=[])
                        new.append(nop)
                    si.on_wait = [waits[-1]]
                new.append(inst)
            bb.instructions = new
    return nc


F32 = mybir.dt.float32
BF = mybir.dt.bfloat16
I32 = mybir.dt.int32
AX = mybir.AxisListType.X
OP = mybir.AluOpType
AF = mybir.ActivationFunctionType
INV = 1.0 / math.sqrt(DH)
CDEG = 1.0 / math.sqrt(AVG_DEG)
WIDTH = CUTOFF / NB

# packed layer-weight column offsets (bf16 [128, WCOLS])
OQ = 0
OK_ = OQ + 4 * DP
OV = OK_ + 4 * DP
OQS = OV + 4 * DP
OO = OQS + 4 * QSW
OF1 = OO + 4 * DP
OF2 = OF1 + 4 * FF
WCOLS = OF2 + 8 * DP


def head_pad_cols(W):
    """[in, 480] -> [in, 512]: head h cols 120h:120h+120 -> 128h:128h+120, pad zeros."""
    out = np.zeros((W.shape[0], DP), W.dtype)
    for h in range(H):
        out[:, 128 * h:128 * h + DH] = W[:, DH * h:DH * (h + 1)]
    return out


def plain_pad(W, rows, cols):
    out = np.zeros((rows, cols), W.dtype)
    out[:W.shape[0], :W.shape[1]] = W
    return out


def _amaj(W, a):
    """[a*128, m] -> [128, a*m] partition-major relayout for lhsT tiles."""
    return np.ascontiguousarray(
        W.reshape(a, 128, -1).transpose(1, 0, 2).reshape(128, -1))


def _balance_perm(dst):
    """Within-core node permutation equalizing per-block in-degree sums.

    Returns perm with perm[old_global_id] = new_global_id. The last block of
    each core has only 98 slots, so it is pre-seeded with a top-k/bottom-(98-k)
    degree mix that lands near the per-block average; the 128-slot blocks are
    then filled greedily (min load, then min count).
    """
    perm = np.empty(N, np.int64)
    caps = [128] * 9 + [NPC - 9 * 128]
    for c in range(NC):
        loc = dst[(dst >= c * NPC) & (dst < (c + 1) * NPC)] - c * NPC
        deg = np.bincount(loc, minlength=NPC).astype(np.int64)
        order = np.argsort(-deg, kind="stable")
        sdeg = deg[order]
        target = deg.sum() / NBLK
        pre_top = np.concatenate([[0], np.cumsum(sdeg)])
        pre_bot = np.concatenate([[0], np.cumsum(sdeg[::-1])])
        c9 = caps[9]
        bestk, bestsum = 0, -1
        for k in range(0, c9 + 1):
            s = pre_top[k] + pre_bot[c9 - k]
            if s <= target + 12 and s > bestsum:
                bestsum, bestk = s, k
        members = [[] for _ in range(NBLK)]
        assigned = np.zeros(NPC, bool)
        for n in np.concatenate([order[:bestk],
                                 order[NPC - (c9 - bestk):] if c9 > bestk
                                 else order[:0]]):
            members[9].append(n)
            assigned[n] = True
        load = [0.0] * NBLK
        load[9] = float(deg[np.array(members[9], np.int64)].sum()) \
            if members[9] else 0.0
        cnt = [len(m) for m in members]
        for n in order:
            if assigned[n]:
                continue
            best = min((b for b in range(9) if cnt[b] < 128),
                       key=lambda b: (load[b], cnt[b]))
            members[best].append(n)
            load[best] += float(deg[n])
            cnt[best] += 1
        for b in range(NBLK):
            base = c * NPC + b * 128
            for i, n in enumerate(members[b]):
                perm[c * NPC + n] = base + i
    return perm


def preprocess(inputs):
    """Returns (shared, per_core, CBLK) host arrays. Integer/relayout work only."""
    src = np.asarray(inputs["edge_src"]).astype(np.int64)
    dst = np.asarray(inputs["edge_dst"]).astype(np.int64)
    batch = np.asarray(inputs["batch"]).astype(np.int64)

    # rebalance node->block assignment to minimize the padded chunk count
    perm = _balance_perm(dst)
    inv = np.empty(N, np.int64)
    inv[perm] = np.arange(N)
    src = perm[src]
    dst = perm[dst]
    batch = batch[inv]
    pos_bal = np.asarray(inputs["pos"])[inv]
    natom_bal = np.asarray(inputs["node_atom"])[inv]

    order = np.argsort(dst, kind="stable")
    dsts, srcs = dst[order], src[order]

    # per (core, block) edge lists
    per_block = [[[] for _ in range(NBLK)] for _ in range(NC)]
    core_of = dsts // NPC
    loc = dsts - core_of * NPC
    blk = loc // 128
    for i in range(E):
        per_block[core_of[i]][blk[i]].append(i)

    CBLK = 0
    for c in range(NC):
        for b in range(NBLK):
            CBLK = max(CBLK, (len(per_block[c][b]) + 127) // 128)
    C = NBLK * CBLK

    per_core = []
    for c in range(NC):
        src_idx = np.zeros((NBLK, CBLK, 128), np.int64)
        dst_glob = np.zeros((NBLK, CBLK, 128), np.int64)
        dst_local = np.full((NBLK, CBLK, 128), -1, np.int64)
        for b in range(NBLK):
            el = per_block[c][b]
            for j, i in enumerate(el):
                ch, p = j // 128, j % 128
                src_idx[b, ch, p] = srcs[i]
                dst_local[b, ch, p] = loc[i] - 128 * b
                dst_glob[b, ch, p] = dsts[i]
        # S [e, n] and S_T [n, e] per chunk, bf16 {0,1}
        iota = np.arange(128)
        S = (dst_local[..., None] == iota[None, None, None, :]).astype(bf16)
        ST = np.ascontiguousarray(np.swapaxes(S, 2, 3))
        # pad edges: point ST/dst at the block's max-in-degree node so the
        # expanded q values stay finite (S stays zero -> no contribution).
        for b in range(NBLK):
            deg_b = np.zeros(128, np.int64)
            for ch in range(CBLK):
                vals = dst_local[b, ch]
                np.add.at(deg_b, vals[vals >= 0], 1)
            assert deg_b.max() > 0, f"block {b} of core {c} has no edges"
            nmax = int(deg_b.argmax())
            for ch in range(CBLK):
                padmask = dst_local[b, ch] < 0
                ST[b, ch, nmax, padmask] = ONE_BF
                dst_glob[b, ch][padmask] = c * NPC + 128 * b + nmax
        # combined [ST | S] per chunk: [128, C*256]
        SST = np.empty((128, C * 256), bf16)
        for b in range(NBLK):
            for ch in range(CBLK):
                cc = b * CBLK + ch
                SST[:, cc * 256:cc * 256 + 128] = ST[b, ch]
                SST[:, cc * 256 + 128:(cc + 1) * 256] = S[b, ch]
        idxT = np.ascontiguousarray(
            src_idx.reshape(C, 128).T).astype(np.int32)
        # host-gathered per-edge endpoint positions [128, C*4] (data movement
        # only; subtraction happens on device)
        pos4 = plain_pad(pos_bal.astype(np.float32), N, 4)
        pes = np.ascontiguousarray(
            pos4[src_idx.reshape(C, 128)].transpose(1, 0, 2).reshape(128, C * 4))
        ped = np.ascontiguousarray(
            pos4[dst_glob.reshape(C, 128)].transpose(1, 0, 2).reshape(128, C * 4))
        # graph one-hot for energy: [NBLK*128, G]
        Sg = np.zeros((NBLK * 128, G), np.float32)
        for nl in range(NPC):
            Sg[nl, batch[c * NPC + nl]] = 1.0
        per_core.append(dict(SST=SST, idxT=idxT, pes=pes, ped=ped, Sg=Sg))

    f32 = np.float32
    i = {k: np.asarray(v) for k, v in inputs.items()}

    # per-layer packed weights [L, 128, WCOLS]
    Wq = i["Wq"].astype(f32)
    Wk = i["Wk"].astype(f32)
    Wv = i["Wv"].astype(f32)
    Wsh = i["Wsh"].astype(f32)
    Wo = i["Wo"].astype(f32)
    Wf1 = i["Wf1"].astype(f32)
    Wf2 = i["Wf2"].astype(f32)
    Wall = np.zeros((L, 128, WCOLS), bf16)
    for l in range(L):
        Wqsh = np.zeros((DP, QSW), f32)
        for h in range(H):
            wq_h = Wq[l][:, DH * h:DH * (h + 1)]
            wsh_h = Wsh[l][:, DH * h:DH * (h + 1)]
            Wqsh[:D, 16 * h:16 * h + SH] = wq_h @ wsh_h.T
        parts = [
            _amaj(plain_pad(head_pad_cols(Wq[l]), DP, DP).astype(bf16), 4),
            _amaj(plain_pad(head_pad_cols(Wk[l]), DP, DP).astype(bf16), 4),
            _amaj(plain_pad(head_pad_cols(Wv[l]), DP, DP).astype(bf16), 4),
            _amaj(Wqsh.astype(bf16), 4),
            _amaj(plain_pad(_head_rows(Wo[l]), DP, DP).astype(bf16), 4),
            _amaj(plain_pad(Wf1[l], DP, FF).astype(bf16), 4),
            _amaj(plain_pad(Wf2[l], FF, DP).astype(bf16), 8),
        ]
        Wall[l] = np.concatenate(parts, axis=1)

    shared = dict(
        pos_pad=plain_pad(i["pos"].astype(f32), N, 64),
        atom_pad=plain_pad(i["atom_table"].astype(f32), 64, DP).astype(bf16),
        node_atom=natom_bal.astype(np.int32),
        wdeg16=plain_pad(i["Wdeg"].astype(f32), 16, DP).astype(bf16),
        Wd1=i["Wd1"].astype(bf16), Wd2=i["Wd2"].astype(bf16),
        Wd3=plain_pad(i["Wd3"].astype(f32), 64, 4).astype(bf16),
        W1=i["W1"].astype(bf16), W2=i["W2"].astype(bf16), W3=i["W3"].astype(bf16),
        Wall=Wall,
        Wh1=plain_pad(i["Wh1"].astype(f32), DP, DP).astype(bf16),
        Wh2=plain_pad(i["Wh2"].astype(f32), DP, 4).astype(bf16),
        centers=np.linspace(0, CUTOFF, NB).astype(f32),
    )
    return shared, per_core, CBLK


def _head_rows(W):
    """[480, m] -> [512, m]: head h rows 120h:120h+120 -> 128h:128h+120."""
    out = np.zeros((DP, W.shape[1]), W.dtype)
    for h in range(H):
        out[128 * h:128 * h + DH, :] = W[DH * h:DH * (h + 1), :]
    return out


def make_inmaps(inputs, shared=None, per_core=None, CBLK=None):
    if shared is None:
        shared, per_core, CBLK = preprocess(inputs)
    i32, f32 = np.int32, np.float32
    cenrep = np.broadcast_to(shared["centers"][None, :], (128, NB)).copy()
    na = shared["node_atom"]
    in_maps = []
    for c in range(NC):
        pc = per_core[c]
        naT = np.zeros((128, NBLK), i32)
        na_loc = np.zeros(NBLK * 128, i32)
        na_loc[:NPC] = na[c * NPC:(c + 1) * NPC]
        naT[:] = na_loc.reshape(NBLK, 128).T
        m = dict(
            atom_pad=shared["atom_pad"],
            idxT=pc["idxT"], naT=naT,
            pes=pc["pes"], ped=pc["ped"],
            SST=pc["SST"], Sg=pc["Sg"],
            cenrep=cenrep,
            wdeg16=shared["wdeg16"],
            Wd1=shared["Wd1"], Wd2=shared["Wd2"], Wd3=shared["Wd3"],
            W1=shared["W1"], W2=shared["W2"], W3=shared["W3"],
            Wall=shared["Wall"],
            Wh1=shared["Wh1"], Wh2=shared["Wh2"],
        )
        in_maps.append(m)
    return in_maps, CBLK


def build(CBLK, n_layers=L, n_blocks=NBLK, debug_dumps=()):
    C = n_blocks * CBLK
    nc = bass.Bass("TRN2")
    dt = {}

    def inp(name, shape, dtype):
        dt[name] = nc.dram_tensor(name, shape, dtype, kind="ExternalInput")
        return dt[name]

    inp("atom_pad", [64, DP], BF)
    inp("idxT", [128, C], I32)
    inp("pes", [128, C * 4], F32)
    inp("ped", [128, C * 4], F32)
    inp("naT", [128, NBLK], I32)
    inp("SST", [128, C * 256], BF)
    inp("Sg", [NBLK * 128, G], F32)
    inp("cenrep", [128, NB], F32)
    inp("wdeg16", [16, DP], BF)
    inp("Wd1", [NB, 64], BF)
    inp("Wd2", [64, 64], BF)
    inp("Wd3", [64, 4], BF)
    inp("W1", [L, NB, 64], BF)
    inp("W2", [L, 64, 64], BF)
    inp("W3", [L, 64, H], BF)
    inp("Wall", [L, 128, WCOLS], BF)
    inp("Wh1", [DP, DP], BF)
    inp("Wh2", [DP, 4], BF)

    energy_out = nc.dram_tensor("energy", [1, G], F32, kind="ExternalOutput")
    dumps = {}
    if "x" in debug_dumps:
        dumps["x"] = nc.dram_tensor("x_dump", [128, NBLK * DP], F32,
                                    kind="ExternalOutput")

    RG = [list(range(NC))]

    with TileContext(nc) as tc:
        with (
            tc.tile_pool(name="cst", bufs=1) as cst,
            tc.tile_pool(name="big", bufs=1) as big,
            tc.tile_pool(name="wp", bufs=1) as wp,
            tc.tile_pool(name="wpa", bufs=2) as wpa,
            tc.tile_pool(name="gp", bufs=1) as gp,
            tc.tile_pool(name="dram", bufs=1, space="DRAM") as dram,
        ):
            # ---------------- constants ----------------
            ident = cst.tile([128, 128], BF, tag="ident")
            make_identity(nc, ident[:])
            identf = cst.tile([128, 128], F32, tag="identf")
            make_identity(nc, identf[:])
            eps5 = cst.tile([128, 1], F32, tag="eps5")
            nc.vector.memset(eps5[:], 1e-5)
            cenrep = cst.tile([128, NB], F32, tag="cenrep")
            nc.sync.dma_start(out=cenrep[:], in_=dt["cenrep"][:])
            idxT_t = cst.tile([128, C], I32, tag="idxT")
            nc.sync.dma_start(out=idxT_t[:], in_=dt["idxT"][:])
            naT_t = cst.tile([128, NBLK], I32, tag="naT")
            nc.sync.dma_start(out=naT_t[:], in_=dt["naT"][:])
            wd1 = cst.tile([NB, 64], BF, tag="wd1")
            nc.sync.dma_start(out=wd1[:], in_=dt["Wd1"][:])
            wd2 = cst.tile([64, 64], BF, tag="wd2")
            nc.sync.dma_start(out=wd2[:], in_=dt["Wd2"][:])
            wd3 = cst.tile([64, 4], BF, tag="wd3")
            nc.sync.dma_start(out=wd3[:], in_=dt["Wd3"][:])
            w1g = cst.tile([NB, L * 64], BF, tag="w1g")
            nc.sync.dma_start(out=w1g[:].rearrange("k (l m) -> k l m", l=L),
                in_=dt["W1"].ap().rearrange("l k m -> k l m"))
            w2g = cst.tile([64, L * 64], BF, tag="w2g")
            nc.sync.dma_start(out=w2g[:].rearrange("k (l m) -> k l m", l=L),
                in_=dt["W2"].ap().rearrange("l k m -> k l m"))
            w3g = cst.tile([64, L * H], BF, tag="w3g")
            nc.sync.dma_start(out=w3g[:].rearrange("k (l m) -> k l m", l=L),
                in_=dt["W3"].ap().rearrange("l k m -> k l m"))
            wdeg16 = cst.tile([16, DP], BF, tag="wdeg16")
            nc.sync.dma_start(out=wdeg16[:], in_=dt["wdeg16"][:])
            # ---------------- persistent state ----------------
            x_t = big.tile([128, NBLK * DP], F32, tag="x")
            nc.vector.memset(x_t[:], 0.0)
            xT_t = big.tile([128, NBLK * DP], BF, tag="xT")
            q_t = big.tile([128, NBLK * DP], BF, tag="q")
            qsh_t = big.tile([128, NBLK * QSW], BF, tag="qsh")
            g0_t = big.tile([128, C], F32, tag="g0")
            rr_t = big.tile([128, C], F32, tag="rr")
            shpad = big.tile([128, C * 16], BF, tag="shpad")

            kvloc_d = dram.tile([NPC, 2 * DP], BF, tag="kvloc")
            rbf_d = dram.tile([128, C * 128], BF, tag="rbf_d")
            kvfull_d = nc.dram_tensor("kvfull_sh", [N, 2 * DP], BF,
                                      addr_space="Shared")
            eng_in_d = dram.tile([1, G], F32, tag="eng_in")
            eng_out_d = nc.dram_tensor("engout_sh", [1, G], F32,
                                       addr_space="Shared")

            # ============ PHASE 1: geometry ============
            with (
                tc.tile_pool(name="geo", bufs=1) as geo,
            ):
                shE = geo.tile([128, C * 12], F32, tag="shE")
                sh3 = shE[:].rearrange("p (c f) -> p c f", f=12)
                evi = geo.tile([128, C * 4], F32, tag="evi")
                ev3 = evi[:].rearrange("p (c f) -> p c f", f=4)
                tmp = geo.tile([128, C * 4], F32, tag="evtmp")
                tmp3 = tmp[:].rearrange("p (c f) -> p c f", f=4)
                uu = geo.tile([128, C * 3], F32, tag="uu")
                u3 = uu[:].rearrange("p (c f) -> p c f", f=3)
                rinv = geo.tile([128, C], F32, tag="rinv")

                pes_t = geo.tile([128, C * 4], F32, tag="pes")
                nc.sync.dma_start(out=pes_t[:], in_=dt["pes"][:])
                ped_t = geo.tile([128, C * 4], F32, tag="ped")
                nc.sync.dma_start(out=ped_t[:], in_=dt["ped"][:])
                nc.vector.tensor_tensor(out=evi[:], in0=pes_t[:], in1=ped_t[:],
                                        op=OP.subtract)
                nc.vector.tensor_tensor(out=tmp[:], in0=evi[:], in1=evi[:],
                                        op=OP.mult)
                nc.vector.tensor_reduce(out=ev3[:, :, 3:4], in_=tmp3[:, :, 0:3],
                                        op=OP.add, axis=AX)
                nc.scalar.activation(out=rr_t[:],
                                     in_=ev3[:, :, 3:4].rearrange("p c o -> p (c o)"),
                                     func=AF.Sqrt)
                radd = geo.tile([128, C], F32, tag="radd")
                nc.vector.tensor_scalar(out=radd[:], in0=rr_t[:], scalar1=1e-12,
                                        scalar2=None, op0=OP.add)
                nc.vector.reciprocal(out=rinv[:], in_=radd[:])
                nc.vector.tensor_tensor(
                    out=u3[:, :, 0:3], in0=ev3[:, :, 0:3],
                    in1=rinv[:].rearrange("p (c o) -> p c o", o=1).to_broadcast(
                        [128, C, 3]),
                    op=OP.mult)
                s3c, s5c, s15c = math.sqrt(3.0), math.sqrt(5.0), math.sqrt(15.0)
                nc.vector.memset(shE[:], 0.0)
                nc.vector.memset(sh3[:, :, 0:1].rearrange("p c o -> p (c o)"), 1.0)
                nc.vector.tensor_scalar(out=sh3[:, :, 1:4], in0=u3[:, :, 0:3],
                                        scalar1=s3c, scalar2=None, op0=OP.mult)
                nc.vector.scalar_tensor_tensor(out=sh3[:, :, 4:6], in0=u3[:, :, 0:2],
                                               scalar=s15c, in1=u3[:, :, 1:3],
                                               op0=OP.mult, op1=OP.mult)
                nc.vector.tensor_tensor(out=tmp3[:, :, 0:3], in0=u3[:, :, 0:3],
                                        in1=u3[:, :, 0:3], op=OP.mult)
                nc.vector.tensor_scalar(out=sh3[:, :, 6:7], in0=tmp3[:, :, 2:3],
                                        scalar1=1.5 * s5c, scalar2=-0.5 * s5c,
                                        op0=OP.mult, op1=OP.add)
                nc.vector.scalar_tensor_tensor(out=sh3[:, :, 7:8], in0=u3[:, :, 0:1],
                                               scalar=s15c, in1=u3[:, :, 2:3],
                                               op0=OP.mult, op1=OP.mult)
                nc.vector.tensor_tensor(out=sh3[:, :, 8:9], in0=tmp3[:, :, 0:1],
                                        in1=tmp3[:, :, 1:2], op=OP.subtract)
                nc.vector.tensor_scalar(
                    out=sh3[:, :, 8:9], in0=sh3[:, :, 8:9],
                    scalar1=0.5 * s15c, scalar2=None, op0=OP.mult)
                # shpad [128, C*16]: sh (9 comps) at cols cc*16+s, zero pad
                nc.vector.memset(shpad[:], 0.0)
                nc.scalar.copy(
                    out=shpad[:].rearrange("p (c w) -> p c w", w=16)[:, :, 0:SH],
                    in_=sh3[:, :, 0:SH])

            # ============ PHASE 2: rbf + gate MLPs ============
            with (
                tc.tile_pool(name="rw", bufs=4) as rw,
                tc.tile_pool(name="rw2", bufs=2) as rw2,
                tc.tile_pool(name="rps", bufs=2, space="PSUM") as rps,
                tc.tile_pool(name="rps2", bufs=2, space="PSUM") as rps2,
            ):
                for c0 in range(0, C, 4):
                    nb4 = min(4, C - c0)
                    rbfT = rw.tile([128, 4 * 128], BF, tag="rbfT")
                    for j in range(nb4):
                        cc = c0 + j
                        z = rw.tile([128, NB], F32, tag="z")
                        nc.vector.tensor_scalar(out=z[:], in0=cenrep[:],
                                                scalar1=rr_t[:, cc:cc + 1],
                                                scalar2=1.0 / WIDTH,
                                                op0=OP.subtract, op1=OP.mult)
                        z2 = rw.tile([128, NB], F32, tag="z2")
                        nc.vector.tensor_tensor(out=z2[:], in0=z[:], in1=z[:],
                                                op=OP.mult)
                        rbfe = rw.tile([128, NB], BF, tag="rbfe")
                        nc.scalar.activation(out=rbfe[:], in_=z2[:], func=AF.Exp,
                                             scale=-1.0)
                        rps_t = rps.tile([128, 128], BF, tag="rbf_ps")
                        nc.tensor.transpose(out=rps_t[:], in_=rbfe[:],
                                            identity=ident[:])
                        nc.vector.tensor_copy(out=rbfT[:, j * 128:(j + 1) * 128],
                                              in_=rps_t[:])
                    nc.sync.dma_start(out=rbf_d[:, c0 * 128:(c0 + nb4) * 128],
                                      in_=rbfT[:, 0:nb4 * 128])
                    h1ps = rps.tile([64, 4 * 128], F32, tag="h1ps")
                    nc.tensor.matmul(h1ps[:, 0:nb4 * 128], lhsT=wd1[:],
                                     rhs=rbfT[:, 0:nb4 * 128], start=True, stop=True)
                    h1sb = rw.tile([64, 4 * 128], BF, tag="h1sb")
                    nc.scalar.activation(out=h1sb[:, 0:nb4 * 128],
                                         in_=h1ps[:, 0:nb4 * 128], func=AF.Silu)
                    h2ps = rps.tile([64, 4 * 128], F32, tag="h2ps")
                    nc.tensor.matmul(h2ps[:, 0:nb4 * 128], lhsT=wd2[:],
                                     rhs=h1sb[:, 0:nb4 * 128], start=True, stop=True)
                    h2sb = rw2.tile([64, 4 * 128], BF, tag="h2sb")
                    nc.scalar.activation(out=h2sb[:, 0:nb4 * 128],
                                         in_=h2ps[:, 0:nb4 * 128], func=AF.Silu)
                    gps_o = rps2.tile([128, 16], F32, tag="gate_ps")
                    for j in range(nb4):
                        nc.tensor.matmul(
                            gps_o[:, j * 4:j * 4 + 4],
                            lhsT=h2sb[:, j * 128:(j + 1) * 128],
                            rhs=wd3[:], start=True, stop=True)
                    for j in range(nb4):
                        cc = c0 + j
                        nc.scalar.copy(out=g0_t[:, cc:cc + 1],
                                       in_=gps_o[:, j * 4:j * 4 + 1])

            # ============ PHASE 3: x0 + deg embedding ============
            with (
                tc.tile_pool(name="dw", bufs=3) as dw,
                tc.tile_pool(name="dw2", bufs=2) as dw2,
                tc.tile_pool(name="dps", bufs=2, space="PSUM") as dps,
                tc.tile_pool(name="dpsD", bufs=1, space="PSUM") as dpsD,
            ):
                for b in range(n_blocks):
                    sst = dw2.tile([128, CBLK * 256], BF, tag="sst")
                    nc.sync.dma_start(
                        out=sst[:],
                        in_=dt["SST"][:, b * CBLK * 256:(b + 1) * CBLK * 256])
                    x0g = dw.tile([128, DP], BF, tag="x0g")
                    nc.gpsimd.indirect_dma_start(
                        out=x0g[:], out_offset=None, in_=dt["atom_pad"][:],
                        in_offset=bass.IndirectOffsetOnAxis(ap=naT_t[:, b:b + 1],
                                                            axis=0))
                    shg0 = dw.tile([128, CBLK * 16], BF, tag="shg0")
                    shagg = dpsD.tile([128, 16], F32, tag="shagg")
                    for ch in range(CBLK):
                        cc = b * CBLK + ch
                        nc.vector.tensor_scalar(
                            out=shg0[:, ch * 16:(ch + 1) * 16],
                            in0=shpad[:, cc * 16:(cc + 1) * 16],
                            scalar1=g0_t[:, cc:cc + 1], scalar2=None, op0=OP.mult)
                        nc.tensor.matmul(
                            shagg[:], lhsT=sst[:, ch * 256 + 128:(ch + 1) * 256],
                            rhs=shg0[:, ch * 16:(ch + 1) * 16],
                            start=(ch == 0), stop=(ch == CBLK - 1))
                    shaggb = dw.tile([128, 16], BF, tag="shaggb")
                    nc.scalar.copy(out=shaggb[:], in_=shagg[:])
                    shaggT = dps.tile([128, 128], BF, tag="shaggT")
                    nc.tensor.transpose(out=shaggT[0:16, :], in_=shaggb[:],
                                        identity=ident[:])
                    shaggTb = dw.tile([16, 128], BF, tag="shaggTb")
                    nc.scalar.copy(out=shaggTb[:], in_=shaggT[0:16, :])
                    degps = dps.tile([128, DP], F32, tag="degps")
                    nc.tensor.matmul(degps[:], lhsT=shaggTb[:], rhs=wdeg16[:],
                                     start=True, stop=True)
                    nc.vector.scalar_tensor_tensor(
                        out=x_t[:, b * DP:(b + 1) * DP], in0=degps[:], scalar=CDEG,
                        in1=x0g[:], op0=OP.mult, op1=OP.add)

            if "x" in dumps and n_layers == 0:
                nc.sync.dma_start(out=dumps["x"][:], in_=x_t[:])

            # ============ PHASE 4: layers ============
            # wallA layout: q 0, k 2048, v 4096, qsh 6144 (cols); wallB: wo 0,
            # f1 2048, f2 6144.
            AW = OO          # wallA width (q|k|v|qsh)
            BW = WCOLS - OO  # wallB width (wo|f1|f2)

            def emit_kv_block(b, wallA_t, sb_pool, ps_pool):
                """xT transpose + k/v GEMMs + kvloc store for block b."""
                rows = min(128, NPC - 128 * b)
                xtp = ps_pool.tile([128, DP], F32, tag="ops")
                for f in range(4):
                    nc.tensor.transpose(
                        out=xtp[:, f * 128:(f + 1) * 128],
                        in_=x_t[:, b * DP + f * 128:b * DP + (f + 1) * 128],
                        identity=identf[:])
                nc.scalar.copy(out=xT_t[:, b * DP:(b + 1) * DP], in_=xtp[:])
                kvb = sb_pool.tile([128, 2 * DP], BF, tag="kvb")
                for nm, off, dst_sl in (("k", 2048, kvb[:, 0:DP]),
                                        ("v", 4096, kvb[:, DP:2 * DP])):
                    ps = ps_pool.tile([128, DP], F32, tag="ops")
                    for f in range(4):
                        nc.tensor.matmul(
                            ps[:],
                            lhsT=xT_t[:, b * DP + f * 128:b * DP + (f + 1) * 128],
                            rhs=wallA_t[:, off + f * DP:off + (f + 1) * DP],
                            start=(f == 0), stop=(f == 3))
                    if nm == "k":
                        nc.scalar.copy(out=dst_sl, in_=ps[:])
                    else:
                        nc.vector.tensor_copy(out=dst_sl, in_=ps[:])
                nc.sync.dma_start(out=kvloc_d[128 * b:128 * b + rows, :],
                                  in_=kvb[0:rows, :])

            wallA_cur = wpa.tile([128, AW], BF, tag="wallA")
            nc.sync.dma_start(out=wallA_cur[:], in_=dt["Wall"][0][:, 0:AW])
            with (
                tc.tile_pool(name="pw", bufs=3) as pw,
                tc.tile_pool(name="pps", bufs=2, space="PSUM") as pps,
            ):
                for b in range(n_blocks):
                    emit_kv_block(b, wallA_cur, pw, pps)
            if n_layers > 0:
                nc.gpsimd.collective_compute(
                    "AllGather", OP.bypass, ins=[kvloc_d[:].opt()],
                    outs=[kvfull_d[:].opt()], replica_groups=RG)

            for l in range(n_layers):
                wallB = wp.tile([128, BW], BF, tag="wallB")
                nc.sync.dma_start(out=wallB[:], in_=dt["Wall"][l][:, OO:WCOLS])
                if l + 1 < n_layers:
                    wallA_next = wpa.tile([128, AW], BF, tag="wallA")
                    nc.sync.dma_start(out=wallA_next[:],
                                      in_=dt["Wall"][l + 1][:, 0:AW])
                # q/qsh GEMMs (overlap the AllGather)
                with tc.tile_pool(name="qps", bufs=2, space="PSUM") as qps:
                    for b in range(n_blocks):
                        ps = qps.tile([128, DP], F32, tag="qp")
                        pss = qps.tile([128, QSW], F32, tag="qsp")
                        for f in range(4):
                            nc.tensor.matmul(
                                ps[:],
                                lhsT=xT_t[:, b * DP + f * 128:b * DP + (f + 1) * 128],
                                rhs=wallA_cur[:, f * DP:(f + 1) * DP],
                                start=(f == 0), stop=(f == 3))
                            nc.tensor.matmul(
                                pss[:],
                                lhsT=xT_t[:, b * DP + f * 128:b * DP + (f + 1) * 128],
                                rhs=wallA_cur[:, 6144 + f * QSW:6144 + (f + 1) * QSW],
                                start=(f == 0), stop=(f == 3))
                        nc.scalar.copy(out=q_t[:, b * DP:(b + 1) * DP], in_=ps[:])
                        nc.vector.tensor_copy(out=qsh_t[:, b * QSW:(b + 1) * QSW],
                                              in_=pss[:])
                # gate MLP for this layer (overlaps the AllGather)
                gate_l = gp.tile([128, C * 4], BF, tag="gate")
                with (
                    tc.tile_pool(name="glw", bufs=3) as glw,
                    tc.tile_pool(name="glps", bufs=2, space="PSUM") as glps,
                ):
                    for c0 in range(0, C, 4):
                        nb4 = min(4, C - c0)
                        rbfT = glw.tile([128, 4 * 128], BF, tag="rbfTl")
                        nc.sync.dma_start(out=rbfT[:, 0:nb4 * 128],
                                          in_=rbf_d[:, c0 * 128:(c0 + nb4) * 128])
                        h1ps = glps.tile([64, 4 * 128], F32, tag="h1ps")
                        nc.tensor.matmul(h1ps[:, 0:nb4 * 128],
                                         lhsT=w1g[:, l * 64:(l + 1) * 64],
                                         rhs=rbfT[:, 0:nb4 * 128],
                                         start=True, stop=True)
                        h1sb = glw.tile([64, 4 * 128], BF, tag="h1sb")
                        nc.scalar.activation(out=h1sb[:, 0:nb4 * 128],
                                             in_=h1ps[:, 0:nb4 * 128], func=AF.Silu)
                        h2ps = glps.tile([64, 4 * 128], F32, tag="h2ps")
                        nc.tensor.matmul(h2ps[:, 0:nb4 * 128],
                                         lhsT=w2g[:, l * 64:(l + 1) * 64],
                                         rhs=h1sb[:, 0:nb4 * 128],
                                         start=True, stop=True)
                        h2sb = glw.tile([64, 4 * 128], BF, tag="h2sb")
                        nc.scalar.activation(out=h2sb[:, 0:nb4 * 128],
                                             in_=h2ps[:, 0:nb4 * 128], func=AF.Silu)
                        gpo = glps.tile([128, 16], F32, tag="gpo")
                        for j in range(nb4):
                            nc.tensor.matmul(gpo[:, j * 4:(j + 1) * 4],
                                             lhsT=h2sb[:, j * 128:(j + 1) * 128],
                                             rhs=w3g[:, l * 4:(l + 1) * 4],
                                             start=True, stop=True)
                        nc.vector.tensor_scalar(
                            out=gate_l[:, c0 * 4:(c0 + nb4) * 4],
                            in0=gpo[:, 0:nb4 * 4], scalar1=INV, scalar2=None,
                            op0=OP.mult)

                # ---- edge phase ----
                with (
                    tc.tile_pool(name="ew", bufs=2) as ew,
                    tc.tile_pool(name="ew3", bufs=4) as ew3,
                    tc.tile_pool(name="ew2", bufs=2) as ew2,
                    tc.tile_pool(name="ekv", bufs=3) as ekv,
                    tc.tile_pool(name="eps_q", bufs=2, space="PSUM") as eps_q,
                    tc.tile_pool(name="eps_s", bufs=2, space="PSUM") as eps_s,
                    tc.tile_pool(name="eps_o", bufs=1, space="PSUM") as eps_o,
                    tc.tile_pool(name="epsD", bufs=1, space="PSUM") as epsD,
                    tc.tile_pool(name="epsT", bufs=1, space="PSUM") as epsT,
                ):
                    for b in range(n_blocks):
                        sst = ew2.tile([128, CBLK * 256], BF, tag="sst")
                        nc.sync.dma_start(
                            out=sst[:],
                            in_=dt["SST"][:, b * CBLK * 256:(b + 1) * CBLK * 256])
                        CH1 = (CBLK + 1) // 2
                        kvga = ekv.tile([128, CH1 * 1024], BF, tag="kvg")
                        kvgb = ekv.tile([128, CH1 * 1024], BF, tag="kvg")

                        def kv_sl(ch, w=1024):
                            t = kvga if ch < CH1 else kvgb
                            o = (ch if ch < CH1 else ch - CH1) * 1024
                            return t[:, o:o + w]

                        for ch in range(CBLK):
                            cc = b * CBLK + ch
                            nc.gpsimd.indirect_dma_start(
                                out=kv_sl(ch), out_offset=None, in_=kvfull_d[:],
                                in_offset=bass.IndirectOffsetOnAxis(
                                    ap=idxT_t[:, cc:cc + 1], axis=0))
                        lgall = ew.tile([128, CBLK * 4], F32, tag="lgall")
                        lgsha = ew.tile([128, CBLK * 4], F32, tag="lgsha")
                        qshb = ew.tile([128, CBLK * QSW], BF, tag="qshb")
                        astore = ew.tile([128, CBLK * 4], BF, tag="astore")
                        denps = epsD.tile([128, 4], F32, tag="denps")
                        aggps = epsD.tile([128, DP], F32, tag="aggps")
                        # pass 1: logits
                        for ch in range(CBLK):
                            cc = b * CBLK + ch
                            st_ap = sst[:, ch * 256:ch * 256 + 128]
                            qexp = eps_q.tile([128, DP], F32, tag="qexp")
                            nc.tensor.matmul(qexp[:], lhsT=st_ap,
                                             rhs=q_t[:, b * DP:(b + 1) * DP],
                                             start=True, stop=True)
                            qshe = eps_s.tile([128, QSW], F32, tag="qshe")
                            nc.tensor.matmul(qshe[:], lhsT=st_ap,
                                             rhs=qsh_t[:, b * QSW:(b + 1) * QSW],
                                             start=True, stop=True)
                            qexpb = ew3.tile([128, DP], BF, tag="qexpb")
                            nc.scalar.copy(out=qexpb[:], in_=qexp[:])
                            nc.scalar.copy(out=qshb[:, ch * QSW:(ch + 1) * QSW],
                                           in_=qshe[:])
                            mtj = ew3.tile([128, DP], BF, tag="mtj")
                            for h in range(4):
                                nc.vector.scalar_tensor_tensor(
                                    out=mtj[:, h * 128:(h + 1) * 128],
                                    in0=kv_sl(ch, DP)[:, h * 128:(h + 1) * 128],
                                    scalar=1.0,
                                    in1=qexpb[:, h * 128:(h + 1) * 128],
                                    op0=OP.mult, op1=OP.mult,
                                    accum_out=lgall[:, ch * 4 + h:ch * 4 + h + 1])
                        # batched sh-logit term for all chunks of this block
                        nc.vector.tensor_tensor(
                            out=qshb[:].rearrange("p (c h w) -> p c h w",
                                                  h=H, w=16),
                            in0=qshb[:].rearrange("p (c h w) -> p c h w",
                                                  h=H, w=16),
                            in1=shpad[:, b * CBLK * 16:(b + 1) * CBLK * 16]
                                .rearrange("p (c o w) -> p c o w", o=1, w=16)
                                .to_broadcast([128, CBLK, H, 16]),
                            op=OP.mult)
                        nc.vector.tensor_reduce(
                            out=lgsha[:].rearrange("p (a o) -> p a o", o=1),
                            in_=qshb[:].rearrange("p (a w) -> p a w", w=16),
                            op=OP.add, axis=AX)
                        # gate + exp + den
                        asb = ew.tile([128, CBLK * 4], F32, tag="asb")
                        nc.vector.tensor_tensor(out=asb[:], in0=lgall[:],
                                                in1=lgsha[:], op=OP.add)
                        nc.vector.tensor_tensor(
                            out=asb[:], in0=asb[:],
                            in1=gate_l[:, b * CBLK * 4:(b + 1) * CBLK * 4],
                            op=OP.mult)
                        astf = ew.tile([128, CBLK * 4], F32, tag="astf")
                        nc.scalar.activation(out=astf[:], in_=asb[:], func=AF.Exp)
                        nc.vector.tensor_copy(out=astore[:], in_=astf[:])
                        for ch in range(CBLK):
                            nc.tensor.matmul(
                                denps[:], lhsT=sst[:, ch * 256 + 128:(ch + 1) * 256],
                                rhs=astore[:, ch * 4:(ch + 1) * 4],
                                start=(ch == 0), stop=(ch == CBLK - 1))
                        dene = ew.tile([128, 4], F32, tag="dene")
                        nc.vector.tensor_scalar(out=dene[:], in0=denps[:],
                                                scalar1=1e-30, scalar2=None,
                                                op0=OP.add)
                        recf = ew.tile([128, 4], F32, tag="recf")
                        nc.vector.reciprocal(out=recf[:], in_=dene[:])
                        # pass 2: unnormalized messages + scatter
                        for ch in range(CBLK):
                            msgb = ew3.tile([128, DP], BF, tag="msgb")
                            vsl = kv_sl(ch)
                            for h in range(4):
                                src_sl = vsl[:, 512 + h * 128:512 + (h + 1) * 128]
                                dst_sl = msgb[:, h * 128:(h + 1) * 128]
                                a_col = astf[:, ch * 4 + h:ch * 4 + h + 1]
                                if h == 0:
                                    nc.scalar.mul(out=dst_sl, in_=src_sl, mul=a_col)
                                else:
                                    nc.vector.tensor_scalar(
                                        out=dst_sl, in0=src_sl, scalar1=a_col,
                                        scalar2=None, op0=OP.mult)
                            nc.tensor.matmul(
                                aggps[:], lhsT=sst[:, ch * 256 + 128:(ch + 1) * 256],
                                rhs=msgb[:], start=(ch == 0),
                                stop=(ch == CBLK - 1))
                        # normalize during PSUM evacuation
                        aggb = ew.tile([128, DP], BF, tag="aggb")
                        for h in range(4):
                            if h < 3:
                                nc.scalar.mul(out=aggb[:, h * 128:(h + 1) * 128],
                                              in_=aggps[:, h * 128:(h + 1) * 128],
                                              mul=recf[:, h:h + 1])
                            else:
                                nc.vector.tensor_scalar(
                                    out=aggb[:, h * 128:(h + 1) * 128],
                                    in0=aggps[:, h * 128:(h + 1) * 128],
                                    scalar1=recf[:, h:h + 1], scalar2=None,
                                    op0=OP.mult)
                        aggtp = epsT.tile([128, DP], BF, tag="aggtp")
                        for f in range(4):
                            nc.tensor.transpose(
                                out=aggtp[:, f * 128:(f + 1) * 128],
                                in_=aggb[:, f * 128:(f + 1) * 128],
                                identity=ident[:])
                        aggtb = ew.tile([128, DP], BF, tag="aggtb")
                        nc.vector.tensor_copy(out=aggtb[:], in_=aggtp[:])
                        ops_ = eps_o.tile([128, DP], F32, tag="ops")
                        for f in range(4):
                            nc.tensor.matmul(ops_[:],
                                             lhsT=aggtb[:, f * 128:(f + 1) * 128],
                                             rhs=wallB[:, f * DP:(f + 1) * DP],
                                             start=(f == 0), stop=(f == 3))
                        resid = ew.tile([128, DP], F32, tag="resid")
                        nc.vector.scalar_tensor_tensor(
                            out=resid[:], in0=ops_[:], scalar=CDEG,
                            in1=x_t[:, b * DP:(b + 1) * DP], op0=OP.mult, op1=OP.add)
                        _ln_bn(nc, ew, resid, x_t, b, eps5)
                        # FF block
                        xtp2 = eps_o.tile([128, DP], F32, tag="ops")
                        for f in range(4):
                            nc.tensor.transpose(
                                out=xtp2[:, f * 128:(f + 1) * 128],
                                in_=x_t[:, b * DP + f * 128:b * DP + (f + 1) * 128],
                                identity=identf[:])
                        xtb2 = ew.tile([128, DP], BF, tag="xtb2")
                        nc.scalar.copy(out=xtb2[:], in_=xtp2[:])
                        htb = ew.tile([128, FF], BF, tag="htb")
                        for g2 in range(2):
                            f1a = eps_o.tile([128, DP], F32, tag="ops")
                            for f in range(4):
                                nc.tensor.matmul(
                                    f1a[:],
                                    lhsT=xtb2[:, f * 128:(f + 1) * 128],
                                    rhs=wallB[:, 2048 + f * FF + g2 * DP:
                                              2048 + f * FF + (g2 + 1) * DP],
                                    start=(f == 0), stop=(f == 3))
                            hb = ew.tile([128, DP], BF, tag="hb")
                            nc.scalar.activation(out=hb[:], in_=f1a[:], func=AF.Silu)
                            htp = epsT.tile([128, DP], BF, tag="aggtp")
                            for f in range(4):
                                nc.tensor.transpose(
                                    out=htp[:, f * 128:(f + 1) * 128],
                                    in_=hb[:, f * 128:(f + 1) * 128],
                                    identity=ident[:])
                            nc.vector.tensor_copy(out=htb[:, g2 * DP:(g2 + 1) * DP],
                                                  in_=htp[:])
                        f2p = eps_o.tile([128, DP], F32, tag="ops")
                        for f in range(8):
                            nc.tensor.matmul(f2p[:],
                                             lhsT=htb[:, f * 128:(f + 1) * 128],
                                             rhs=wallB[:, 6144 + f * DP:6144 + (f + 1) * DP],
                                             start=(f == 0), stop=(f == 7))
                        resid2 = ew.tile([128, DP], F32, tag="resid")
                        nc.vector.tensor_tensor(out=resid2[:], in0=f2p[:],
                                                in1=x_t[:, b * DP:(b + 1) * DP],
                                                op=OP.add)
                        _ln_bn(nc, ew, resid2, x_t, b, eps5)
                        if l + 1 < n_layers:
                            emit_kv_block(b, wallA_next, ew, eps_o)
                if l + 1 < n_layers:
                    nc.gpsimd.collective_compute(
                        "AllGather", OP.bypass, ins=[kvloc_d[:].opt()],
                        outs=[kvfull_d[:].opt()], replica_groups=RG)
                    wallA_cur = wallA_next
                if "x" in dumps and l == n_layers - 1:
                    nc.sync.dma_start(out=dumps["x"][:], in_=x_t[:])

            # ============ PHASE 5: readout ============
            with (
                tc.tile_pool(name="fw", bufs=3) as fw,
                tc.tile_pool(name="fps", bufs=1, space="PSUM") as fps,
                tc.tile_pool(name="fpsD", bufs=1, space="PSUM") as fpsD,
            ):
                Sg_t = fw.tile([128, NBLK * G], F32, tag="Sg")
                nc.sync.dma_start(
                    out=Sg_t[:].rearrange("p (b g)   -> p b g", g=G),
                    in_=dt["Sg"].ap().rearrange("(b p) g -> p b g", p=128))
                wh1 = fw.tile([128, 4 * DP], BF, tag="wh1")
                nc.sync.dma_start(
                    out=wh1[:].rearrange("p (a m) -> p a m", a=4),
                    in_=dt["Wh1"].ap().rearrange("(a p) m -> p a m", p=128))
                wh2 = fw.tile([128, 4 * 4], BF, tag="wh2")
                nc.sync.dma_start(
                    out=wh2[:].rearrange("p (a m) -> p a m", a=4),
                    in_=dt["Wh2"].ap().rearrange("(a p) m -> p a m", p=128))
                engps = fpsD.tile([64, 4], F32, tag="engps")
                for b in range(n_blocks):
                    xtp = fps.tile([128, DP], F32, tag="xtp")
                    for f in range(4):
                        nc.tensor.transpose(
                            out=xtp[:, f * 128:(f + 1) * 128],
                            in_=x_t[:, b * DP + f * 128:b * DP + (f + 1) * 128],
                            identity=identf[:])
                    xtb = fw.tile([128, DP], BF, tag="xtb")
                    nc.scalar.copy(out=xtb[:], in_=xtp[:])
                    h1p = fps.tile([128, DP], F32, tag="h1p")
                    for f in range(4):
                        nc.tensor.matmul(h1p[:], lhsT=xtb[:, f * 128:(f + 1) * 128],
                                         rhs=wh1[:, f * DP:(f + 1) * DP],
                                         start=(f == 0), stop=(f == 3))
                    h1b = fw.tile([128, DP], BF, tag="h1b")
                    nc.scalar.activation(out=h1b[:], in_=h1p[:], func=AF.Silu)
                    h1tp = fps.tile([128, DP], BF, tag="h1tp")
                    for f in range(4):
                        nc.tensor.transpose(out=h1tp[:, f * 128:(f + 1) * 128],
                                            in_=h1b[:, f * 128:(f + 1) * 128],
                                            identity=ident[:])
                    h1tb = fw.tile([128, DP], BF, tag="h1tb")
                    nc.scalar.copy(out=h1tb[:], in_=h1tp[:])
                    nep = fps.tile([128, 4], F32, tag="nep")
                    for f in range(4):
                        nc.tensor.matmul(nep[:], lhsT=h1tb[:, f * 128:(f + 1) * 128],
                                         rhs=wh2[:, f * 4:(f + 1) * 4],
                                         start=(f == 0), stop=(f == 3))
                    nef = fw.tile([128, 4], F32, tag="nef")
                    nc.scalar.copy(out=nef[:], in_=nep[:])
                    nc.tensor.matmul(engps[:], lhsT=Sg_t[:, b * G:(b + 1) * G],
                                     rhs=nef[:], start=(b == 0),
                                     stop=(b == n_blocks - 1))
                engsb = fw.tile([64, 1], F32, tag="engsb")
                nc.scalar.mul(out=engsb[:], in_=engps[:, 0:1], mul=1.0 / AVG_NODES)
                engt = fps.tile([64, 64], F32, tag="engt")
                nc.tensor.transpose(out=engt[0:1, 0:64], in_=engsb[:],
                                    identity=identf[0:64, 0:64])
                engrow = fw.tile([1, 64], F32, tag="engrow")
                nc.scalar.copy(out=engrow[:], in_=engt[0:1, 0:64])
                nc.sync.dma_start(out=eng_in_d[:], in_=engrow[:])
                nc.gpsimd.collective_compute(
                    "AllReduce", OP.add, ins=[eng_in_d[:].opt()],
                    outs=[eng_out_d[:].opt()], replica_groups=RG)
                nc.sync.dma_start(out=energy_out[:], in_=eng_out_d[:])

    return nc


def _ln_bn(nc, pool, resid, x_t, b, eps_t):
    """LayerNorm over resid[:, :D] -> x_t[:, b*DP : b*DP+D] via bn_stats."""
    st6 = pool.tile([128, 6], F32, tag="st6")
    nc.vector.bn_stats(out=st6[:], in_=resid[:, 0:D])
    mv = pool.tile([128, 2], F32, tag="mv")
    nc.vector.bn_aggr(out=mv[:], in_=st6[:])
    stdv = pool.tile([128, 1], F32, tag="stdv")
    nc.scalar.activation(out=stdv[:], in_=mv[:, 1:2], func=AF.Sqrt,
                         bias=eps_t[:])
    rstd = pool.tile([128, 1], F32, tag="rstd")
    nc.vector.reciprocal(out=rstd[:], in_=stdv[:])
    nc.vector.tensor_scalar(out=x_t[:, b * DP:b * DP + D], in0=resid[:, 0:D],
                            scalar1=mv[:, 0:1], scalar2=rstd[:],
                            op0=OP.subtract, op1=OP.mult)


# ---------------------------------------------------------------------------
# entry point
# ---------------------------------------------------------------------------

def kernel(**inputs):
    shared, per_core, CBLK = preprocess(inputs)
    in_maps, _ = make_inmaps(inputs, shared, per_core, CBLK)
    nc = build(CBLK)
    split_multi_waits(nc)
    res = run_bass_kernel_spmd(nc, in_maps, core_ids=list(range(NC)))
    return np.asarray(res.results[0]["energy"][0], np.float32).reshape(G)


# revision 67
# speedup vs baseline: 1.0354x; 1.0017x over previous
"""TRN2 Bass kernel: DotProductAttentionTransformer (MD17-style GNN), 8-core SPMD.

Self-contained: host preprocessing (edge sorting/padding, selector matrices,
weight relayout) + Bass/Tile device program (edge-parallel attention with
S-matmul scatter/gather, bf16 GEMMs, fp32 softmax/LN).

v2: single merged k|v gather per edge chunk, spherical-harmonic logits folded
into the q side (Wqsh = Wq_h @ Wsh_h^T), fused multiply-reduce logits,
post-normalized aggregation, engine-balanced PSUM evacuations, and k/v-first
GEMM ordering so the AllGather overlaps q-side work.
"""
import math
import numpy as np
import ml_dtypes

import concourse.bass as bass
import concourse.mybir as mybir
import concourse.tile as tile_mod
from concourse.tile import TileContext
from concourse.masks import make_identity
from concourse.vector_clock import ScopedClock
from concourse.bass_utils import run_bass_kernel_spmd

bf16 = ml_dtypes.bfloat16

N, E, G, D, H, L = 10000, 160000, 64, 480, 4, 6
DH, NB, SH = 120, 128, 9
CUTOFF = 5.0
AVG_DEG = 15.57930850982666
AVG_NODES = 18.03065905448718
NC = 8
NPC = N // NC
NBLK = 10
DP = 512
FF = 1024
QSW = 64          # 4 heads x 16 (9 used) qsh columns
ONE_BF = np.float32(1.0).astype(bf16)

# ---------------------------------------------------------------------------
# harness patches: this walrus build allows only ONE sync-wait per
# instruction; split extras onto same-engine NoOps.
# ---------------------------------------------------------------------------

def _patched_drain_and_barrier(self, tick_clock, wait_clock):
    nc = self.nc
    drain_inst = nc.sync.drain()
    wait_clock.add_sem_waits(drain_inst.ins,
                             ScopedClock({None: tick_clock.global_clock}))
    si = drain_inst.ins.sync_info
    waits = list(si.on_wait or []) if si is not None else []
    if len(waits) > 1:
        id2sem = {h.num: h for h in self.sems.allocated().values()}
        si.on_wait = [waits[0]]
        for w in waits[1:]:
            nop = nc.sync.nop(nofuse=True)
            nop.wait_op(id2sem[w.id], w.wait_value, "sem-ge")
    nc.all_engine_barrier()
    popped = nc._tile_sem_poison_stack.pop()
    assert popped is self._sem_poison
    nc.clear_and_free_semaphores(list(self.sems.allocated().values()))
    nc.all_engine_barrier()


tile_mod.TileContext._drain_and_barrier = _patched_drain_and_barrier

_waitnop_counter = [0]


def split_multi_waits(nc):
    for f in nc.m.functions:
        for bb in f.blocks:
            insts = bb.instructions
            if not any(i.sync_info is not None and i.sync_info.on_wait
                       and len(i.sync_info.on_wait) > 1 for i in insts):
                continue
            new = []
            for inst in insts:
                si = inst.sync_info
                if si is not None and si.on_wait and len(si.on_wait) > 1:
                    waits = list(si.on_wait)
                    for w in waits[:-1]:
                        _waitnop_counter[0] += 1
                        nop = mybir.InstNoOp(
                            name=f"waitnop-{_waitnop_counter[0]}", ins=[], outs=[])
                        nop.engine = inst.engine
                        nop.sync_info = mybir.SyncInfo(on_wait=[w], on_update=[])
                        new.append(nop)
                    si.on_wait = [waits[-1]]
                new.append(inst)
            bb.instructions = new
    return nc


F32 = mybir.dt.float32
BF = mybir.dt.bfloat16
I32 = mybir.dt.int32
AX = mybir.AxisListType.X
OP = mybir.AluOpType
AF = mybir.ActivationFunctionType
INV = 1.0 / math.sqrt(DH)
CDEG = 1.0 / math.sqrt(AVG_DEG)
WIDTH = CUTOFF / NB

# packed layer-weight column offsets (bf16 [128, WCOLS])
OQ = 0
OK_ = OQ + 4 * DP
OV = OK_ + 4 * DP
OQS = OV + 4 * DP
OO = OQS + 4 * QSW
OF1 = OO + 4 * DP
OF2 = OF1 + 4 * FF
WCOLS = OF2 + 8 * DP


def head_pad_cols(W):
    """[in, 480] -> [in, 512]: head h cols 120h:120h+120 -> 128h:128h+120, pad zeros."""
    out = np.zeros((W.shape[0], DP), W.dtype)
    for h in range(H):
        out[:, 128 * h:128 * h + DH] = W[:, DH * h:DH * (h + 1)]
    return out


def plain_pad(W, rows, cols):
    out = np.zeros((rows, cols), W.dtype)
    out[:W.shape[0], :W.shape[1]] = W
    return out


def _amaj(W, a):
    """[a*128, m] -> [128, a*m] partition-major relayout for lhsT tiles."""
    return np.ascontiguousarray(
        W.reshape(a, 128, -1).transpose(1, 0, 2).reshape(128, -1))


def _balance_perm(dst):
    """Within-core node permutation equalizing per-block in-degree sums.

    Returns perm with perm[old_global_id] = new_global_id. The last block of
    each core has only 98 slots, so it is pre-seeded with a top-k/bottom-(98-k)
    degree mix that lands near the per-block average; the 128-slot blocks are
    then filled greedily (min load, then min count).
    """
    perm = np.empty(N, np.int64)
    caps = [128] * 9 + [NPC - 9 * 128]
    for c in range(NC):
        loc = dst[(dst >= c * NPC) & (dst < (c + 1) * NPC)] - c * NPC
        deg = np.bincount(loc, minlength=NPC).astype(np.int64)
        order = np.argsort(-deg, kind="stable")
        sdeg = deg[order]
        target = deg.sum() / NBLK
        pre_top = np.concatenate([[0], np.cumsum(sdeg)])
        pre_bot = np.concatenate([[0], np.cumsum(sdeg[::-1])])
        c9 = caps[9]
        bestk, bestsum = 0, -1
        for k in range(0, c9 + 1):
            s = pre_top[k] + pre_bot[c9 - k]
            if s <= target + 12 and s > bestsum:
                bestsum, bestk = s, k
        members = [[] for _ in range(NBLK)]
        assigned = np.zeros(NPC, bool)
        for n in np.concatenate([order[:bestk],
                                 order[NPC - (c9 - bestk):] if c9 > bestk
                                 else order[:0]]):
            members[9].append(n)
            assigned[n] = True
        load = [0.0] * NBLK
        load[9] = float(deg[np.array(members[9], np.int64)].sum()) \
            if members[9] else 0.0
        cnt = [len(m) for m in members]
        for n in order:
            if assigned[n]:
                continue
            best = min((b for b in range(9) if cnt[b] < 128),
                       key=lambda b: (load[b], cnt[b]))
            members[best].append(n)
            load[best] += float(deg[n])
            cnt[best] += 1
        for b in range(NBLK):
            base = c * NPC + b * 128
            for i, n in enumerate(members[b]):
                perm[c * NPC + n] = base + i
    return perm


def preprocess(inputs):
    """Returns (shared, per_core, CBLK) host arrays. Integer/relayout work only."""
    src = np.asarray(inputs["edge_src"]).astype(np.int64)
    dst = np.asarray(inputs["edge_dst"]).astype(np.int64)
    batch = np.asarray(inputs["batch"]).astype(np.int64)

    # rebalance node->block assignment to minimize the padded chunk count
    perm = _balance_perm(dst)
    inv = np.empty(N, np.int64)
    inv[perm] = np.arange(N)
    src = perm[src]
    dst = perm[dst]
    batch = batch[inv]
    pos_bal = np.asarray(inputs["pos"])[inv]
    natom_bal = np.asarray(inputs["node_atom"])[inv]

    order = np.argsort(dst, kind="stable")
    dsts, srcs = dst[order], src[order]

    # per (core, block) edge lists
    per_block = [[[] for _ in range(NBLK)] for _ in range(NC)]
    core_of = dsts // NPC
    loc = dsts - core_of * NPC
    blk = loc // 128
    for i in range(E):
        per_block[core_of[i]][blk[i]].append(i)

    CBLK = 0
    for c in range(NC):
        for b in range(NBLK):
            CBLK = max(CBLK, (len(per_block[c][b]) + 127) // 128)
    C = NBLK * CBLK

    per_core = []
    for c in range(NC):
        src_idx = np.zeros((NBLK, CBLK, 128), np.int64)
        dst_glob = np.zeros((NBLK, CBLK, 128), np.int64)
        dst_local = np.full((NBLK, CBLK, 128), -1, np.int64)
        for b in range(NBLK):
            el = per_block[c][b]
            for j, i in enumerate(el):
                ch, p = j // 128, j % 128
                src_idx[b, ch, p] = srcs[i]
                dst_local[b, ch, p] = loc[i] - 128 * b
                dst_glob[b, ch, p] = dsts[i]
        # S [e, n] and S_T [n, e] per chunk, bf16 {0,1}
        iota = np.arange(128)
        S = (dst_local[..., None] == iota[None, None, None, :]).astype(bf16)
        ST = np.ascontiguousarray(np.swapaxes(S, 2, 3))
        # pad edges: point ST/dst at the block's max-in-degree node so the
        # expanded q values stay finite (S stays zero -> no contribution).
        for b in range(NBLK):
            deg_b = np.zeros(128, np.int64)
            for ch in range(CBLK):
                vals = dst_local[b, ch]
                np.add.at(deg_b, vals[vals >= 0], 1)
            assert deg_b.max() > 0, f"block {b} of core {c} has no edges"
            nmax = int(deg_b.argmax())
            for ch in range(CBLK):
                padmask = dst_local[b, ch] < 0
                ST[b, ch, nmax, padmask] = ONE_BF
                dst_glob[b, ch][padmask] = c * NPC + 128 * b + nmax
        # combined [ST | S] per chunk: [128, C*256]
        SST = np.empty((128, C * 256), bf16)
        for b in range(NBLK):
            for ch in range(CBLK):
                cc = b * CBLK + ch
                SST[:, cc * 256:cc * 256 + 128] = ST[b, ch]
                SST[:, cc * 256 + 128:(cc + 1) * 256] = S[b, ch]
        idxT = np.ascontiguousarray(
            src_idx.reshape(C, 128).T).astype(np.int32)
        # host-gathered per-edge endpoint positions [128, C*4] (data movement
        # only; subtraction happens on device)
        pos4 = plain_pad(pos_bal.astype(np.float32), N, 4)
        pes = np.ascontiguousarray(
            pos4[src_idx.reshape(C, 128)].transpose(1, 0, 2).reshape(128, C * 4))
        ped = np.ascontiguousarray(
            pos4[dst_glob.reshape(C, 128)].transpose(1, 0, 2).reshape(128, C * 4))
        # graph one-hot for energy: [NBLK*128, G]
        Sg = np.zeros((NBLK * 128, G), np.float32)
        for nl in range(NPC):
            Sg[nl, batch[c * NPC + nl]] = 1.0
        per_core.append(dict(SST=SST, idxT=idxT, pes=pes, ped=ped, Sg=Sg))

    f32 = np.float32
    i = {k: np.asarray(v) for k, v in inputs.items()}

    # per-layer packed weights [L, 128, WCOLS]
    Wq = i["Wq"].astype(f32)
    Wk = i["Wk"].astype(f32)
    Wv = i["Wv"].astype(f32)
    Wsh = i["Wsh"].astype(f32)
    Wo = i["Wo"].astype(f32)
    Wf1 = i["Wf1"].astype(f32)
    Wf2 = i["Wf2"].astype(f32)
    Wall = np.zeros((L, 128, WCOLS), bf16)
    for l in range(L):
        Wqsh = np.zeros((DP, QSW), f32)
        for h in range(H):
            wq_h = Wq[l][:, DH * h:DH * (h + 1)]
            wsh_h = Wsh[l][:, DH * h:DH * (h + 1)]
            Wqsh[:D, 16 * h:16 * h + SH] = wq_h @ wsh_h.T
        parts = [
            _amaj(plain_pad(head_pad_cols(Wq[l]), DP, DP).astype(bf16), 4),
            _amaj(plain_pad(head_pad_cols(Wk[l]), DP, DP).astype(bf16), 4),
            _amaj(plain_pad(head_pad_cols(Wv[l]), DP, DP).astype(bf16), 4),
            _amaj(Wqsh.astype(bf16), 4),
            _amaj(plain_pad(_head_rows(Wo[l]), DP, DP).astype(bf16), 4),
            _amaj(plain_pad(Wf1[l], DP, FF).astype(bf16), 4),
            _amaj(plain_pad(Wf2[l], FF, DP).astype(bf16), 8),
        ]
        Wall[l] = np.concatenate(parts, axis=1)

    shared = dict(
        pos_pad=plain_pad(i["pos"].astype(f32), N, 64),
        atom_pad=plain_pad(i["atom_table"].astype(f32), 64, DP).astype(bf16),
        node_atom=natom_bal.astype(np.int32),
        wdeg16=plain_pad(i["Wdeg"].astype(f32), 16, DP).astype(bf16),
        Wd1=i["Wd1"].astype(bf16), Wd2=i["Wd2"].astype(bf16),
        Wd3=plain_pad(i["Wd3"].astype(f32), 64, 4).astype(bf16),
        W1=i["W1"].astype(bf16), W2=i["W2"].astype(bf16), W3=i["W3"].astype(bf16),
        Wall=Wall,
        Wh1=plain_pad(i["Wh1"].astype(f32), DP, DP).astype(bf16),
        Wh2=plain_pad(i["Wh2"].astype(f32), DP, 4).astype(bf16),
        centers=np.linspace(0, CUTOFF, NB).astype(f32),
    )
    return shared, per_core, CBLK


def _head_rows(W):
    """[480, m] -> [512, m]: head h rows 120h:120h+120 -> 128h:128h+120."""
    out = np.zeros((DP, W.shape[1]), W.dtype)
    for h in range(H):
        out[128 * h:128 * h + DH, :] = W[DH * h:DH * (h + 1), :]
    return out


def make_inmaps(inputs, shared=None, per_core=None, CBLK=None):
    if shared is None:
        shared, per_core, CBLK = preprocess(inputs)
    i32, f32 = np.int32, np.float32
    cenrep = np.broadcast_to(shared["centers"][None, :], (128, NB)).copy()
    na = shared["node_atom"]
    in_maps = []
    for c in range(NC):
        pc = per_core[c]
        naT = np.zeros((128, NBLK), i32)
        na_loc = np.zeros(NBLK * 128, i32)
        na_loc[:NPC] = na[c * NPC:(c + 1) * NPC]
        naT[:] = na_loc.reshape(NBLK, 128).T
        m = dict(
            atom_pad=shared["atom_pad"],
            idxT=pc["idxT"], naT=naT,
            pes=pc["pes"], ped=pc["ped"],
            SST=pc["SST"], Sg=pc["Sg"],
            cenrep=cenrep,
            wdeg16=shared["wdeg16"],
            Wd1=shared["Wd1"], Wd2=shared["Wd2"], Wd3=shared["Wd3"],
            W1=shared["W1"], W2=shared["W2"], W3=shared["W3"],
            Wall=shared["Wall"],
            Wh1=shared["Wh1"], Wh2=shared["Wh2"],
        )
        in_maps.append(m)
    return in_maps, CBLK


def build(CBLK, n_layers=L, n_blocks=NBLK, debug_dumps=()):
    C = n_blocks * CBLK
    nc = bass.Bass("TRN2")
    dt = {}

    def inp(name, shape, dtype):
        dt[name] = nc.dram_tensor(name, shape, dtype, kind="ExternalInput")
        return dt[name]

    inp("atom_pad", [64, DP], BF)
    inp("idxT", [128, C], I32)
    inp("pes", [128, C * 4], F32)
    inp("ped", [128, C * 4], F32)
    inp("naT", [128, NBLK], I32)
    inp("SST", [128, C * 256], BF)
    inp("Sg", [NBLK * 128, G], F32)
    inp("cenrep", [128, NB], F32)
    inp("wdeg16", [16, DP], BF)
    inp("Wd1", [NB, 64], BF)
    inp("Wd2", [64, 64], BF)
    inp("Wd3", [64, 4], BF)
    inp("W1", [L, NB, 64], BF)
    inp("W2", [L, 64, 64], BF)
    inp("W3", [L, 64, H], BF)
    inp("Wall", [L, 128, WCOLS], BF)
    inp("Wh1", [DP, DP], BF)
    inp("Wh2", [DP, 4], BF)

    energy_out = nc.dram_tensor("energy", [1, G], F32, kind="ExternalOutput")
    dumps = {}
    if "x" in debug_dumps:
        dumps["x"] = nc.dram_tensor("x_dump", [128, NBLK * DP], F32,
                                    kind="ExternalOutput")

    RG = [list(range(NC))]

    with TileContext(nc) as tc:
        with (
            tc.tile_pool(name="cst", bufs=1) as cst,
            tc.tile_pool(name="big", bufs=1) as big,
            tc.tile_pool(name="wp", bufs=1) as wp,
            tc.tile_pool(name="wpa", bufs=2) as wpa,
            tc.tile_pool(name="gp", bufs=1) as gp,
            tc.tile_pool(name="dram", bufs=1, space="DRAM") as dram,
        ):
            # ---------------- constants ----------------
            ident = cst.tile([128, 128], BF, tag="ident")
            make_identity(nc, ident[:])
            identf = cst.tile([128, 128], F32, tag="identf")
            make_identity(nc, identf[:])
            eps5 = cst.tile([128, 1], F32, tag="eps5")
            nc.vector.memset(eps5[:], 1e-5)
            cenrep = cst.tile([128, NB], F32, tag="cenrep")
            nc.sync.dma_start(out=cenrep[:], in_=dt["cenrep"][:])
            idxT_t = cst.tile([128, C], I32, tag="idxT")
            nc.sync.dma_start(out=idxT_t[:], in_=dt["idxT"][:])
            naT_t = cst.tile([128, NBLK], I32, tag="naT")
            nc.sync.dma_start(out=naT_t[:], in_=dt["naT"][:])
            wd1 = cst.tile([NB, 64], BF, tag="wd1")
            nc.sync.dma_start(out=wd1[:], in_=dt["Wd1"][:])
            wd2 = cst.tile([64, 64], BF, tag="wd2")
            nc.sync.dma_start(out=wd2[:], in_=dt["Wd2"][:])
            wd3 = cst.tile([64, 4], BF, tag="wd3")
            nc.sync.dma_start(out=wd3[:], in_=dt["Wd3"][:])
            w1g = cst.tile([NB, L * 64], BF, tag="w1g")
            nc.sync.dma_start(out=w1g[:].rearrange("k (l m) -> k l m", l=L),
                in_=dt["W1"].ap().rearrange("l k m -> k l m"))
            w2g = cst.tile([64, L * 64], BF, tag="w2g")
            nc.sync.dma_start(out=w2g[:].rearrange("k (l m) -> k l m", l=L),
                in_=dt["W2"].ap().rearrange("l k m -> k l m"))
            w3g = cst.tile([64, L * H], BF, tag="w3g")
            nc.sync.dma_start(out=w3g[:].rearrange("k (l m) -> k l m", l=L),
                in_=dt["W3"].ap().rearrange("l k m -> k l m"))
            wdeg16 = cst.tile([16, DP], BF, tag="wdeg16")
            nc.sync.dma_start(out=wdeg16[:], in_=dt["wdeg16"][:])
            # ---------------- persistent state ----------------
            x_t = big.tile([128, NBLK * DP], F32, tag="x")
            nc.vector.memset(x_t[:], 0.0)
            xT_t = big.tile([128, NBLK * DP], BF, tag="xT")
            q_t = big.tile([128, NBLK * DP], BF, tag="q")
            qsh_t = big.tile([128, NBLK * QSW], BF, tag="qsh")
            g0_t = big.tile([128, C], F32, tag="g0")
            rr_t = big.tile([128, C], F32, tag="rr")
            shpad = big.tile([128, C * 16], BF, tag="shpad")

            kvloc_d = dram.tile([NPC, 2 * DP], BF, tag="kvloc")
            rbf_d = dram.tile([128, C * 128], BF, tag="rbf_d")
            kvfull_d = nc.dram_tensor("kvfull_sh", [N, 2 * DP], BF,
                                      addr_space="Shared")
            eng_in_d = dram.tile([1, G], F32, tag="eng_in")
            eng_out_d = nc.dram_tensor("engout_sh", [1, G], F32,
                                       addr_space="Shared")

            # ============ PHASE 1: geometry ============
            with (
                tc.tile_pool(name="geo", bufs=1) as geo,
            ):
                shE = geo.tile([128, C * 12], F32, tag="shE")
                sh3 = shE[:].rearrange("p (c f) -> p c f", f=12)
                evi = geo.tile([128, C * 4], F32, tag="evi")
                ev3 = evi[:].rearrange("p (c f) -> p c f", f=4)
                tmp = geo.tile([128, C * 4], F32, tag="evtmp")
                tmp3 = tmp[:].rearrange("p (c f) -> p c f", f=4)
                uu = geo.tile([128, C * 3], F32, tag="uu")
                u3 = uu[:].rearrange("p (c f) -> p c f", f=3)
                rinv = geo.tile([128, C], F32, tag="rinv")

                pes_t = geo.tile([128, C * 4], F32, tag="pes")
                nc.sync.dma_start(out=pes_t[:], in_=dt["pes"][:])
                ped_t = geo.tile([128, C * 4], F32, tag="ped")
                nc.sync.dma_start(out=ped_t[:], in_=dt["ped"][:])
                nc.vector.tensor_tensor(out=evi[:], in0=pes_t[:], in1=ped_t[:],
                                        op=OP.subtract)
                nc.vector.tensor_tensor(out=tmp[:], in0=evi[:], in1=evi[:],
                                        op=OP.mult)
                nc.vector.tensor_reduce(out=ev3[:, :, 3:4], in_=tmp3[:, :, 0:3],
                                        op=OP.add, axis=AX)
                nc.scalar.activation(out=rr_t[:],
                                     in_=ev3[:, :, 3:4].rearrange("p c o -> p (c o)"),
                                     func=AF.Sqrt)
                radd = geo.tile([128, C], F32, tag="radd")
                nc.vector.tensor_scalar(out=radd[:], in0=rr_t[:], scalar1=1e-12,
                                        scalar2=None, op0=OP.add)
                nc.vector.reciprocal(out=rinv[:], in_=radd[:])
                nc.vector.tensor_tensor(
                    out=u3[:, :, 0:3], in0=ev3[:, :, 0:3],
                    in1=rinv[:].rearrange("p (c o) -> p c o", o=1).to_broadcast(
                        [128, C, 3]),
                    op=OP.mult)
                s3c, s5c, s15c = math.sqrt(3.0), math.sqrt(5.0), math.sqrt(15.0)
                nc.vector.memset(shE[:], 0.0)
                nc.vector.memset(sh3[:, :, 0:1].rearrange("p c o -> p (c o)"), 1.0)
                nc.vector.tensor_scalar(out=sh3[:, :, 1:4], in0=u3[:, :, 0:3],
                                        scalar1=s3c, scalar2=None, op0=OP.mult)
                nc.vector.scalar_tensor_tensor(out=sh3[:, :, 4:6], in0=u3[:, :, 0:2],
                                               scalar=s15c, in1=u3[:, :, 1:3],
                                               op0=OP.mult, op1=OP.mult)
                nc.vector.tensor_tensor(out=tmp3[:, :, 0:3], in0=u3[:, :, 0:3],
                                        in1=u3[:, :, 0:3], op=OP.mult)
                nc.vector.tensor_scalar(out=sh3[:, :, 6:7], in0=tmp3[:, :, 2:3],
                                        scalar1=1.5 * s5c, scalar2=-0.5 * s5c,
                                        op0=OP.mult, op1=OP.add)
                nc.vector.scalar_tensor_tensor(out=sh3[:, :, 7:8], in0=u3[:, :, 0:1],
                                               scalar=s15c, in1=u3[:, :, 2:3],
                                               op0=OP.mult, op1=OP.mult)
                nc.vector.tensor_tensor(out=sh3[:, :, 8:9], in0=tmp3[:, :, 0:1],
                                        in1=tmp3[:, :, 1:2], op=OP.subtract)
                nc.vector.tensor_scalar(
                    out=sh3[:, :, 8:9], in0=sh3[:, :, 8:9],
                    scalar1=0.5 * s15c, scalar2=None, op0=OP.mult)
                # shpad [128, C*16]: sh (9 comps) at cols cc*16+s, zero pad
                nc.vector.memset(shpad[:], 0.0)
                nc.scalar.copy(
                    out=shpad[:].rearrange("p (c w) -> p c w", w=16)[:, :, 0:SH],
                    in_=sh3[:, :, 0:SH])

            # ============ PHASE 2: rbf + gate MLPs ============
            with (
                tc.tile_pool(name="rw", bufs=4) as rw,
                tc.tile_pool(name="rw2", bufs=2) as rw2,
                tc.tile_pool(name="rps", bufs=2, space="PSUM") as rps,
                tc.tile_pool(name="rps2", bufs=2, space="PSUM") as rps2,
            ):
                for c0 in range(0, C, 4):
                    nb4 = min(4, C - c0)
                    rbfT = rw.tile([128, 4 * 128], BF, tag="rbfT")
                    for j in range(nb4):
                        cc = c0 + j
                        z = rw.tile([128, NB], F32, tag="z")
                        nc.vector.tensor_scalar(out=z[:], in0=cenrep[:],
                                                scalar1=rr_t[:, cc:cc + 1],
                                                scalar2=1.0 / WIDTH,
                                                op0=OP.subtract, op1=OP.mult)
                        z2 = rw.tile([128, NB], F32, tag="z2")
                        nc.vector.tensor_tensor(out=z2[:], in0=z[:], in1=z[:],
                                                op=OP.mult)
                        rbfe = rw.tile([128, NB], BF, tag="rbfe")
                        nc.scalar.activation(out=rbfe[:], in_=z2[:], func=AF.Exp,
                                             scale=-1.0)
                        rps_t = rps.tile([128, 128], BF, tag="rbf_ps")
                        nc.tensor.transpose(out=rps_t[:], in_=rbfe[:],
                                            identity=ident[:])
                        nc.vector.tensor_copy(out=rbfT[:, j * 128:(j + 1) * 128],
                                              in_=rps_t[:])
                    nc.sync.dma_start(out=rbf_d[:, c0 * 128:(c0 + nb4) * 128],
                                      in_=rbfT[:, 0:nb4 * 128])
                    h1ps = rps.tile([64, 4 * 128], F32, tag="h1ps")
                    nc.tensor.matmul(h1ps[:, 0:nb4 * 128], lhsT=wd1[:],
                                     rhs=rbfT[:, 0:nb4 * 128], start=True, stop=True)
                    h1sb = rw.tile([64, 4 * 128], BF, tag="h1sb")
                    nc.scalar.activation(out=h1sb[:, 0:nb4 * 128],
                                         in_=h1ps[:, 0:nb4 * 128], func=AF.Silu)
                    h2ps = rps.tile([64, 4 * 128], F32, tag="h2ps")
                    nc.tensor.matmul(h2ps[:, 0:nb4 * 128], lhsT=wd2[:],
                                     rhs=h1sb[:, 0:nb4 * 128], start=True, stop=True)
                    h2sb = rw2.tile([64, 4 * 128], BF, tag="h2sb")
                    nc.scalar.activation(out=h2sb[:, 0:nb4 * 128],
                                         in_=h2ps[:, 0:nb4 * 128], func=AF.Silu)
                    gps_o = rps2.tile([128, 16], F32, tag="gate_ps")
                    for j in range(nb4):
                        nc.tensor.matmul(
                            gps_o[:, j * 4:j * 4 + 4],
                            lhsT=h2sb[:, j * 128:(j + 1) * 128],
                            rhs=wd3[:], start=True, stop=True)
                    for j in range(nb4):
                        cc = c0 + j
                        nc.scalar.copy(out=g0_t[:, cc:cc + 1],
                                       in_=gps_o[:, j * 4:j * 4 + 1])

            # ============ PHASE 3: x0 + deg embedding ============
            with (
                tc.tile_pool(name="dw", bufs=3) as dw,
                tc.tile_pool(name="dw2", bufs=2) as dw2,
                tc.tile_pool(name="dps", bufs=2, space="PSUM") as dps,
                tc.tile_pool(name="dpsD", bufs=1, space="PSUM") as dpsD,
            ):
                for b in range(n_blocks):
                    sst = dw2.tile([128, CBLK * 256], BF, tag="sst")
                    nc.sync.dma_start(
                        out=sst[:],
                        in_=dt["SST"][:, b * CBLK * 256:(b + 1) * CBLK * 256])
                    x0g = dw.tile([128, DP], BF, tag="x0g")
                    nc.gpsimd.indirect_dma_start(
                        out=x0g[:], out_offset=None, in_=dt["atom_pad"][:],
                        in_offset=bass.IndirectOffsetOnAxis(ap=naT_t[:, b:b + 1],
                                                            axis=0))
                    shg0 = dw.tile([128, CBLK * 16], BF, tag="shg0")
                    shagg = dpsD.tile([128, 16], F32, tag="shagg")
                    for ch in range(CBLK):
                        cc = b * CBLK + ch
                        nc.vector.tensor_scalar(
                            out=shg0[:, ch * 16:(ch + 1) * 16],
                            in0=shpad[:, cc * 16:(cc + 1) * 16],
                            scalar1=g0_t[:, cc:cc + 1], scalar2=None, op0=OP.mult)
                        nc.tensor.matmul(
                            shagg[:], lhsT=sst[:, ch * 256 + 128:(ch + 1) * 256],
                            rhs=shg0[:, ch * 16:(ch + 1) * 16],
                            start=(ch == 0), stop=(ch == CBLK - 1))
                    shaggb = dw.tile([128, 16], BF, tag="shaggb")
                    nc.scalar.copy(out=shaggb[:], in_=shagg[:])
                    shaggT = dps.tile([128, 128], BF, tag="shaggT")
                    nc.tensor.transpose(out=shaggT[0:16, :], in_=shaggb[:],
                                        identity=ident[:])
                    shaggTb = dw.tile([16, 128], BF, tag="shaggTb")
                    nc.scalar.copy(out=shaggTb[:], in_=shaggT[0:16, :])
                    degps = dps.tile([128, DP], F32, tag="degps")
                    nc.tensor.matmul(degps[:], lhsT=shaggTb[:], rhs=wdeg16[:],
                                     start=True, stop=True)
                    nc.vector.scalar_tensor_tensor(
                        out=x_t[:, b * DP:(b + 1) * DP], in0=degps[:], scalar=CDEG,
                        in1=x0g[:], op0=OP.mult, op1=OP.add)

            if "x" in dumps and n_layers == 0:
                nc.sync.dma_start(out=dumps["x"][:], in_=x_t[:])

            # ============ PHASE 4: layers ============
            # wallA layout: q 0, k 2048, v 4096, qsh 6144 (cols); wallB: wo 0,
            # f1 2048, f2 6144.
            AW = OO          # wallA width (q|k|v|qsh)
            BW = WCOLS - OO  # wallB width (wo|f1|f2)

            def emit_kv_block(b, wallA_t, sb_pool, ps_pool):
                """xT transpose + k/v GEMMs + kvloc store for block b."""
                rows = min(128, NPC - 128 * b)
                xtp = ps_pool.tile([128, DP], F32, tag="ops")
                for f in range(4):
                    nc.tensor.transpose(
                        out=xtp[:, f * 128:(f + 1) * 128],
                        in_=x_t[:, b * DP + f * 128:b * DP + (f + 1) * 128],
                        identity=identf[:])
                nc.scalar.copy(out=xT_t[:, b * DP:(b + 1) * DP], in_=xtp[:])
                kvb = sb_pool.tile([128, 2 * DP], BF, tag="kvb")
                for nm, off, dst_sl in (("k", 2048, kvb[:, 0:DP]),
                                        ("v", 4096, kvb[:, DP:2 * DP])):
                    ps = ps_pool.tile([128, DP], F32, tag="ops")
                    for f in range(4):
                        nc.tensor.matmul(
                            ps[:],
                            lhsT=xT_t[:, b * DP + f * 128:b * DP + (f + 1) * 128],
                            rhs=wallA_t[:, off + f * DP:off + (f + 1) * DP],
                            start=(f == 0), stop=(f == 3))
                    if nm == "k":
                        nc.scalar.copy(out=dst_sl, in_=ps[:])
                    else:
                        nc.vector.tensor_copy(out=dst_sl, in_=ps[:])
                nc.sync.dma_start(out=kvloc_d[128 * b:128 * b + rows, :],
                                  in_=kvb[0:rows, :])

            wallA_cur = wpa.tile([128, AW], BF, tag="wallA")
            nc.sync.dma_start(out=wallA_cur[:], in_=dt["Wall"][0][:, 0:AW])
            with (
                tc.tile_pool(name="pw", bufs=3) as pw,
                tc.tile_pool(name="pps", bufs=2, space="PSUM") as pps,
            ):
                for b in range(n_blocks):
                    emit_kv_block(b, wallA_cur, pw, pps)
            if n_layers > 0:
                nc.gpsimd.collective_compute(
                    "AllGather", OP.bypass, ins=[kvloc_d[:].opt()],
                    outs=[kvfull_d[:].opt()], replica_groups=RG)

            for l in range(n_layers):
                wallB = wp.tile([128, BW], BF, tag="wallB")
                nc.sync.dma_start(out=wallB[:], in_=dt["Wall"][l][:, OO:WCOLS])
                if l + 1 < n_layers:
                    wallA_next = wpa.tile([128, AW], BF, tag="wallA")
                    nc.sync.dma_start(out=wallA_next[:],
                                      in_=dt["Wall"][l + 1][:, 0:AW])
                # q/qsh GEMMs (overlap the AllGather)
                with tc.tile_pool(name="qps", bufs=2, space="PSUM") as qps:
                    for b in range(n_blocks):
                        ps = qps.tile([128, DP], F32, tag="qp")
                        pss = qps.tile([128, QSW], F32, tag="qsp")
                        for f in range(4):
                            nc.tensor.matmul(
                                ps[:],
                                lhsT=xT_t[:, b * DP + f * 128:b * DP + (f + 1) * 128],
                                rhs=wallA_cur[:, f * DP:(f + 1) * DP],
                                start=(f == 0), stop=(f == 3))
                            nc.tensor.matmul(
                                pss[:],
                                lhsT=xT_t[:, b * DP + f * 128:b * DP + (f + 1) * 128],
                                rhs=wallA_cur[:, 6144 + f * QSW:6144 + (f + 1) * QSW],
                                start=(f == 0), stop=(f == 3))
                        nc.scalar.copy(out=q_t[:, b * DP:(b + 1) * DP], in_=ps[:])
                        nc.vector.tensor_copy(out=qsh_t[:, b * QSW:(b + 1) * QSW],
                                              in_=pss[:])
                # gate MLP for this layer (overlaps the AllGather)
                gate_l = gp.tile([128, C * 4], BF, tag="gate")
                with (
                    tc.tile_pool(name="glw", bufs=3) as glw,
                    tc.tile_pool(name="glps", bufs=2, space="PSUM") as glps,
                ):
                    for c0 in range(0, C, 4):
                        nb4 = min(4, C - c0)
                        rbfT = glw.tile([128, 4 * 128], BF, tag="rbfTl")
                        nc.sync.dma_start(out=rbfT[:, 0:nb4 * 128],
                                          in_=rbf_d[:, c0 * 128:(c0 + nb4) * 128])
                        h1ps = glps.tile([64, 4 * 128], F32, tag="h1ps")
                        nc.tensor.matmul(h1ps[:, 0:nb4 * 128],
                                         lhsT=w1g[:, l * 64:(l + 1) * 64],
                                         rhs=rbfT[:, 0:nb4 * 128],
                                         start=True, stop=True)
                        h1sb = glw.tile([64, 4 * 128], BF, tag="h1sb")
                        nc.scalar.activation(out=h1sb[:, 0:nb4 * 128],
                                             in_=h1ps[:, 0:nb4 * 128], func=AF.Silu)
                        h2ps = glps.tile([64, 4 * 128], F32, tag="h2ps")
                        nc.tensor.matmul(h2ps[:, 0:nb4 * 128],
                                         lhsT=w2g[:, l * 64:(l + 1) * 64],
                                         rhs=h1sb[:, 0:nb4 * 128],
                                         start=True, stop=True)
                        h2sb = glw.tile([64, 4 * 128], BF, tag="h2sb")
                        nc.scalar.activation(out=h2sb[:, 0:nb4 * 128],
                                             in_=h2ps[:, 0:nb4 * 128], func=AF.Silu)
                        gpo = glps.tile([128, 16], F32, tag="gpo")
                        for j in range(nb4):
                            nc.tensor.matmul(gpo[:, j * 4:(j + 1) * 4],
                                             lhsT=h2sb[:, j * 128:(j + 1) * 128],
                                             rhs=w3g[:, l * 4:(l + 1) * 4],
                                             start=True, stop=True)
                        nc.vector.tensor_scalar(
                            out=gate_l[:, c0 * 4:(c0 + nb4) * 4],
                            in0=gpo[:, 0:nb4 * 4], scalar1=INV, scalar2=None,
                            op0=OP.mult)

                # ---- edge phase ----
                with (
                    tc.tile_pool(name="ew", bufs=2) as ew,
                    tc.tile_pool(name="ew3", bufs=4) as ew3,
                    tc.tile_pool(name="ew2", bufs=2) as ew2,
                    tc.tile_pool(name="ekv", bufs=3) as ekv,
                    tc.tile_pool(name="eps_q", bufs=2, space="PSUM") as eps_q,
                    tc.tile_pool(name="eps_s", bufs=2, space="PSUM") as eps_s,
                    tc.tile_pool(name="eps_o", bufs=1, space="PSUM") as eps_o,
                    tc.tile_pool(name="epsD", bufs=1, space="PSUM") as epsD,
                    tc.tile_pool(name="epsT", bufs=1, space="PSUM") as epsT,
                ):
                    for b in range(n_blocks):
                        sst = ew2.tile([128, CBLK * 256], BF, tag="sst")
                        nc.sync.dma_start(
                            out=sst[:],
                            in_=dt["SST"][:, b * CBLK * 256:(b + 1) * CBLK * 256])
                        CH1 = (CBLK + 1) // 2
                        kvga = ekv.tile([128, CH1 * 1024], BF, tag="kvg")
                        kvgb = ekv.tile([128, CH1 * 1024], BF, tag="kvg")

                        def kv_sl(ch, w=1024):
                            t = kvga if ch < CH1 else kvgb
                            o = (ch if ch < CH1 else ch - CH1) * 1024
                            return t[:, o:o + w]

                        for ch in range(CBLK):
                            cc = b * CBLK + ch
                            nc.gpsimd.indirect_dma_start(
                                out=kv_sl(ch), out_offset=None, in_=kvfull_d[:],
                                in_offset=bass.IndirectOffsetOnAxis(
                                    ap=idxT_t[:, cc:cc + 1], axis=0))
                        lgall = ew.tile([128, CBLK * 4], F32, tag="lgall")
                        lgsha = ew.tile([128, CBLK * 4], F32, tag="lgsha")
                        qshb = ew.tile([128, CBLK * QSW], BF, tag="qshb")
                        astore = ew.tile([128, CBLK * 4], BF, tag="astore")
                        denps = epsD.tile([128, 4], F32, tag="denps")
                        aggps = epsD.tile([128, DP], F32, tag="aggps")
                        # pass 1: logits
                        for ch in range(CBLK):
                            cc = b * CBLK + ch
                            st_ap = sst[:, ch * 256:ch * 256 + 128]
                            qexp = eps_q.tile([128, DP], F32, tag="qexp")
                            nc.tensor.matmul(qexp[:], lhsT=st_ap,
                                             rhs=q_t[:, b * DP:(b + 1) * DP],
                                             start=True, stop=True)
                            qshe = eps_s.tile([128, QSW], F32, tag="qshe")
                            nc.tensor.matmul(qshe[:], lhsT=st_ap,
                                             rhs=qsh_t[:, b * QSW:(b + 1) * QSW],
                                             start=True, stop=True)
                            qexpb = ew3.tile([128, DP], BF, tag="qexpb")
                            nc.scalar.copy(out=qexpb[:], in_=qexp[:])
                            nc.scalar.copy(out=qshb[:, ch * QSW:(ch + 1) * QSW],
                                           in_=qshe[:])
                            mtj = ew3.tile([128, DP], BF, tag="mtj")
                            for h in range(4):
                                nc.vector.scalar_tensor_tensor(
                                    out=mtj[:, h * 128:(h + 1) * 128],
                                    in0=kv_sl(ch, DP)[:, h * 128:(h + 1) * 128],
                                    scalar=1.0,
                                    in1=qexpb[:, h * 128:(h + 1) * 128],
                                    op0=OP.mult, op1=OP.mult,
                                    accum_out=lgall[:, ch * 4 + h:ch * 4 + h + 1])
                        # batched sh-logit term for all chunks of this block
                        nc.vector.tensor_tensor(
                            out=qshb[:].rearrange("p (c h w) -> p c h w",
                                                  h=H, w=16),
                            in0=qshb[:].rearrange("p (c h w) -> p c h w",
                                                  h=H, w=16),
                            in1=shpad[:, b * CBLK * 16:(b + 1) * CBLK * 16]
                                .rearrange("p (c o w) -> p c o w", o=1, w=16)
                                .to_broadcast([128, CBLK, H, 16]),
                            op=OP.mult)
                        nc.vector.tensor_reduce(
                            out=lgsha[:].rearrange("p (a o) -> p a o", o=1),
                            in_=qshb[:].rearrange("p (a w) -> p a w", w=16),
                            op=OP.add, axis=AX)
                        # gate + exp + den
                        asb = ew.tile([128, CBLK * 4], F32, tag="asb")
                        nc.vector.tensor_tensor(out=asb[:], in0=lgall[:],
                                                in1=lgsha[:], op=OP.add)
                        nc.vector.tensor_tensor(
                            out=asb[:], in0=asb[:],
                            in1=gate_l[:, b * CBLK * 4:(b + 1) * CBLK * 4],
                            op=OP.mult)
                        astf = ew.tile([128, CBLK * 4], F32, tag="astf")
                        nc.scalar.activation(out=astf[:], in_=asb[:], func=AF.Exp)
                        nc.vector.tensor_copy(out=astore[:], in_=astf[:])
                        for ch in range(CBLK):
                            nc.tensor.matmul(
                                denps[:], lhsT=sst[:, ch * 256 + 128:(ch + 1) * 256],
                                rhs=astore[:, ch * 4:(ch + 1) * 4],
                                start=(ch == 0), stop=(ch == CBLK - 1))
                        # pass 2: unnormalized messages + scatter
                        for ch in range(CBLK):
                            msgb = ew3.tile([128, DP], BF, tag="msgb")
                            vsl = kv_sl(ch)
                            for h in range(4):
                                src_sl = vsl[:, 512 + h * 128:512 + (h + 1) * 128]
                                dst_sl = msgb[:, h * 128:(h + 1) * 128]
                                a_col = astf[:, ch * 4 + h:ch * 4 + h + 1]
                                if h == 0:
                                    nc.scalar.mul(out=dst_sl, in_=src_sl, mul=a_col)
                                else:
                                    nc.vector.tensor_scalar(
                                        out=dst_sl, in0=src_sl, scalar1=a_col,
                                        scalar2=None, op0=OP.mult)
                            nc.tensor.matmul(
                                aggps[:], lhsT=sst[:, ch * 256 + 128:(ch + 1) * 256],
                                rhs=msgb[:], start=(ch == 0),
                                stop=(ch == CBLK - 1))
                        # normalize during PSUM evacuation
                        dene = ew.tile([128, 4], F32, tag="dene")
                        nc.vector.tensor_scalar(out=dene[:], in0=denps[:],
                                                scalar1=1e-30, scalar2=None,
                                                op0=OP.add)
                        recf = ew.tile([128, 4], F32, tag="recf")
                        nc.vector.reciprocal(out=recf[:], in_=dene[:])
                        aggb = ew.tile([128, DP], BF, tag="aggb")
                        for h in range(4):
                            if h < 3:
                                nc.scalar.mul(out=aggb[:, h * 128:(h + 1) * 128],
                                              in_=aggps[:, h * 128:(h + 1) * 128],
                                              mul=recf[:, h:h + 1])
                            else:
                                nc.vector.tensor_scalar(
                                    out=aggb[:, h * 128:(h + 1) * 128],
                                    in0=aggps[:, h * 128:(h + 1) * 128],
                                    scalar1=recf[:, h:h + 1], scalar2=None,
                                    op0=OP.mult)
                        aggtp = epsT.tile([128, DP], BF, tag="aggtp")
                        for f in range(4):
                            nc.tensor.transpose(
                                out=aggtp[:, f * 128:(f + 1) * 128],
                                in_=aggb[:, f * 128:(f + 1) * 128],
                                identity=ident[:])
                        aggtb = ew.tile([128, DP], BF, tag="aggtb")
                        nc.vector.tensor_copy(out=aggtb[:], in_=aggtp[:])
                        ops_ = eps_o.tile([128, DP], F32, tag="ops")
                        for f in range(4):
                            nc.tensor.matmul(ops_[:],
                                             lhsT=aggtb[:, f * 128:(f + 1) * 128],
                                             rhs=wallB[:, f * DP:(f + 1) * DP],
                                             start=(f == 0), stop=(f == 3))
                        resid = ew.tile([128, DP], F32, tag="resid")
                        nc.vector.scalar_tensor_tensor(
                            out=resid[:], in0=ops_[:], scalar=CDEG,
                            in1=x_t[:, b * DP:(b + 1) * DP], op0=OP.mult, op1=OP.add)
                        _ln_bn(nc, ew, resid, x_t, b, eps5)
                        # FF block
                        xtp2 = eps_o.tile([128, DP], F32, tag="ops")
                        for f in range(4):
                            nc.tensor.transpose(
                                out=xtp2[:, f * 128:(f + 1) * 128],
                                in_=x_t[:, b * DP + f * 128:b * DP + (f + 1) * 128],
                                identity=identf[:])
                        xtb2 = ew.tile([128, DP], BF, tag="xtb2")
                        nc.scalar.copy(out=xtb2[:], in_=xtp2[:])
                        htb = ew.tile([128, FF], BF, tag="htb")
                        for g2 in range(2):
                            f1a = eps_o.tile([128, DP], F32, tag="ops")
                            for f in range(4):
                                nc.tensor.matmul(
                                    f1a[:],
                                    lhsT=xtb2[:, f * 128:(f + 1) * 128],
                                    rhs=wallB[:, 2048 + f * FF + g2 * DP:
                                              2048 + f * FF + (g2 + 1) * DP],
                                    start=(f == 0), stop=(f == 3))
                            hb = ew.tile([128, DP], BF, tag="hb")
                            nc.scalar.activation(out=hb[:], in_=f1a[:], func=AF.Silu)
                            htp = epsT.tile([128, DP], BF, tag="aggtp")
                            for f in range(4):
                                nc.tensor.transpose(
                                    out=htp[:, f * 128:(f + 1) * 128],
                                    in_=hb[:, f * 128:(f + 1) * 128],
                                    identity=ident[:])
                            nc.vector.tensor_copy(out=htb[:, g2 * DP:(g2 + 1) * DP],
                                                  in_=htp[:])
                        f2p = eps_o.tile([128, DP], F32, tag="ops")
                        for f in range(8):
                            nc.tensor.matmul(f2p[:],
                                             lhsT=htb[:, f * 128:(f + 1) * 128],
                                             rhs=wallB[:, 6144 + f * DP:6144 + (f + 1) * DP],
                                             start=(f == 0), stop=(f == 7))
                        resid2 = ew.tile([128, DP], F32, tag="resid")
                        nc.vector.tensor_tensor(out=resid2[:], in0=f2p[:],
                                                in1=x_t[:, b * DP:(b + 1) * DP],
                                                op=OP.add)
                        _ln_bn(nc, ew, resid2, x_t, b, eps5)
                        if l + 1 < n_layers:
                            emit_kv_block(b, wallA_next, ew, eps_o)
                if l + 1 < n_layers:
                    nc.gpsimd.collective_compute(
                        "AllGather", OP.bypass, ins=[kvloc_d[:].opt()],
                        outs=[kvfull_d[:].opt()], replica_groups=RG)
                    wallA_cur = wallA_next
                if "x" in dumps and l == n_layers - 1:
                    nc.sync.dma_start(out=dumps["x"][:], in_=x_t[:])

            # ============ PHASE 5: readout ============
            with (
                tc.tile_pool(name="fw", bufs=3) as fw,
                tc.tile_pool(name="fps", bufs=1, space="PSUM") as fps,
                tc.tile_pool(name="fpsD", bufs=1, space="PSUM") as fpsD,
            ):
                Sg_t = fw.tile([128, NBLK * G], F32, tag="Sg")
                nc.sync.dma_start(
                    out=Sg_t[:].rearrange("p (b g)   -> p b g", g=G),
                    in_=dt["Sg"].ap().rearrange("(b p) g -> p b g", p=128))
                wh1 = fw.tile([128, 4 * DP], BF, tag="wh1")
                nc.sync.dma_start(
                    out=wh1[:].rearrange("p (a m) -> p a m", a=4),
                    in_=dt["Wh1"].ap().rearrange("(a p) m -> p a m", p=128))
                wh2 = fw.tile([128, 4 * 4], BF, tag="wh2")
                nc.sync.dma_start(
                    out=wh2[:].rearrange("p (a m) -> p a m", a=4),
                    in_=dt["Wh2"].ap().rearrange("(a p) m -> p a m", p=128))
                engps = fpsD.tile([64, 4], F32, tag="engps")
                for b in range(n_blocks):
                    xtp = fps.tile([128, DP], F32, tag="xtp")
                    for f in range(4):
                        nc.tensor.transpose(
                            out=xtp[:, f * 128:(f + 1) * 128],
                            in_=x_t[:, b * DP + f * 128:b * DP + (f + 1) * 128],
                            identity=identf[:])
                    xtb = fw.tile([128, DP], BF, tag="xtb")
                    nc.scalar.copy(out=xtb[:], in_=xtp[:])
                    h1p = fps.tile([128, DP], F32, tag="h1p")
                    for f in range(4):
                        nc.tensor.matmul(h1p[:], lhsT=xtb[:, f * 128:(f + 1) * 128],
                                         rhs=wh1[:, f * DP:(f + 1) * DP],
                                         start=(f == 0), stop=(f == 3))
                    h1b = fw.tile([128, DP], BF, tag="h1b")
                    nc.scalar.activation(out=h1b[:], in_=h1p[:], func=AF.Silu)
                    h1tp = fps.tile([128, DP], BF, tag="h1tp")
                    for f in range(4):
                        nc.tensor.transpose(out=h1tp[:, f * 128:(f + 1) * 128],
                                            in_=h1b[:, f * 128:(f + 1) * 128],
                                            identity=ident[:])
                    h1tb = fw.tile([128, DP], BF, tag="h1tb")
                    nc.scalar.copy(out=h1tb[:], in_=h1tp[:])
                    nep = fps.tile([128, 4], F32, tag="nep")
                    for f in range(4):
                        nc.tensor.matmul(nep[:], lhsT=h1tb[:, f * 128:(f + 1) * 128],
                                         rhs=wh2[:, f * 4:(f + 1) * 4],
                                         start=(f == 0), stop=(f == 3))
                    nef = fw.tile([128, 4], F32, tag="nef")
                    nc.scalar.copy(out=nef[:], in_=nep[:])
                    nc.tensor.matmul(engps[:], lhsT=Sg_t[:, b * G:(b + 1) * G],
                                     rhs=nef[:], start=(b == 0),
                                     stop=(b == n_blocks - 1))
                engsb = fw.tile([64, 1], F32, tag="engsb")
                nc.scalar.mul(out=engsb[:], in_=engps[:, 0:1], mul=1.0 / AVG_NODES)
                engt = fps.tile([64, 64], F32, tag="engt")
                nc.tensor.transpose(out=engt[0:1, 0:64], in_=engsb[:],
                                    identity=identf[0:64, 0:64])
                engrow = fw.tile([1, 64], F32, tag="engrow")
                nc.scalar.copy(out=engrow[:], in_=engt[0:1, 0:64])
                nc.sync.dma_start(out=eng_in_d[:], in_=engrow[:])
                nc.gpsimd.collective_compute(
                    "AllReduce", OP.add, ins=[eng_in_d[:].opt()],
                    outs=[eng_out_d[:].opt()], replica_groups=RG)
                nc.sync.dma_start(out=energy_out[:], in_=eng_out_d[:])

    return nc


def _ln_bn(nc, pool, resid, x_t, b, eps_t):
    """LayerNorm over resid[:, :D] -> x_t[:, b*DP : b*DP+D] via bn_stats."""
    st6 = pool.tile([128, 6], F32, tag="st6")
    nc.vector.bn_stats(out=st6[:], in_=resid[:, 0:D])
    mv = pool.tile([128, 2], F32, tag="mv")
    nc.vector.bn_aggr(out=mv[:], in_=st6[:])
    stdv = pool.tile([128, 1], F32, tag="stdv")
    nc.scalar.activation(out=stdv[:], in_=mv[:, 1:2], func=AF.Sqrt,
                         bias=eps_t[:])
    rstd = pool.tile([128, 1], F32, tag="rstd")
    nc.vector.reciprocal(out=rstd[:], in_=stdv[:])
    nc.vector.tensor_scalar(out=x_t[:, b * DP:b * DP + D], in0=resid[:, 0:D],
                            scalar1=mv[:, 0:1], scalar2=rstd[:],
                            op0=OP.subtract, op1=OP.mult)


# ---------------------------------------------------------------------------
# entry point
# ---------------------------------------------------------------------------

def kernel(**inputs):
    shared, per_core, CBLK = preprocess(inputs)
    in_maps, _ = make_inmaps(inputs, shared, per_core, CBLK)
    nc = build(CBLK)
    split_multi_waits(nc)
    res = run_bass_kernel_spmd(nc, in_maps, core_ids=list(range(NC)))
    return np.asarray(res.results[0]["energy"][0], np.float32).reshape(G)
